# revision 18
# baseline (speedup 1.0000x reference)
"""Trainium2 Bass kernel for DeformableMultiHeadedAttention.

Data-parallel over batch B=8 across 8 NeuronCores (one batch element per
core, identical programs, no collectives).

Per-core pipeline (matmuls bf16 with f32 accumulate):
  1. q,k,v [4096,512] f32 -> SWDGE cast-DMA -> DRAM bf16 -> batched HWDGE
     DMA-transpose (one [512,512] xbar transpose per chunk) -> feature-major
     XT [128,4,tok] chunks in SBUF.
  2. Projections on PE: K'T/Q'T feature-major (lhsT=W, rhs=XT); V' token-major
     (lhsT=XT tile, rhs=W), bv folded out on host (bo' = bo + 2*bv@Wo, LN
     input gets +bv on chip).
  3. Q pooling (AvgPool k=5, stride 1, zero pad) as 3 shifted adds; the 1/5
     is folded into the softmax exp scale.
  4. DSA (windows of 8 tokens): per 128-token tile, 8 heads: S_T[k,q] on PE,
     exp on ACT, block-diag mask mul on DVE, attn@V plus ones-col denominator
     sharing the lhsT, per-partition 1/den scale on DVE. Token-major DSA
     output -> DRAM (bf16).
  5. DRAM round-trips: batched DMA-transpose -> attn_xT feature-major;
     strided gather -> PV window-major [kw, (slot, head, hd)].
  6. Incremental (per 128-window group, overlapping phase 1 tail): win_tok
     +bv, LayerNorm moments via ones-matmuls, exact GELU, pq/pk projections.
  7. PSA restructured: raw exp-scores kept unnormalized; denominators via
     N=1 matmuls against a ones column (per-partition 1/den on DVE); pout
     window-major [wq, (slot,hd)] at M=128 (half the PE rows of the
     feature-major form); z = pout*recip + attn_x in window-major form;
     PE identity-transposes + strided ACT copies build feature-major zT.
  8. out = Z @ Wo + bo' with Z as the stationary operand -> token-major f32
     output, streamed per window-group.
"""

import sys
from contextlib import ExitStack

for _p in ("/opt/trn_rl_repo/concourse", "/opt/trn_rl_repo"):
    if _p not in sys.path:
        sys.path.insert(0, _p)

import numpy as np
import ml_dtypes

import concourse.bass as bass
import concourse.mybir as mybir
import concourse.tile as tile
from concourse import bacc
from concourse.tile import add_dep_helper
from concourse.bass_utils import run_bass_kernel_spmd

BF16 = mybir.dt.bfloat16
F32 = mybir.dt.float32
AF = mybir.ActivationFunctionType
ALU = mybir.AluOpType

B, M, D = 8, 4096, 512
H, HD = 8, 64
WIN = 7
PW = WIN + 1
QNB = 5
QLEN = 3584
WN = M // PW
SCALE = D ** -0.5
EPS = 1e-5
NCHUNK = 8
CH = 512
NG = 4                   # window groups of 128 for phase 2
GW = WN // NG            # 128 windows per group
PERM = [(h % 2) * 4 + h // 2 for h in range(H)]  # head -> DSA psum slot


def build_program():
    nc = bacc.Bacc("TRN2", target_bir_lowering=False, debug=False, num_devices=8)

    t = {}
    t["q_in"] = nc.dram_tensor("q", [M, D], F32, kind="ExternalInput")
    t["k_in"] = nc.dram_tensor("k", [M, D], F32, kind="ExternalInput")
    t["v_in"] = nc.dram_tensor("v", [M, D], F32, kind="ExternalInput")
    for nm in ("wq", "wk", "wv", "wpq", "wpk", "wo"):
        t[nm] = nc.dram_tensor(nm, [D, D], BF16, kind="ExternalInput")
    for nm in ("bq_c", "bk_c", "bpq_c", "bpk_c", "ln_g_c", "ln_b_c", "bv_c"):
        t[nm] = nc.dram_tensor(nm, [128, 4], F32, kind="ExternalInput")
    t["bo_r"] = nc.dram_tensor("bo_r", [1, D], BF16, kind="ExternalInput")
    t["bmask"] = nc.dram_tensor("bmask", [128, 128], BF16, kind="ExternalInput")
    t["ident"] = nc.dram_tensor("ident", [128, 128], BF16, kind="ExternalInput")
    t["out"] = nc.dram_tensor("out", [QLEN, D], F32, kind="ExternalOutput")
    t["axd"] = nc.dram_tensor("axd_s", [M, D], BF16, kind="Internal")
    t["qb"] = nc.dram_tensor("qb_s", [M, D], BF16, kind="Internal")
    t["kb"] = nc.dram_tensor("kb_s", [M, D], BF16, kind="Internal")
    t["vb"] = nc.dram_tensor("vb_s", [M, D], BF16, kind="Internal")

    with tile.TileContext(nc) as tc:
        _build(nc, tc, t)
    nc.compile()
    return nc


def _build(nc, tc, t):
    qb, kb, vb = t["qb"], t["kb"], t["vb"]
    axd, out = t["axd"], t["out"]

    with ExitStack() as octx:
        singles = octx.enter_context(tc.tile_pool(name="singles", bufs=1))

        # phase-1 weights first (needed by the first projections), then the
        # input casts, then everything else so the casts win the DMA engines.
        W = {}
        for nm in ("wq", "wk", "wv"):
            W[nm] = singles.tile([128, 4, D], BF16, tag=nm, name=f"w_{nm}")
            nc.scalar.dma_start(out=W[nm][:],
                                in_=t[nm].ap().rearrange("(c p) d -> p c d", p=128))

        cast_insts = {"q": [], "k": [], "v": []}

        def issue_casts(lo, hi):
            for nm, srcd, dst in (("q", t["q_in"], qb), ("k", t["k_in"], kb),
                                  ("v", t["v_in"], vb)):
                ci = nc.gpsimd.dma_start(
                    out=dst[lo * CH:hi * CH, :],
                    in_=srcd[lo * CH:hi * CH, :])
                cast_insts[nm].append(((lo, hi), ci))

        issue_casts(0, 1)
        issue_casts(1, 2)

        bias_cols = {}
        for nm in ("bq_c", "bk_c"):
            bias_cols[nm] = singles.tile([128, 4], F32, tag=nm, name=f"bc_{nm}")
            nc.scalar.dma_start(out=bias_cols[nm][:], in_=t[nm][:, :])
        mask_sb = singles.tile([128, 128], BF16)
        nc.scalar.dma_start(out=mask_sb[:], in_=t["bmask"][:, :])
        ones_row = singles.tile([1, 128], BF16)
        nc.vector.memset(ones_row[:], 1.0)
        ones_col = singles.tile([128, 1], BF16)
        nc.vector.memset(ones_col[:], 1.0)
        ones_full = singles.tile([128, 128], BF16)
        nc.vector.memset(ones_full[:], 1.0)
        eps_sb = singles.tile([128, 1], F32)
        nc.vector.memset(eps_sb[:], EPS)

        for c in range(2, NCHUNK):
            issue_casts(c, c + 1)

        axd_writers = {}
        p2a = octx.enter_context(tc.tile_pool(name="p2a", bufs=1))
        axt = p2a.tile([128, 4, M], BF16, tag="axt")

        # ================= phase 2 weights ==============================
        # deferred behind the early input casts so they don't hog the DMA
        # engines during the pipeline ramp
        gate = cast_insts["v"][2][1]
        for nm in ("wpq", "wpk", "wo"):
            W[nm] = singles.tile([128, 4, D], BF16, tag=nm, name=f"w_{nm}")
            di = nc.scalar.dma_start(
                out=W[nm][:], in_=t[nm].ap().rearrange("(c p) d -> p c d", p=128))
            add_dep_helper(di.ins, gate.ins, reason="defer p2 weight load")
        for nm in ("bpq_c", "bpk_c", "ln_g_c", "ln_b_c", "bv_c"):
            bias_cols[nm] = singles.tile([128, 4], F32, tag=nm, name=f"bc_{nm}")
            di = nc.scalar.dma_start(out=bias_cols[nm][:], in_=t[nm][:, :])
            add_dep_helper(di.ins, gate.ins, reason="defer p2 const load")
        bo_sb = singles.tile([1, D], BF16)
        di = nc.scalar.dma_start(out=bo_sb[:], in_=t["bo_r"][:, :])
        add_dep_helper(di.ins, gate.ins, reason="defer p2 const load")
        ident_sb = singles.tile([128, 128], BF16)
        di = nc.scalar.dma_start(out=ident_sb[:], in_=t["ident"][:, :])
        add_dep_helper(di.ins, gate.ins, reason="defer p2 const load")

        # ================= phase 1 =================
        with ExitStack() as ctx:
            p1 = ctx.enter_context(tc.tile_pool(name="p1", bufs=1))
            kT = p1.tile([128, 4, 3, CH], BF16, tag="kT")        # ring of 3 chunks
            qpT = p1.tile([128, 4, 3, CH], BF16, tag="qpT")      # ring of 3 chunks
            vtm = p1.tile([128, 12, 8, 65], BF16, tag="vtm")     # ring of 12 tiles, 65-col/head
            nc.vector.memset(vtm[:, :, :, 64:65], 1.0)           # ones col for denominators
            qraw = p1.tile([128, 4, M + 4], BF16, tag="qraw")    # full, padded +-2
            nc.vector.memset(qraw[:, :, 0:2], 0.0)
            nc.vector.memset(qraw[:, :, M + 2:M + 4], 0.0)

            xtp = ctx.enter_context(tc.tile_pool(name="xtp", bufs=2))
            ps_proj = ctx.enter_context(tc.tile_pool(name="ps_proj", bufs=2, space="PSUM"))
            ps_st = ctx.enter_context(tc.tile_pool(name="ps_st", bufs=2, space="PSUM"))
            ps_out = ctx.enter_context(tc.tile_pool(name="ps_out", bufs=1, space="PSUM"))
            dsa_sb = ctx.enter_context(tc.tile_pool(name="dsa_sb", bufs=2))
            pool_tmp = ctx.enter_context(tc.tile_pool(name="pool_tmp", bufs=2))
            ax_pool = ctx.enter_context(tc.tile_pool(name="ax_sb", bufs=2))

            def load_xt(nm, dram, c):
                xt = xtp.tile([128, 4, CH], BF16, tag=f"xt_{nm}", name=f"xt_{nm}_{c}")
                ti = nc.sync.dma_start(out=xt[:],
                                       in_=dram[c * CH:(c + 1) * CH, :],
                                       transpose=True)
                for (lo, hi), ci in cast_insts[nm]:
                    if lo <= c < hi:
                        add_dep_helper(ti.ins, ci.ins,
                                       reason="transpose reads cast output")
                return xt

            def proj_fm_group(xt, wname, bname, dst_fn, j):
                ps = ps_proj.tile([128, CH], F32, tag="proj",
                                  name=f"ps_{wname}_{j}")
                for dk in range(4):
                    nc.tensor.matmul(ps[:], W[wname][:, dk, j * 128:(j + 1) * 128],
                                     xt[:, dk, :], start=(dk == 0), stop=(dk == 3))
                nc.scalar.activation(dst_fn(j), ps[:], AF.Identity,
                                     bias=bias_cols[bname][:, j:j + 1], scale=1.0)

            def proj_v_group(xt, c, tt):
                ps = ps_proj.tile([128, D], F32, tag="proj", name=f"ps_v_{tt}")
                for dk in range(4):
                    nc.tensor.matmul(ps[:], xt[:, dk, tt * 128:(tt + 1) * 128],
                                     W["wv"][:, dk, :], start=(dk == 0),
                                     stop=(dk == 3), skip_group_check=True)
                nc.vector.tensor_copy(vtm[:, (c * 4 + tt) % 12, :, 0:64],
                                      ps[:].rearrange("p (h d) -> p h d", h=H))

            def pool_chunk(c):
                base = c * CH
                ta = pool_tmp.tile([128, 4, CH + 2], BF16, tag="ta")
                nc.vector.tensor_add(ta[:], qraw[:, :, base:base + CH + 2],
                                     qraw[:, :, base + 1:base + CH + 3])
                tb = pool_tmp.tile([128, 4, CH], BF16, tag="tb")
                nc.vector.tensor_add(tb[:], ta[:, :, 0:CH], ta[:, :, 2:CH + 2])
                nc.vector.tensor_add(qpT[:, :, c % 3, :], tb[:],
                                     qraw[:, :, base + 4:base + CH + 4])

            def dsa_scores(c, lt):
                """MM1 + exp + mask for tile lt of chunk c -> masked sbuf tile."""
                st = ps_st.tile([128, 8, 128], F32, tag="st", name=f"st_{c}_{lt}")
                for h in range(H):
                    hp = PERM[h]
                    base = (h % 2) * 64
                    lhsT = kT[base:base + 64, h // 2, c % 3, lt * 128:(lt + 1) * 128]
                    rhs = qpT[base:base + 64, h // 2, c % 3, lt * 128:(lt + 1) * 128]
                    nc.tensor.matmul(st[:, hp, :], lhsT, rhs, start=True, stop=True,
                                     skip_group_check=True)
                expS = dsa_sb.tile([128, 8, 128], BF16, tag="expS",
                                   name=f"expS_{c}_{lt}")
                nc.scalar.activation(expS[:], st[:], AF.Exp, scale=SCALE / QNB)
                masked = dsa_sb.tile([128, 8, 128], BF16, tag="masked",
                                     name=f"masked_{c}_{lt}")
                nc.vector.tensor_mul(masked[:], expS[:],
                                     mask_sb[:].unsqueeze(1).to_broadcast((128, 8, 128)))
                return masked

            def dsa_out(c, lt, masked, ax_out):
                """attn@V with ones-col denominators, then normalize."""
                outp = ps_out.tile([128, 2, 512], F32, tag="outp",
                                   name=f"outp_{c}_{lt}")
                for h in range(H):
                    hp = PERM[h]
                    nc.tensor.matmul(outp[:, h // 4, (h % 4) * 65:(h % 4) * 65 + 65],
                                     masked[:, hp, :],
                                     vtm[:, (c * 4 + lt) % 12, h, :],
                                     start=True, stop=True, skip_group_check=True)
                recip = dsa_sb.tile([128, 2, 4], F32, tag="recip",
                                    name=f"recip_{c}_{lt}")
                den_view = bass.AP(outp.tensor, outp[:].offset + 64,
                                   [outp[:].ap[0], [512, 2], [65, 4]])
                nc.vector.reciprocal(recip[:], den_view)
                av_view = bass.AP(outp.tensor, outp[:].offset,
                                  [outp[:].ap[0], [512, 2], [65, 4], [1, 64]])
                nc.vector.tensor_mul(
                    ax_out.rearrange("p (a b d) -> p a b d", a=2, b=4),
                    av_view,
                    recip[:].unsqueeze(3).to_broadcast((128, 2, 4, 64)))

            def dsa_group_list(c, ax):
                masked = {}

                def out_and_store(lt):
                    dsa_out(c, lt, masked.pop(lt), ax[:, lt, :])
                    store_ax(c, lt, ax)

                g = []
                g.append(lambda: masked.__setitem__(0, dsa_scores(c, 0)))
                g.append(lambda: masked.__setitem__(1, dsa_scores(c, 1)))
                g.append(lambda: out_and_store(0))
                g.append(lambda: masked.__setitem__(2, dsa_scores(c, 2)))
                g.append(lambda: out_and_store(1))
                g.append(lambda: masked.__setitem__(3, dsa_scores(c, 3)))
                g.append(lambda: out_and_store(2))
                g.append(lambda: out_and_store(3))
                return g

            def store_ax(c, lt, ax):
                wi = nc.gpsimd.dma_start(
                    out=axd[c * CH + lt * 128:c * CH + (lt + 1) * 128, :],
                    in_=ax[:, lt, :])
                axd_writers[(c, lt)] = wi

            for c in range(NCHUNK + 2):
                pgroups = []
                if c < NCHUNK:
                    qxt = load_xt("q", qb, c)
                    kxt = load_xt("k", kb, c)
                    vxt = load_xt("v", vb, c)
                    for j in range(4):
                        pgroups.append(lambda j=j, x=qxt, c=c: proj_fm_group(
                            x, "wq", "bq_c",
                            lambda jj, c=c: qraw[:, jj, 2 + c * CH:2 + (c + 1) * CH], j))
                    for j in range(4):
                        pgroups.append(lambda j=j, x=kxt, c=c: proj_fm_group(
                            x, "wk", "bk_c", lambda jj, c=c: kT[:, jj, c % 3, :], j))
                    for tt in range(4):
                        pgroups.append(lambda tt=tt, x=vxt, c=c: proj_v_group(x, c, tt))
                dgroups = []
                ax = None
                if c >= 2:
                    ax = ax_pool.tile([128, 4, D], BF16, tag="ax", name=f"ax_{c - 2}")
                    dgroups = dsa_group_list(c - 2, ax)
                # weave: spread D groups evenly through the P stream;
                # pool(c-1) after the 4 Q-projection groups
                npg, ndg = len(pgroups), len(dgroups)
                dpos = {int(round((k + 1) * npg / (ndg + 1))): k for k in range(ndg)} \
                    if npg else {}
                for i in range(max(npg, 1)):
                    if i < npg:
                        pgroups[i]()
                    if i == 3 and 1 <= c <= NCHUNK:
                        pool_chunk(c - 1)
                    if i in dpos:
                        dgroups[dpos[i]]()
                if not pgroups:
                    if 1 <= c <= NCHUNK:
                        pool_chunk(c - 1)
                    for g in dgroups:
                        g()


        # ================= phase 2 =================
        with ExitStack() as ctx:
            p2 = ctx.enter_context(tc.tile_pool(name="p2", bufs=1))

            pv = p2.tile([128, 4, WIN, D], BF16, tag="pv")
            wtn = p2.tile([128, 4, WN], BF16, tag="wtn")
            pqT = p2.tile([128, 4, WN], BF16, tag="pqT")
            pkT = p2.tile([128, 4, WN], BF16, tag="pkT")
            esA = p2.tile([128, H, 4, WN], BF16, tag="esA")
            zt = p2.tile([128, 4, QLEN], BF16, tag="zt")
            recip_sb = p2.tile([128, H, 4], F32, tag="recips")

            # issue axt transposes + pv gathers in dependency-arrival order
            srcv = axd.ap().rearrange("(cc p w) d -> cc p w d", p=128, w=PW)
            for c in range(NCHUNK):
                for lt in range(4):
                    r0 = c * CH + lt * 128
                    ti = nc.sync.dma_start(
                        out=axt[:, :, r0:r0 + 128],
                        in_=axd[r0:r0 + 128, :],
                        transpose=True)
                    add_dep_helper(ti.ins, axd_writers[(c, lt)].ins,
                                   reason="axt transpose reads axd tile")
                if c % 2 == 1:
                    cc = c // 2
                    gi = nc.sync.dma_start(out=pv[:, cc, :, :], in_=srcv[cc, :, 1:PW, :])
                    for c2 in (2 * cc, 2 * cc + 1):
                        for lt in range(4):
                            add_dep_helper(gi.ins, axd_writers[(c2, lt)].ins,
                                           reason="pv gather")

            # ---- win_tok (+bv) LN + GELU + pq/pk, per 128-window group ----
            with ExitStack() as lctx:
                ps_ln = lctx.enter_context(
                    tc.tile_pool(name="ps_ln", bufs=3, space="PSUM"))
                lnp = lctx.enter_context(tc.tile_pool(name="lnp", bufs=2))

                lnA = {}

                def ln_phase_a(g):
                    """Moments + rstd; ACT funcs all within one table set
                    (Identity/Square/Copy/Sqrt)."""
                    wt_g = axt[:, :, g * GW * PW:(g + 1) * GW * PW:PW]
                    wtb = lnp.tile([128, 4, GW], BF16, tag="wtb", bufs=4,
                                   name=f"wtb_{g}")
                    for j in range(4):
                        nc.scalar.activation(wtb[:, j, :], wt_g[:, j, :],
                                             AF.Identity,
                                             bias=bias_cols["bv_c"][:, j:j + 1],
                                             scale=1.0)
                    wsq = lnp.tile([128, 4, GW], BF16, tag="wsq", name=f"wsq_{g}")
                    nc.scalar.activation(wsq[:], wtb[:], AF.Square)
                    ps_mu = ps_ln.tile([128, GW], F32, tag="psln", name=f"psmu_{g}")
                    ps_var = ps_ln.tile([128, GW], F32, tag="psln", name=f"psvar_{g}")
                    for j in range(4):
                        nc.tensor.matmul(ps_mu[:], ones_full[:], wtb[:, j, :],
                                         start=(j == 0), stop=(j == 3),
                                         skip_group_check=True)
                        nc.tensor.matmul(ps_var[:], ones_full[:], wsq[:, j, :],
                                         start=(j == 0), stop=(j == 3),
                                         skip_group_check=True)
                    mu = lnp.tile([128, GW], F32, tag="mu", bufs=4, name=f"mu_{g}")
                    nc.scalar.mul(mu[:], ps_mu[:], 1.0 / D)
                    ex2 = lnp.tile([128, GW], F32, tag="ex2", bufs=1,
                                   name=f"ex2_{g}")
                    nc.scalar.mul(ex2[:], ps_var[:], 1.0 / D)
                    var = lnp.tile([128, GW], F32, tag="var", bufs=1,
                                   name=f"var_{g}")
                    nc.vector.tensor_mul(var[:], mu[:], mu[:])
                    nc.vector.tensor_sub(var[:], ex2[:], var[:])
                    sd = lnp.tile([128, GW], F32, tag="sd", bufs=1, name=f"sd_{g}")
                    nc.scalar.activation(sd[:], var[:], AF.Sqrt, bias=eps_sb[:])
                    rstd = lnp.tile([128, GW], F32, tag="rstd", bufs=4,
                                    name=f"rstd_{g}")
                    nc.vector.reciprocal(rstd[:], sd[:])
                    lnA[g] = (wtb, mu, rstd)

                def ln_phase_b(g):
                    """GELU + pq/pk projections (Gelu/Identity table set)."""
                    gs = g * GW
                    wtb, mu, rstd = lnA.pop(g)
                    for j in range(4):
                        tmp = lnp.tile([128, GW], F32, tag="lnt", name=f"lnt_{g}_{j}")
                        nc.vector.tensor_sub(tmp[:], wtb[:, j, :], mu[:])
                        nc.vector.tensor_mul(tmp[:], tmp[:], rstd[:])
                        nc.scalar.activation(wtn[:, j, gs:gs + GW], tmp[:],
                                             AF.Gelu,
                                             bias=bias_cols["ln_b_c"][:, j:j + 1],
                                             scale=bias_cols["ln_g_c"][:, j:j + 1])
                    for dst, wname, bname in ((pqT, "wpq", "bpq_c"),
                                              (pkT, "wpk", "bpk_c")):
                        for j in range(4):
                            ps = ps_ln.tile([128, GW], F32, tag="psln",
                                            name=f"pp_{wname}_{g}_{j}")
                            for dk in range(4):
                                nc.tensor.matmul(
                                    ps[:], W[wname][:, dk, j * 128:(j + 1) * 128],
                                    wtn[:, dk, gs:gs + GW],
                                    start=(dk == 0), stop=(dk == 3))
                            nc.scalar.activation(dst[:, j, gs:gs + GW], ps[:],
                                                 AF.Identity,
                                                 bias=bias_cols[bname][:, j:j + 1],
                                                 scale=1.0)

                # A0..A2 then B0..B2 (one Sqrt->Gelu table switch), then the
                # last group's A3+B3 pair on the critical path (one more
                # switch pair).
                for g in range(NG - 1):
                    ln_phase_a(g)
                for g in range(NG - 1):
                    ln_phase_b(g)
                ln_phase_a(NG - 1)
                ln_phase_b(NG - 1)

            # ---- PSA: raw exp scores; den via N=1 matmuls; window-major pout
            with ExitStack() as pctx:
                # PSUM budget (8 banks): es/fin share slots (disjoint
                # lifetimes, same shape) 2 + po 2 + ztps 2 + den 1 = 7.
                ps_es = pctx.enter_context(
                    tc.tile_pool(name="ps_es", bufs=2, space="PSUM"))
                ps_po = pctx.enter_context(
                    tc.tile_pool(name="ps_po", bufs=2, space="PSUM"))
                ps_ztden = pctx.enter_context(
                    tc.tile_pool(name="ps_ztden", bufs=2, space="PSUM"))
                ps_fin = ps_es
                zwp = pctx.enter_context(tc.tile_pool(name="zwp", bufs=2))
                ztp = pctx.enter_context(tc.tile_pool(name="ztp", bufs=2))
                osb = pctx.enter_context(tc.tile_pool(name="osb", bufs=3))

                def psa_scores(h):
                    base = (h % 2) * 64
                    for cc in range(4):
                        ps = ps_es.tile([128, WN], F32, tag="es",
                                    name=f"es_{h}_{cc}")
                        nc.tensor.matmul(
                            ps[:], pkT[base:base + 64, h // 2,
                                       cc * 128:(cc + 1) * 128],
                            pqT[base:base + 64, h // 2, :], start=True, stop=True)
                        nc.scalar.activation(esA[:, h, cc, :], ps[:], AF.Exp,
                                             scale=SCALE)

                den_ps = None

                def psa_den(h):
                    for qt in range(4):
                        idx = h * 4 + qt
                        for cc in range(4):
                            nc.tensor.matmul(
                                den_ps[:, idx:idx + 1],
                                esA[:, h, cc, qt * 128:(qt + 1) * 128],
                                ones_col[:], start=(cc == 0), stop=(cc == 3),
                                skip_group_check=True)
                    nc.vector.reciprocal(recip_sb[:, h, :],
                                         den_ps[:, h * 4:(h + 1) * 4])

                def pout_one(h, qt, zwin):
                    po = ps_po.tile([128, WIN, HD], F32, tag="po",
                                    name=f"po_{h}_{qt}")
                    for cc in range(4):
                        nc.tensor.matmul(
                            po[:], esA[:, h, cc, qt * 128:(qt + 1) * 128],
                            pv[:, cc, :, h * 64:(h + 1) * 64],
                            start=(cc == 0), stop=(cc == 3),
                            skip_group_check=True)
                    ztmp = ztp.tile([128, WIN, HD], BF16, tag="ztmp",
                                    name=f"ztmp_{h}_{qt}")
                    nc.vector.tensor_scalar_mul(ztmp[:], po[:],
                                                recip_sb[:, h, qt:qt + 1])
                    nc.vector.tensor_add(zwin[:, :, h * 64:(h + 1) * 64], ztmp[:],
                                         pv[:, qt, :, h * 64:(h + 1) * 64])

                def ztrans_one(qt, i, zwin):
                    zt_ps = ps_ztden.tile([128, 4, 128], BF16, tag="ztps",
                                          name=f"ztps_{qt}_{i}")
                    for fg in range(4):
                        nc.tensor.transpose(zt_ps[:, fg, :],
                                            zwin[:, i, fg * 128:(fg + 1) * 128],
                                            ident_sb[:])
                    nc.vector.tensor_copy(
                        zt[:, :, qt * GW * WIN + i:(qt + 1) * GW * WIN:WIN],
                        zt_ps[:])

                osb_tiles = {}

                def fin_one(tt):
                    ps = ps_fin.tile([128, D], F32, tag="es", name=f"fin_{tt}")
                    for dk in range(4):
                        nc.tensor.matmul(ps[:], zt[:, dk, tt * 128:(tt + 1) * 128],
                                         W["wo"][:, dk, :], start=(dk == 0),
                                         stop=False, skip_group_check=True)
                    nc.tensor.matmul(ps[:], ones_row[:], bo_sb[:], start=False,
                                     stop=True, skip_group_check=True)
                    g = tt // 2
                    if tt % 2 == 0:
                        osb_tiles[g] = osb.tile([128, 2, D], F32, tag="osb",
                                                name=f"osb_{g}")
                    nc.vector.tensor_copy(osb_tiles[g][:, tt % 2, :], ps[:])
                    if tt % 2 == 1:
                        outv = out.ap().rearrange("(g tt p) d -> g p tt d",
                                                  tt=2, p=128)
                        nc.sync.dma_start(out=outv[g], in_=osb_tiles.pop(g)[:])

                # weave: exp-scores pipeline, then per-qt pout with previous
                # group's transposes + final projections interleaved.
                for h in range(H):
                    psa_scores(h)
                den_ps = ps_ztden.tile([128, H * 4], F32, tag="den", name="den",
                                       bufs=1)
                for h in range(H):
                    psa_den(h)

                zw = {}
                prev = None

                def tail_items(qt):
                    items = []
                    zwin_p = zw[qt]
                    for i in range(WIN):
                        items.append(lambda i=i: ztrans_one(qt, i, zwin_p))
                    for j in range(WIN):
                        items.append(lambda j=j: fin_one(qt * WIN + j))
                    return items

                for qt in range(NG):
                    zw[qt] = zwp.tile([128, WIN, D], BF16, tag="zwin",
                                      name=f"zwin_{qt}")
                    titems = tail_items(prev) if prev is not None else []
                    ti = 0
                    for h in range(H):
                        pout_one(h, qt, zw[qt])
                        # spread up to 2 tail items of the previous group
                        for _ in range(2):
                            if ti < len(titems) and (h * 14) // H >= ti:
                                titems[ti]()
                                ti += 1
                    while ti < len(titems):
                        titems[ti]()
                        ti += 1
                    if prev is not None:
                        zw.pop(prev)
                    prev = qt
                for it in tail_items(prev):
                    it()


_NC_CACHE = None


def _get_program():
    global _NC_CACHE
    if _NC_CACHE is None:
        _NC_CACHE = build_program()
    return _NC_CACHE


def _host_consts(Wk, bk, Wv, bv, Wq, bq, ln_g, ln_b, Wpq, bpq, Wpk, bpk, Wo, bo):
    bf = ml_dtypes.bfloat16
    col = lambda b: np.asarray(b, np.float32).reshape(4, 128).T.copy()
    bo2 = np.asarray(bo, np.float32) + 2.0 * (
        np.asarray(bv, np.float32) @ np.asarray(Wo, np.float32))
    consts = {
        "wq": np.asarray(Wq, np.float32).astype(bf),
        "wk": np.asarray(Wk, np.float32).astype(bf),
        "wv": np.asarray(Wv, np.float32).astype(bf),
        "wpq": np.asarray(Wpq, np.float32).astype(bf),
        "wpk": np.asarray(Wpk, np.float32).astype(bf),
        "wo": np.asarray(Wo, np.float32).astype(bf),
        "bq_c": col(bq), "bk_c": col(bk),
        "bpq_c": col(bpq), "bpk_c": col(bpk),
        "ln_g_c": col(ln_g), "ln_b_c": col(ln_b),
        "bv_c": col(bv),
        "bo_r": bo2.reshape(1, D).astype(bf),
        "ident": np.eye(128, dtype=np.float32).astype(bf),
    }
    m = np.zeros((128, 128), np.float32)
    for g in range(16):
        m[g * PW:(g + 1) * PW, g * PW:(g + 1) * PW] = 1.0
    consts["bmask"] = m.astype(bf)
    return consts


def kernel(k, v, q, query_len, Wk, bk, Wv, bv, Wq, bq, ln_g, ln_b,
           Wpq, bpq, Wpk, bpk, Wo, bo):
    nc = _get_program()
    consts = _host_consts(Wk, bk, Wv, bv, Wq, bq, ln_g, ln_b,
                          Wpq, bpq, Wpk, bpk, Wo, bo)
    k = np.asarray(k, np.float32)
    v = np.asarray(v, np.float32)
    q = np.asarray(q, np.float32)
    in_maps = []
    for b in range(B):
        m = {"q": np.ascontiguousarray(q[b]), "k": np.ascontiguousarray(k[b]),
             "v": np.ascontiguousarray(v[b])}
        m.update(consts)
        in_maps.append(m)
    res = run_bass_kernel_spmd(nc, in_maps, core_ids=list(range(B)))
    return np.stack([res.results[b]["out"] for b in range(B)], axis=0)


if __name__ == "__main__":
    nc = build_program()
    print("program built ok")


# revision 33
# speedup vs baseline: 1.4902x; 1.4902x over previous
"""Trainium2 Bass kernel for DeformableMultiHeadedAttention.

Data-parallel over batch B=8 across 8 NeuronCores (one batch element per
core, identical programs, no collectives).

Per-core pipeline (matmuls bf16 with f32 accumulate):
  1. q,k,v [4096,512] f32 -> SWDGE cast-DMA -> DRAM bf16 -> batched HWDGE
     DMA-transpose (one [512,512] xbar transpose per chunk) -> feature-major
     XT [128,4,tok] chunks in SBUF.
  2. Projections on PE: K'T/Q'T feature-major (lhsT=W, rhs=XT); V' token-major
     (lhsT=XT tile, rhs=W), bv folded out on host (bo' = bo + 2*bv@Wo, LN
     input gets +bv on chip).
  3. Q pooling (AvgPool k=5, stride 1, zero pad) as 3 shifted adds; the 1/5
     is folded into the softmax exp scale.
  4. DSA (windows of 8 tokens): per 128-token tile, 8 heads: S_T[k,q] on PE,
     exp on ACT, block-diag mask mul on DVE, attn@V plus ones-col denominator
     sharing the lhsT, per-partition 1/den scale on DVE. Token-major DSA
     output -> DRAM (bf16).
  5. DRAM round-trips: batched DMA-transpose -> attn_xT feature-major;
     strided gather -> PV window-major [kw, (slot, head, hd)].
  6. Incremental (per 128-window group, overlapping phase 1 tail): win_tok
     +bv, LayerNorm moments via ones-matmuls, exact GELU, pq/pk projections.
  7. PSA restructured: raw exp-scores kept unnormalized; denominators via
     N=1 matmuls against a ones column (per-partition 1/den on DVE); pout
     window-major [wq, (slot,hd)] at M=128 (half the PE rows of the
     feature-major form); z = pout*recip + attn_x in window-major form;
     PE identity-transposes + strided ACT copies build feature-major zT.
  8. out = Z @ Wo + bo' with Z as the stationary operand -> token-major f32
     output, streamed per window-group.
"""

import sys
from contextlib import ExitStack

for _p in ("/opt/trn_rl_repo/concourse", "/opt/trn_rl_repo"):
    if _p not in sys.path:
        sys.path.insert(0, _p)

import numpy as np
import ml_dtypes

import concourse.bass as bass
import concourse.mybir as mybir
import concourse.tile as tile
from concourse import bacc
from concourse.tile import add_dep_helper
from concourse.bass_utils import run_bass_kernel_spmd

BF16 = mybir.dt.bfloat16
F32 = mybir.dt.float32
AF = mybir.ActivationFunctionType
ALU = mybir.AluOpType

B, M, D = 8, 4096, 512
H, HD = 8, 64
WIN = 7
PW = WIN + 1
QNB = 5
QLEN = 3584
WN = M // PW
SCALE = D ** -0.5
EPS = 1e-5
NCHUNK = 8
CH = 512
NG = 4                   # window groups of 128 for phase 2
GW = WN // NG            # 128 windows per group
PERM = [(h % 2) * 4 + h // 2 for h in range(H)]  # head -> DSA psum slot


def build_program():
    nc = bacc.Bacc("TRN2", target_bir_lowering=False, debug=False, num_devices=8)

    t = {}
    t["q_in"] = nc.dram_tensor("q", [M, D], F32, kind="ExternalInput")
    t["k_in"] = nc.dram_tensor("k", [M, D], F32, kind="ExternalInput")
    t["v_in"] = nc.dram_tensor("v", [M, D], F32, kind="ExternalInput")
    for nm in ("wq", "wk", "wv", "wpq", "wpk", "wo"):
        t[nm] = nc.dram_tensor(nm, [D, D], BF16, kind="ExternalInput")
    for nm in ("bq_c", "bk_c", "bpq_c", "bpk_c", "ln_g_c", "ln_b_c", "bv_c"):
        t[nm] = nc.dram_tensor(nm, [128, 4], F32, kind="ExternalInput")
    t["bo_r"] = nc.dram_tensor("bo_r", [1, D], BF16, kind="ExternalInput")
    t["bmask"] = nc.dram_tensor("bmask", [128, 128], BF16, kind="ExternalInput")
    t["ident"] = nc.dram_tensor("ident", [128, 128], BF16, kind="ExternalInput")
    t["out"] = nc.dram_tensor("out", [QLEN, D], F32, kind="ExternalOutput")
    t["axd"] = nc.dram_tensor("axd_s", [M, D], BF16, kind="Internal")
    t["zd"] = nc.dram_tensor("zd_s", [QLEN, D], BF16, kind="Internal")
    t["qb"] = nc.dram_tensor("qb_s", [M, D], BF16, kind="Internal")
    t["kb"] = nc.dram_tensor("kb_s", [M, D], BF16, kind="Internal")
    t["vb"] = nc.dram_tensor("vb_s", [M, D], BF16, kind="Internal")

    with tile.TileContext(nc) as tc:
        _build(nc, tc, t)
    nc.compile()
    return nc


def _build(nc, tc, t):
    qb, kb, vb = t["qb"], t["kb"], t["vb"]
    axd, out = t["axd"], t["out"]
    zd = t["zd"]

    with ExitStack() as octx:
        singles = octx.enter_context(tc.tile_pool(name="singles", bufs=1))

        # phase-1 weights first (needed by the first projections), then the
        # input casts, then everything else so the casts win the DMA engines.
        W = {}
        for nm in ("wq", "wk", "wv"):
            W[nm] = singles.tile([128, 4, D], BF16, tag=nm, name=f"w_{nm}")
            nc.scalar.dma_start(out=W[nm][:],
                                in_=t[nm].ap().rearrange("(c p) d -> p c d", p=128))

        cast_insts = {"q": [], "k": [], "v": []}

        def issue_casts(lo, hi):
            for nm, srcd, dst in (("q", t["q_in"], qb), ("k", t["k_in"], kb),
                                  ("v", t["v_in"], vb)):
                ci = nc.gpsimd.dma_start(
                    out=dst[lo * CH:hi * CH, :],
                    in_=srcd[lo * CH:hi * CH, :])
                cast_insts[nm].append(((lo, hi), ci))

        issue_casts(0, 1)
        issue_casts(1, 2)

        bias_cols = {}
        for nm in ("bq_c", "bk_c"):
            bias_cols[nm] = singles.tile([128, 4], F32, tag=nm, name=f"bc_{nm}")
            nc.scalar.dma_start(out=bias_cols[nm][:], in_=t[nm][:, :])
        mask_sb = singles.tile([128, 128], BF16)
        nc.scalar.dma_start(out=mask_sb[:], in_=t["bmask"][:, :])
        ones_row = singles.tile([1, 128], BF16)
        nc.vector.memset(ones_row[:], 1.0)
        ones_col = singles.tile([128, 1], BF16)
        nc.vector.memset(ones_col[:], 1.0)
        ones_full = singles.tile([128, 128], BF16)
        nc.vector.memset(ones_full[:], 1.0)
        eps_sb = singles.tile([128, 1], F32)
        nc.vector.memset(eps_sb[:], EPS)

        issue_casts(2, 4)
        issue_casts(4, 6)
        issue_casts(6, 8)

        axd_writers = {}
        p2a = octx.enter_context(tc.tile_pool(name="p2a", bufs=1))
        axt = p2a.tile([128, 4, M], BF16, tag="axt")

        # ================= phase 2 weights ==============================
        # deferred behind the early input casts so they don't hog the DMA
        # engines during the pipeline ramp
        # Allocated here; DMAs issued mid-phase-1 (see chunk loop, c==3)
        # on the sync queue so they neither hog the DMA engines at startup
        # nor get scheduled into the phase boundary.
        for nm in ("wpq", "wpk", "wo"):
            W[nm] = singles.tile([128, 4, D], BF16, tag=nm, name=f"w_{nm}")
        for nm in ("bpq_c", "bpk_c", "ln_g_c", "ln_b_c", "bv_c"):
            bias_cols[nm] = singles.tile([128, 4], F32, tag=nm, name=f"bc_{nm}")
        bo_sb = singles.tile([1, D], BF16)
        ident_sb = singles.tile([128, 128], BF16)

        def load_p2_consts(gate):
            dis = []
            for nm in ("wpq", "wpk", "wo"):
                dis.append(nc.sync.dma_start(
                    out=W[nm][:],
                    in_=t[nm].ap().rearrange("(c p) d -> p c d", p=128)))
            for nm in ("bpq_c", "bpk_c", "ln_g_c", "ln_b_c", "bv_c"):
                dis.append(nc.sync.dma_start(out=bias_cols[nm][:], in_=t[nm][:, :]))
            dis.append(nc.sync.dma_start(out=bo_sb[:], in_=t["bo_r"][:, :]))
            dis.append(nc.sync.dma_start(out=ident_sb[:], in_=t["ident"][:, :]))
            for di in dis:
                add_dep_helper(di.ins, gate.ins,
                               reason="const loads after startup transposes")

        # ================= phase 1 =================
        with ExitStack() as ctx:
            p1 = ctx.enter_context(tc.tile_pool(name="p1", bufs=1))
            kT = p1.tile([128, 4, 3, CH], BF16, tag="kT")        # ring of 3 chunks
            qpT = p1.tile([128, 4, 3, CH], BF16, tag="qpT")      # ring of 3 chunks
            vtm = p1.tile([128, 12, 8, 65], BF16, tag="vtm")     # ring of 12 tiles, 65-col/head
            nc.vector.memset(vtm[:, :, :, 64:65], 1.0)           # ones col for denominators
            qraw = p1.tile([128, 4, M + 4], BF16, tag="qraw")    # full, padded +-2
            nc.vector.memset(qraw[:, :, 0:2], 0.0)
            nc.vector.memset(qraw[:, :, M + 2:M + 4], 0.0)

            xtp = ctx.enter_context(tc.tile_pool(name="xtp", bufs=2))
            ps_proj = ctx.enter_context(tc.tile_pool(name="ps_proj", bufs=2, space="PSUM"))
            ps_st = ctx.enter_context(tc.tile_pool(name="ps_st", bufs=2, space="PSUM"))
            ps_out = ctx.enter_context(tc.tile_pool(name="ps_out", bufs=1, space="PSUM"))
            dsa_sb = ctx.enter_context(tc.tile_pool(name="dsa_sb", bufs=2))
            pool_tmp = ctx.enter_context(tc.tile_pool(name="pool_tmp", bufs=2))
            ax_pool = ctx.enter_context(tc.tile_pool(name="ax_sb", bufs=2))

            xt_gate = {}

            def load_xt(nm, dram, c):
                xt = xtp.tile([128, 4, CH], BF16, tag=f"xt_{nm}", name=f"xt_{nm}_{c}")
                ti = nc.sync.dma_start(out=xt[:],
                                       in_=dram[c * CH:(c + 1) * CH, :],
                                       transpose=True)
                for (lo, hi), ci in cast_insts[nm]:
                    if lo <= c < hi:
                        add_dep_helper(ti.ins, ci.ins,
                                       reason="transpose reads cast output")
                xt_gate[(nm, c)] = ti
                return xt

            def proj_fm_group(xt, wname, bname, dst_fn, j):
                ps = ps_proj.tile([128, CH], F32, tag="proj",
                                  name=f"ps_{wname}_{j}")
                for dk in range(4):
                    nc.tensor.matmul(ps[:], W[wname][:, dk, j * 128:(j + 1) * 128],
                                     xt[:, dk, :], start=(dk == 0), stop=(dk == 3))
                nc.scalar.activation(dst_fn(j), ps[:], AF.Identity,
                                     bias=bias_cols[bname][:, j:j + 1], scale=1.0)

            def proj_v_group(xt, c, tt):
                ps = ps_proj.tile([128, D], F32, tag="proj", name=f"ps_v_{tt}")
                for dk in range(4):
                    nc.tensor.matmul(ps[:], xt[:, dk, tt * 128:(tt + 1) * 128],
                                     W["wv"][:, dk, :], start=(dk == 0),
                                     stop=(dk == 3), skip_group_check=True)
                nc.scalar.copy(vtm[:, (c * 4 + tt) % 12, :, 0:64],
                               ps[:].rearrange("p (h d) -> p h d", h=H))

            def pool_chunk(c):
                base = c * CH
                ta = pool_tmp.tile([128, 4, CH + 2], BF16, tag="ta")
                nc.vector.tensor_add(ta[:], qraw[:, :, base:base + CH + 2],
                                     qraw[:, :, base + 1:base + CH + 3])
                tb = pool_tmp.tile([128, 4, CH], BF16, tag="tb")
                nc.vector.tensor_add(tb[:], ta[:, :, 0:CH], ta[:, :, 2:CH + 2])
                nc.vector.tensor_add(qpT[:, :, c % 3, :], tb[:],
                                     qraw[:, :, base + 4:base + CH + 4])

            def dsa_scores(c, lt):
                """MM1 + exp + mask for tile lt of chunk c -> masked sbuf tile."""
                st = ps_st.tile([128, 8, 128], F32, tag="st", name=f"st_{c}_{lt}")
                for h in range(H):
                    hp = PERM[h]
                    base = (h % 2) * 64
                    lhsT = kT[base:base + 64, h // 2, c % 3, lt * 128:(lt + 1) * 128]
                    rhs = qpT[base:base + 64, h // 2, c % 3, lt * 128:(lt + 1) * 128]
                    nc.tensor.matmul(st[:, hp, :], lhsT, rhs, start=True, stop=True,
                                     skip_group_check=True)
                expS = dsa_sb.tile([128, 8, 128], BF16, tag="expS",
                                   name=f"expS_{c}_{lt}")
                nc.scalar.activation(expS[:], st[:], AF.Exp, scale=SCALE / QNB)
                masked = dsa_sb.tile([128, 8, 128], BF16, tag="masked",
                                     name=f"masked_{c}_{lt}")
                nc.vector.tensor_mul(masked[:], expS[:],
                                     mask_sb[:].unsqueeze(1).to_broadcast((128, 8, 128)))
                return masked

            def dsa_out(c, lt, masked, ax_out):
                """attn@V with ones-col denominators, then normalize."""
                outp = ps_out.tile([128, 2, 512], F32, tag="outp",
                                   name=f"outp_{c}_{lt}")
                for h in range(H):
                    hp = PERM[h]
                    nc.tensor.matmul(outp[:, h // 4, (h % 4) * 65:(h % 4) * 65 + 65],
                                     masked[:, hp, :],
                                     vtm[:, (c * 4 + lt) % 12, h, :],
                                     start=True, stop=True, skip_group_check=True)
                recip = dsa_sb.tile([128, 2, 4], F32, tag="recip",
                                    name=f"recip_{c}_{lt}")
                den_view = bass.AP(outp.tensor, outp[:].offset + 64,
                                   [outp[:].ap[0], [512, 2], [65, 4]])
                nc.vector.reciprocal(recip[:], den_view)
                av_view = bass.AP(outp.tensor, outp[:].offset,
                                  [outp[:].ap[0], [512, 2], [65, 4], [1, 64]])
                nc.vector.tensor_mul(
                    ax_out.rearrange("p (a b d) -> p a b d", a=2, b=4),
                    av_view,
                    recip[:].unsqueeze(3).to_broadcast((128, 2, 4, 64)))

            def dsa_group_list(c, ax):
                masked = {}
                g = []
                g.append(lambda: masked.__setitem__(0, dsa_scores(c, 0)))
                g.append(lambda: masked.__setitem__(1, dsa_scores(c, 1)))
                g.append(lambda: dsa_out(c, 0, masked.pop(0), ax[:, 0, :]))
                g.append(lambda: masked.__setitem__(2, dsa_scores(c, 2)))
                g.append(lambda: dsa_out(c, 1, masked.pop(1), ax[:, 1, :]))
                g.append(lambda: masked.__setitem__(3, dsa_scores(c, 3)))
                g.append(lambda: dsa_out(c, 2, masked.pop(2), ax[:, 2, :]))
                g.append(lambda: dsa_out(c, 3, masked.pop(3), ax[:, 3, :]))
                return g

            def store_ax(c, ax):
                dst = axd.ap().rearrange("(cc lt p) d -> cc p lt d", lt=4, p=128)[c]
                wi = nc.gpsimd.dma_start(out=dst, in_=ax[:])
                axd_writers[c] = wi

            for c in range(NCHUNK + 2):
                pgroups = []
                if c < NCHUNK:
                    qxt = load_xt("q", qb, c)
                    kxt = load_xt("k", kb, c)
                    vxt = load_xt("v", vb, c)
                    if c == 3:
                        load_p2_consts(xt_gate[("q", 2)])
                    for j in range(4):
                        pgroups.append(lambda j=j, x=qxt, c=c: proj_fm_group(
                            x, "wq", "bq_c",
                            lambda jj, c=c: qraw[:, jj, 2 + c * CH:2 + (c + 1) * CH], j))
                    for j in range(4):
                        pgroups.append(lambda j=j, x=kxt, c=c: proj_fm_group(
                            x, "wk", "bk_c", lambda jj, c=c: kT[:, jj, c % 3, :], j))
                    for tt in range(4):
                        pgroups.append(lambda tt=tt, x=vxt, c=c: proj_v_group(x, c, tt))
                dgroups = []
                ax = None
                if c >= 2:
                    ax = ax_pool.tile([128, 4, D], BF16, tag="ax", name=f"ax_{c - 2}")
                    dgroups = dsa_group_list(c - 2, ax)
                # weave: spread D groups evenly through the P stream;
                # pool(c-1) after the 4 Q-projection groups
                npg, ndg = len(pgroups), len(dgroups)
                dpos = {int(round((k + 1) * npg / (ndg + 1))): k for k in range(ndg)} \
                    if npg else {}
                for i in range(max(npg, 1)):
                    if i < npg:
                        pgroups[i]()
                    if i == 3 and 1 <= c <= NCHUNK:
                        pool_chunk(c - 1)
                    if i in dpos:
                        dgroups[dpos[i]]()
                if not pgroups:
                    if 1 <= c <= NCHUNK:
                        pool_chunk(c - 1)
                    for g in dgroups:
                        g()
                if ax is not None:
                    store_ax(c - 2, ax)


        # ================= phase 2 =================
        with ExitStack() as ctx:
            p2 = ctx.enter_context(tc.tile_pool(name="p2", bufs=1))

            pv = p2.tile([128, 4, WIN, D], BF16, tag="pv")
            wtn = p2.tile([128, 4, WN], BF16, tag="wtn")
            pqT = p2.tile([128, 4, WN], BF16, tag="pqT")
            pkT = p2.tile([128, 4, WN], BF16, tag="pkT")
            esA = p2.tile([128, H, 4, WN], BF16, tag="esA")
            zt = p2.tile([128, 4, QLEN], BF16, tag="zt")
            recip_sb = p2.tile([128, H, 4], F32, tag="recips")

            # issue axt transposes + pv gathers in dependency-arrival order
            srcv = axd.ap().rearrange("(cc p w) d -> cc p w d", p=128, w=PW)
            for c in range(NCHUNK):
                ti = nc.sync.dma_start(
                    out=axt[:, :, c * CH:(c + 1) * CH],
                    in_=axd[c * CH:(c + 1) * CH, :],
                    transpose=True)
                add_dep_helper(ti.ins, axd_writers[c].ins,
                               reason="axt transpose reads axd chunk")
                if c % 2 == 1:
                    cc = c // 2
                    gi = nc.sync.dma_start(out=pv[:, cc, :, :], in_=srcv[cc, :, 1:PW, :])
                    add_dep_helper(gi.ins, axd_writers[2 * cc].ins, reason="pv gather")
                    add_dep_helper(gi.ins, axd_writers[2 * cc + 1].ins, reason="pv gather")

            # ---- win_tok (+bv) LN + GELU + pq/pk, per 128-window group ----
            with ExitStack() as lctx:
                ps_ln = lctx.enter_context(
                    tc.tile_pool(name="ps_ln", bufs=3, space="PSUM"))
                lnp = lctx.enter_context(tc.tile_pool(name="lnp", bufs=2))

                lnA = {}

                def ln_phase_a(g):
                    """Moments + rstd; ACT funcs all within one table set
                    (Identity/Square/Copy/Sqrt)."""
                    wt_g = axt[:, :, g * GW * PW:(g + 1) * GW * PW:PW]
                    wtb = lnp.tile([128, 4, GW], BF16, tag="wtb", bufs=4,
                                   name=f"wtb_{g}")
                    for j in range(4):
                        nc.scalar.activation(wtb[:, j, :], wt_g[:, j, :],
                                             AF.Identity,
                                             bias=bias_cols["bv_c"][:, j:j + 1],
                                             scale=1.0)
                    wsq = lnp.tile([128, 4, GW], BF16, tag="wsq", name=f"wsq_{g}")
                    nc.scalar.activation(wsq[:], wtb[:], AF.Square)
                    ps_mu = ps_ln.tile([128, GW], F32, tag="psln", name=f"psmu_{g}")
                    ps_var = ps_ln.tile([128, GW], F32, tag="psln", name=f"psvar_{g}")
                    for j in range(4):
                        nc.tensor.matmul(ps_mu[:], ones_full[:], wtb[:, j, :],
                                         start=(j == 0), stop=(j == 3),
                                         skip_group_check=True)
                        nc.tensor.matmul(ps_var[:], ones_full[:], wsq[:, j, :],
                                         start=(j == 0), stop=(j == 3),
                                         skip_group_check=True)
                    mu = lnp.tile([128, GW], F32, tag="mu", bufs=4, name=f"mu_{g}")
                    nc.scalar.mul(mu[:], ps_mu[:], 1.0 / D)
                    ex2 = lnp.tile([128, GW], F32, tag="ex2", bufs=1,
                                   name=f"ex2_{g}")
                    nc.scalar.mul(ex2[:], ps_var[:], 1.0 / D)
                    var = lnp.tile([128, GW], F32, tag="var", bufs=1,
                                   name=f"var_{g}")
                    nc.vector.tensor_mul(var[:], mu[:], mu[:])
                    nc.vector.tensor_sub(var[:], ex2[:], var[:])
                    sd = lnp.tile([128, GW], F32, tag="sd", bufs=1, name=f"sd_{g}")
                    nc.scalar.activation(sd[:], var[:], AF.Sqrt, bias=eps_sb[:])
                    rstd = lnp.tile([128, GW], F32, tag="rstd", bufs=4,
                                    name=f"rstd_{g}")
                    nc.vector.reciprocal(rstd[:], sd[:])
                    lnA[g] = (wtb, mu, rstd)

                def ln_phase_b(g):
                    """GELU + pq/pk projections (Gelu/Identity table set)."""
                    gs = g * GW
                    wtb, mu, rstd = lnA.pop(g)
                    for j in range(4):
                        tmp = lnp.tile([128, GW], F32, tag="lnt", name=f"lnt_{g}_{j}")
                        nc.vector.tensor_sub(tmp[:], wtb[:, j, :], mu[:])
                        nc.vector.tensor_mul(tmp[:], tmp[:], rstd[:])
                        nc.scalar.activation(wtn[:, j, gs:gs + GW], tmp[:],
                                             AF.Gelu,
                                             bias=bias_cols["ln_b_c"][:, j:j + 1],
                                             scale=bias_cols["ln_g_c"][:, j:j + 1])
                    for dst, wname, bname in ((pqT, "wpq", "bpq_c"),
                                              (pkT, "wpk", "bpk_c")):
                        for j in range(4):
                            ps = ps_ln.tile([128, GW], F32, tag="psln",
                                            name=f"pp_{wname}_{g}_{j}")
                            for dk in range(4):
                                nc.tensor.matmul(
                                    ps[:], W[wname][:, dk, j * 128:(j + 1) * 128],
                                    wtn[:, dk, gs:gs + GW],
                                    start=(dk == 0), stop=(dk == 3))
                            nc.vector.tensor_scalar_add(
                                dst[:, j, gs:gs + GW], ps[:],
                                bias_cols[bname][:, j:j + 1])

                # A0..A2 then B0..B2 (one Sqrt->Gelu table switch), then the
                # last group's A3+B3 pair on the critical path (one more
                # switch pair).
                for g in range(NG - 1):
                    ln_phase_a(g)
                for g in range(NG - 1):
                    ln_phase_b(g)
                ln_phase_a(NG - 1)
                ln_phase_b(NG - 1)

            # ---- PSA: raw exp scores; den via N=1 matmuls; window-major pout
            with ExitStack() as pctx:
                # PSUM budget (8 banks): es/fin share slots (disjoint
                # lifetimes, same shape) 2 + po 2 + ztps 2 + den 1 = 7.
                ps_es = pctx.enter_context(
                    tc.tile_pool(name="ps_es", bufs=2, space="PSUM"))
                ps_po = pctx.enter_context(
                    tc.tile_pool(name="ps_po", bufs=2, space="PSUM"))
                ps_ztden = pctx.enter_context(
                    tc.tile_pool(name="ps_ztden", bufs=2, space="PSUM"))
                ps_fin = ps_es
                zwp = pctx.enter_context(tc.tile_pool(name="zwp", bufs=2))
                ztp = pctx.enter_context(tc.tile_pool(name="ztp", bufs=2))
                osb = pctx.enter_context(tc.tile_pool(name="osb", bufs=3))

                def psa_scores(h):
                    base = (h % 2) * 64
                    for cc in range(4):
                        ps = ps_es.tile([128, WN], F32, tag="es",
                                    name=f"es_{h}_{cc}")
                        nc.tensor.matmul(
                            ps[:], pkT[base:base + 64, h // 2,
                                       cc * 128:(cc + 1) * 128],
                            pqT[base:base + 64, h // 2, :], start=True, stop=True)
                        nc.scalar.activation(esA[:, h, cc, :], ps[:], AF.Exp,
                                             scale=SCALE)

                den_ps = None

                def psa_den(h):
                    for qt in range(4):
                        idx = h * 4 + qt
                        for cc in range(4):
                            nc.tensor.matmul(
                                den_ps[:, idx:idx + 1],
                                esA[:, h, cc, qt * 128:(qt + 1) * 128],
                                ones_col[:], start=(cc == 0), stop=(cc == 3),
                                skip_group_check=True)
                    nc.vector.reciprocal(recip_sb[:, h, :],
                                         den_ps[:, h * 4:(h + 1) * 4])

                def pout_one(h, qt, zwin):
                    po = ps_po.tile([128, WIN, HD], F32, tag="po",
                                    name=f"po_{h}_{qt}")
                    for cc in range(4):
                        nc.tensor.matmul(
                            po[:], esA[:, h, cc, qt * 128:(qt + 1) * 128],
                            pv[:, cc, :, h * 64:(h + 1) * 64],
                            start=(cc == 0), stop=(cc == 3),
                            skip_group_check=True)
                    ztmp = ztp.tile([128, WIN, HD], BF16, tag="ztmp",
                                    name=f"ztmp_{h}_{qt}")
                    nc.vector.tensor_scalar_mul(ztmp[:], po[:],
                                                recip_sb[:, h, qt:qt + 1])
                    nc.vector.tensor_add(zwin[:, :, h * 64:(h + 1) * 64], ztmp[:],
                                         pv[:, qt, :, h * 64:(h + 1) * 64])

                def ztrans_one(qt, ii, zwin):
                    """Transpose payload slots ii..ii+1 (or just ii at the
                    tail) of group qt into feature-major zt."""
                    ni = min(2, WIN - ii)
                    zt_ps = ps_ztden.tile([128, 4, 2, 128], BF16, tag="ztps",
                                          name=f"ztps_{qt}_{ii}")
                    for di in range(ni):
                        for fg in range(4):
                            nc.tensor.transpose(
                                zt_ps[:, fg, di, :],
                                zwin[:, ii + di, fg * 128:(fg + 1) * 128],
                                ident_sb[:])
                    base = qt * GW * WIN
                    dst = zt[:, :, base + ii:base + GW * WIN:WIN]
                    dst = bass.AP(dst.tensor, dst.offset,
                                  [dst.ap[0], dst.ap[1], [1, ni], [WIN, 128]])
                    src_ap = zt_ps[:, :, 0:ni, :]
                    nc.scalar.copy(dst, src_ap)

                osb_tiles = {}

                def fin_one(tt):
                    ps = ps_fin.tile([128, D], F32, tag="es", name=f"fin_{tt}")
                    for dk in range(4):
                        nc.tensor.matmul(ps[:], zt[:, dk, tt * 128:(tt + 1) * 128],
                                         W["wo"][:, dk, :], start=(dk == 0),
                                         stop=False, skip_group_check=True)
                    nc.tensor.matmul(ps[:], ones_row[:], bo_sb[:], start=False,
                                     stop=True, skip_group_check=True)
                    g = tt // 2
                    if tt % 2 == 0:
                        osb_tiles[g] = osb.tile([128, 2, D], F32, tag="osb",
                                                name=f"osb_{g}")
                    nc.vector.tensor_copy(osb_tiles[g][:, tt % 2, :], ps[:])
                    if tt % 2 == 1:
                        outv = out.ap().rearrange("(g tt p) d -> g p tt d",
                                                  tt=2, p=128)
                        nc.sync.dma_start(out=outv[g], in_=osb_tiles.pop(g)[:])

                # head-outer pipeline: as soon as head h's exp-scores are
                # done, its denominators and all four pout groups flow; the
                # transposes + final projections drain afterwards per group.
                for h in range(H):
                    psa_scores(h)
                den_ps = ps_ztden.tile([128, H * 4], F32, tag="den", name="den",
                                       bufs=1)
                for h in range(H):
                    psa_den(h)

                zw = {}
                prev = None

                def tail_items(qt):
                    items = []
                    zwin_p = zw[qt]
                    for ii in range(0, WIN, 2):
                        items.append(lambda ii=ii: ztrans_one(qt, ii, zwin_p))
                    for j in range(WIN):
                        items.append(lambda j=j: fin_one(qt * WIN + j))
                    return items

                for qt in range(NG):
                    zw[qt] = zwp.tile([128, WIN, D], BF16, tag="zwin",
                                      name=f"zwin_{qt}")
                    titems = tail_items(prev) if prev is not None else []
                    ti = 0
                    for h in range(H):
                        pout_one(h, qt, zw[qt])
                        for _ in range(2):
                            if ti < len(titems) and (h * 12) // H >= ti:
                                titems[ti]()
                                ti += 1
                    while ti < len(titems):
                        titems[ti]()
                        ti += 1
                    if prev is not None:
                        zw.pop(prev)
                    prev = qt
                for it in tail_items(prev):
                    it()


_NC_CACHE = None


def _get_program():
    global _NC_CACHE
    if _NC_CACHE is None:
        _NC_CACHE = build_program()
    return _NC_CACHE


def _host_consts(Wk, bk, Wv, bv, Wq, bq, ln_g, ln_b, Wpq, bpq, Wpk, bpk, Wo, bo):
    bf = ml_dtypes.bfloat16
    col = lambda b: np.asarray(b, np.float32).reshape(4, 128).T.copy()
    bo2 = np.asarray(bo, np.float32) + 2.0 * (
        np.asarray(bv, np.float32) @ np.asarray(Wo, np.float32))
    consts = {
        "wq": np.asarray(Wq, np.float32).astype(bf),
        "wk": np.asarray(Wk, np.float32).astype(bf),
        "wv": np.asarray(Wv, np.float32).astype(bf),
        "wpq": np.asarray(Wpq, np.float32).astype(bf),
        "wpk": np.asarray(Wpk, np.float32).astype(bf),
        "wo": np.asarray(Wo, np.float32).astype(bf),
        "bq_c": col(bq), "bk_c": col(bk),
        "bpq_c": col(bpq), "bpk_c": col(bpk),
        "ln_g_c": col(ln_g), "ln_b_c": col(ln_b),
        "bv_c": col(bv),
        "bo_r": bo2.reshape(1, D).astype(bf),
        "ident": np.eye(128, dtype=np.float32).astype(bf),
    }
    m = np.zeros((128, 128), np.float32)
    for g in range(16):
        m[g * PW:(g + 1) * PW, g * PW:(g + 1) * PW] = 1.0
    consts["bmask"] = m.astype(bf)
    return consts


def kernel(k, v, q, query_len, Wk, bk, Wv, bv, Wq, bq, ln_g, ln_b,
           Wpq, bpq, Wpk, bpk, Wo, bo):
    nc = _get_program()
    consts = _host_consts(Wk, bk, Wv, bv, Wq, bq, ln_g, ln_b,
                          Wpq, bpq, Wpk, bpk, Wo, bo)
    k = np.asarray(k, np.float32)
    v = np.asarray(v, np.float32)
    q = np.asarray(q, np.float32)
    in_maps = []
    for b in range(B):
        m = {"q": np.ascontiguousarray(q[b]), "k": np.ascontiguousarray(k[b]),
             "v": np.ascontiguousarray(v[b])}
        m.update(consts)
        in_maps.append(m)
    res = run_bass_kernel_spmd(nc, in_maps, core_ids=list(range(B)))
    return np.stack([res.results[b]["out"] for b in range(B)], axis=0)


if __name__ == "__main__":
    nc = build_program()
    print("program built ok")


# revision 46
# speedup vs baseline: 1.5177x; 1.0184x over previous
"""Trainium2 Bass kernel for DeformableMultiHeadedAttention.

Data-parallel over batch B=8 across 8 NeuronCores (one batch element per
core, identical programs, no collectives).

Per-core pipeline (matmuls bf16 with f32 accumulate):
  1. q,k,v [4096,512] f32 -> SWDGE cast-DMA -> DRAM bf16 -> batched HWDGE
     DMA-transpose (one [512,512] xbar transpose per chunk) -> feature-major
     XT [128,4,tok] chunks in SBUF.
  2. Projections on PE: K'T/Q'T feature-major (lhsT=W, rhs=XT); V' token-major
     (lhsT=XT tile, rhs=W), bv folded out on host (bo' = bo + 2*bv@Wo, LN
     input gets +bv on chip).
  3. Q pooling (AvgPool k=5, stride 1, zero pad) as 3 shifted adds; the 1/5
     is folded into the softmax exp scale.
  4. DSA (windows of 8 tokens): per 128-token tile, 8 heads: S_T[k,q] on PE,
     exp on ACT, block-diag mask mul on DVE, attn@V plus ones-col denominator
     sharing the lhsT, per-partition 1/den scale on DVE. Token-major DSA
     output -> DRAM (bf16).
  5. DRAM round-trips: batched DMA-transpose -> attn_xT feature-major;
     strided gather -> PV window-major [kw, (slot, head, hd)].
  6. Incremental (per 128-window group, overlapping phase 1 tail): win_tok
     +bv, LayerNorm moments via ones-matmuls, exact GELU, pq/pk projections.
  7. PSA restructured: raw exp-scores kept unnormalized; denominators via
     N=1 matmuls against a ones column (per-partition 1/den on DVE); pout
     window-major [wq, (slot,hd)] at M=128 (half the PE rows of the
     feature-major form); z = pout*recip + attn_x in window-major form;
     PE identity-transposes + strided ACT copies build feature-major zT.
  8. out = Z @ Wo + bo' with Z as the stationary operand -> token-major f32
     output, streamed per window-group.
"""

import sys
from contextlib import ExitStack

for _p in ("/opt/trn_rl_repo/concourse", "/opt/trn_rl_repo"):
    if _p not in sys.path:
        sys.path.insert(0, _p)

import numpy as np
import ml_dtypes

import concourse.bass as bass
import concourse.mybir as mybir
import concourse.tile as tile
from concourse import bacc
from concourse.tile import add_dep_helper
from concourse.bass_utils import run_bass_kernel_spmd

BF16 = mybir.dt.bfloat16
F32 = mybir.dt.float32
AF = mybir.ActivationFunctionType
ALU = mybir.AluOpType

B, M, D = 8, 4096, 512
H, HD = 8, 64
WIN = 7
PW = WIN + 1
QNB = 5
QLEN = 3584
WN = M // PW
SCALE = D ** -0.5
EPS = 1e-5
NCHUNK = 8
CH = 512
NG = 4                   # window groups of 128 for phase 2
GW = WN // NG            # 128 windows per group
PERM = [(h % 2) * 4 + h // 2 for h in range(H)]  # head -> DSA psum slot


def build_program():
    nc = bacc.Bacc("TRN2", target_bir_lowering=False, debug=False, num_devices=8)

    t = {}
    t["q_in"] = nc.dram_tensor("q", [M, D], F32, kind="ExternalInput")
    t["k_in"] = nc.dram_tensor("k", [M, D], F32, kind="ExternalInput")
    t["v_in"] = nc.dram_tensor("v", [M, D], F32, kind="ExternalInput")
    for nm in ("wq", "wk", "wv", "wpq", "wpk", "wo"):
        t[nm] = nc.dram_tensor(nm, [D, D], BF16, kind="ExternalInput")
    for nm in ("bq_c", "bk_c", "bpq_c", "bpk_c", "ln_g_c", "ln_b_c", "bv_c"):
        t[nm] = nc.dram_tensor(nm, [128, 4], F32, kind="ExternalInput")
    t["bo_r"] = nc.dram_tensor("bo_r", [1, D], BF16, kind="ExternalInput")
    t["bmask"] = nc.dram_tensor("bmask", [128, 128], BF16, kind="ExternalInput")
    t["ident"] = nc.dram_tensor("ident", [128, 128], BF16, kind="ExternalInput")
    t["out"] = nc.dram_tensor("out", [QLEN, D], F32, kind="ExternalOutput")
    t["axd"] = nc.dram_tensor("axd_s", [M, D], BF16, kind="Internal")
    t["zd"] = nc.dram_tensor("zd_s", [QLEN, D], BF16, kind="Internal")
    t["qb"] = nc.dram_tensor("qb_s", [M, D], BF16, kind="Internal")
    t["kb"] = nc.dram_tensor("kb_s", [M, D], BF16, kind="Internal")
    t["vb"] = nc.dram_tensor("vb_s", [M, D], BF16, kind="Internal")

    with tile.TileContext(nc) as tc:
        _build(nc, tc, t)
    nc.compile()
    return nc


def _build(nc, tc, t):
    qb, kb, vb = t["qb"], t["kb"], t["vb"]
    axd, out = t["axd"], t["out"]
    zd = t["zd"]

    with ExitStack() as octx:
        singles = octx.enter_context(tc.tile_pool(name="singles", bufs=1))

        # phase-1 weights first (needed by the first projections), then the
        # input casts, then everything else so the casts win the DMA engines.
        cast_insts = {"q": [], "k": [], "v": []}

        def issue_casts(lo, hi):
            for nm, srcd, dst in (("q", t["q_in"], qb), ("k", t["k_in"], kb),
                                  ("v", t["v_in"], vb)):
                ci = nc.gpsimd.dma_start(
                    out=dst[lo * CH:hi * CH, :],
                    in_=srcd[lo * CH:hi * CH, :])
                cast_insts[nm].append(((lo, hi), ci))

        issue_casts(0, 1)
        W = {}
        for nm in ("wq", "wk", "wv"):
            W[nm] = singles.tile([128, 4, D], BF16, tag=nm, name=f"w_{nm}")
            nc.scalar.dma_start(out=W[nm][:],
                                in_=t[nm].ap().rearrange("(c p) d -> p c d", p=128))
        issue_casts(1, 2)

        bias_cols = {}
        for nm in ("bq_c", "bk_c"):
            bias_cols[nm] = singles.tile([128, 4], F32, tag=nm, name=f"bc_{nm}")
            nc.scalar.dma_start(out=bias_cols[nm][:], in_=t[nm][:, :])
        mask_sb = singles.tile([128, 128], BF16)
        nc.scalar.dma_start(out=mask_sb[:], in_=t["bmask"][:, :])
        ones_row = singles.tile([1, 128], BF16)
        nc.vector.memset(ones_row[:], 1.0)
        ones_col = singles.tile([128, 1], BF16)
        nc.vector.memset(ones_col[:], 1.0)
        ones_full = singles.tile([128, 128], BF16)
        nc.vector.memset(ones_full[:], 1.0)
        eps_sb = singles.tile([128, 1], F32)
        nc.vector.memset(eps_sb[:], EPS)

        issue_casts(2, 4)
        issue_casts(4, 6)
        issue_casts(6, 8)

        axd_writers = {}
        p2a = octx.enter_context(tc.tile_pool(name="p2a", bufs=1))
        axt = p2a.tile([128, 4, M], BF16, tag="axt")

        # ================= phase 2 weights ==============================
        # deferred behind the early input casts so they don't hog the DMA
        # engines during the pipeline ramp
        # Allocated here; DMAs issued mid-phase-1 (see chunk loop, c==3)
        # on the sync queue so they neither hog the DMA engines at startup
        # nor get scheduled into the phase boundary.
        for nm in ("wpq", "wpk", "wo"):
            W[nm] = singles.tile([128, 4, D], BF16, tag=nm, name=f"w_{nm}")
        for nm in ("bpq_c", "bpk_c", "ln_g_c", "ln_b_c", "bv_c"):
            bias_cols[nm] = singles.tile([128, 4], F32, tag=nm, name=f"bc_{nm}")
        bo_sb = singles.tile([128, D], BF16)
        ident_sb = singles.tile([128, 128], BF16)

        def load_p2_consts(gate):
            dis = []
            for nm in ("wpq", "wpk", "wo"):
                dis.append(nc.sync.dma_start(
                    out=W[nm][:],
                    in_=t[nm].ap().rearrange("(c p) d -> p c d", p=128)))
            for nm in ("bpq_c", "bpk_c", "ln_g_c", "ln_b_c", "bv_c"):
                dis.append(nc.sync.dma_start(out=bias_cols[nm][:], in_=t[nm][:, :]))
            dis.append(nc.sync.dma_start(
                out=bo_sb[:],
                in_=t["bo_r"].ap().to_broadcast((128, D))))
            dis.append(nc.sync.dma_start(out=ident_sb[:], in_=t["ident"][:, :]))
            for di in dis:
                add_dep_helper(di.ins, gate.ins,
                               reason="const loads after startup transposes")

        # ================= phase 1 =================
        with ExitStack() as ctx:
            p1 = ctx.enter_context(tc.tile_pool(name="p1", bufs=1))
            kT = p1.tile([128, 4, 3, CH], BF16, tag="kT")        # ring of 3 chunks
            qpT = p1.tile([128, 4, 3, CH], BF16, tag="qpT")      # ring of 3 chunks
            vtm = p1.tile([128, 12, 8, 65], BF16, tag="vtm")     # ring of 12 tiles, 65-col/head
            nc.vector.memset(vtm[:, :, :, 64:65], 1.0)           # ones col for denominators
            qraw = p1.tile([128, 4, M + 4], BF16, tag="qraw")    # full, padded +-2
            nc.vector.memset(qraw[:, :, 0:2], 0.0)
            nc.vector.memset(qraw[:, :, M + 2:M + 4], 0.0)

            xtp = ctx.enter_context(tc.tile_pool(name="xtp", bufs=2))
            ps_proj = ctx.enter_context(tc.tile_pool(name="ps_proj", bufs=2, space="PSUM"))
            ps_st = ctx.enter_context(tc.tile_pool(name="ps_st", bufs=2, space="PSUM"))
            ps_out = ctx.enter_context(tc.tile_pool(name="ps_out", bufs=1, space="PSUM"))
            dsa_sb = ctx.enter_context(tc.tile_pool(name="dsa_sb", bufs=2))
            pool_tmp = ctx.enter_context(tc.tile_pool(name="pool_tmp", bufs=2))
            ax_pool = ctx.enter_context(tc.tile_pool(name="ax_sb", bufs=2))

            xt_gate = {}

            def load_xt(nm, dram, c):
                xt = xtp.tile([128, 4, CH], BF16, tag=f"xt_{nm}", name=f"xt_{nm}_{c}")
                ti = nc.sync.dma_start(out=xt[:],
                                       in_=dram[c * CH:(c + 1) * CH, :],
                                       transpose=True)
                for (lo, hi), ci in cast_insts[nm]:
                    if lo <= c < hi:
                        add_dep_helper(ti.ins, ci.ins,
                                       reason="transpose reads cast output")
                xt_gate[(nm, c)] = ti
                return xt

            def proj_fm_group(xt, wname, bname, dst_fn, j):
                ps = ps_proj.tile([128, CH], F32, tag="proj",
                                  name=f"ps_{wname}_{j}")
                for dk in range(4):
                    nc.tensor.matmul(ps[:], W[wname][:, dk, j * 128:(j + 1) * 128],
                                     xt[:, dk, :], start=(dk == 0), stop=(dk == 3))
                nc.scalar.activation(dst_fn(j), ps[:], AF.Identity,
                                     bias=bias_cols[bname][:, j:j + 1], scale=1.0)

            def proj_v_group(xt, c, tt):
                ps = ps_proj.tile([128, D], F32, tag="proj", name=f"ps_v_{tt}")
                for dk in range(4):
                    nc.tensor.matmul(ps[:], xt[:, dk, tt * 128:(tt + 1) * 128],
                                     W["wv"][:, dk, :], start=(dk == 0),
                                     stop=(dk == 3), skip_group_check=True)
                nc.scalar.copy(vtm[:, (c * 4 + tt) % 12, :, 0:64],
                               ps[:].rearrange("p (h d) -> p h d", h=H))

            def pool_chunk(c):
                base = c * CH
                ta = pool_tmp.tile([128, 4, CH + 2], BF16, tag="ta")
                nc.vector.tensor_add(ta[:], qraw[:, :, base:base + CH + 2],
                                     qraw[:, :, base + 1:base + CH + 3])
                tb = pool_tmp.tile([128, 4, CH], BF16, tag="tb")
                nc.vector.tensor_add(tb[:], ta[:, :, 0:CH], ta[:, :, 2:CH + 2])
                nc.vector.tensor_add(qpT[:, :, c % 3, :], tb[:],
                                     qraw[:, :, base + 4:base + CH + 4])

            def dsa_scores(c, lt):
                """MM1 + exp + mask for tile lt of chunk c -> masked sbuf tile."""
                st = ps_st.tile([128, 8, 128], F32, tag="st", name=f"st_{c}_{lt}")
                for h in range(H):
                    hp = PERM[h]
                    base = (h % 2) * 64
                    lhsT = kT[base:base + 64, h // 2, c % 3, lt * 128:(lt + 1) * 128]
                    rhs = qpT[base:base + 64, h // 2, c % 3, lt * 128:(lt + 1) * 128]
                    nc.tensor.matmul(st[:, hp, :], lhsT, rhs, start=True, stop=True,
                                     skip_group_check=True)
                expS = dsa_sb.tile([128, 8, 128], BF16, tag="expS",
                                   name=f"expS_{c}_{lt}")
                nc.scalar.activation(expS[:], st[:], AF.Exp, scale=SCALE / QNB)
                masked = dsa_sb.tile([128, 8, 128], BF16, tag="masked",
                                     name=f"masked_{c}_{lt}")
                nc.vector.tensor_mul(masked[:], expS[:],
                                     mask_sb[:].unsqueeze(1).to_broadcast((128, 8, 128)))
                return masked

            def dsa_out(c, lt, masked, ax_out):
                """attn@V with ones-col denominators, then normalize."""
                outp = ps_out.tile([128, 2, 512], F32, tag="outp",
                                   name=f"outp_{c}_{lt}")
                for h in range(H):
                    hp = PERM[h]
                    nc.tensor.matmul(outp[:, h // 4, (h % 4) * 65:(h % 4) * 65 + 65],
                                     masked[:, hp, :],
                                     vtm[:, (c * 4 + lt) % 12, h, :],
                                     start=True, stop=True, skip_group_check=True)
                recip = dsa_sb.tile([128, 2, 4], F32, tag="recip",
                                    name=f"recip_{c}_{lt}")
                den_view = bass.AP(outp.tensor, outp[:].offset + 64,
                                   [outp[:].ap[0], [512, 2], [65, 4]])
                nc.vector.reciprocal(recip[:], den_view)
                av_view = bass.AP(outp.tensor, outp[:].offset,
                                  [outp[:].ap[0], [512, 2], [65, 4], [1, 64]])
                nc.vector.tensor_mul(
                    ax_out.rearrange("p (a b d) -> p a b d", a=2, b=4),
                    av_view,
                    recip[:].unsqueeze(3).to_broadcast((128, 2, 4, 64)))

            def dsa_group_list(c, ax):
                masked = {}
                g = []
                g.append(lambda: masked.__setitem__(0, dsa_scores(c, 0)))
                g.append(lambda: masked.__setitem__(1, dsa_scores(c, 1)))
                g.append(lambda: dsa_out(c, 0, masked.pop(0), ax[:, 0, :]))
                g.append(lambda: masked.__setitem__(2, dsa_scores(c, 2)))
                g.append(lambda: dsa_out(c, 1, masked.pop(1), ax[:, 1, :]))
                g.append(lambda: masked.__setitem__(3, dsa_scores(c, 3)))
                g.append(lambda: dsa_out(c, 2, masked.pop(2), ax[:, 2, :]))
                g.append(lambda: dsa_out(c, 3, masked.pop(3), ax[:, 3, :]))
                return g

            def store_ax(c, ax):
                dst = axd.ap().rearrange("(cc lt p) d -> cc p lt d", lt=4, p=128)[c]
                wi = nc.gpsimd.dma_start(out=dst, in_=ax[:])
                axd_writers[c] = wi

            def issue_axt(c):
                ti = nc.sync.dma_start(
                    out=axt[:, :, c * CH:(c + 1) * CH],
                    in_=axd[c * CH:(c + 1) * CH, :],
                    transpose=True)
                add_dep_helper(ti.ins, axd_writers[c].ins,
                               reason="axt transpose reads axd chunk")

            for c in range(NCHUNK + 2):
                pgroups = []
                if c < NCHUNK:
                    qxt = load_xt("q", qb, c)
                    kxt = load_xt("k", kb, c)
                    vxt = load_xt("v", vb, c)
                    if c == 3:
                        load_p2_consts(xt_gate[("q", 2)])
                    for j in range(4):
                        pgroups.append(lambda j=j, x=qxt, c=c: proj_fm_group(
                            x, "wq", "bq_c",
                            lambda jj, c=c: qraw[:, jj, 2 + c * CH:2 + (c + 1) * CH], j))
                    for j in range(4):
                        pgroups.append(lambda j=j, x=kxt, c=c: proj_fm_group(
                            x, "wk", "bk_c", lambda jj, c=c: kT[:, jj, c % 3, :], j))
                    for tt in range(4):
                        pgroups.append(lambda tt=tt, x=vxt, c=c: proj_v_group(x, c, tt))
                dgroups = []
                ax = None
                if c >= 2:
                    ax = ax_pool.tile([128, 4, D], BF16, tag="ax", name=f"ax_{c - 2}")
                    dgroups = dsa_group_list(c - 2, ax)
                # weave: spread D groups evenly through the P stream;
                # pool(c-1) after the 4 Q-projection groups
                npg, ndg = len(pgroups), len(dgroups)
                dpos = {int(round((k + 1) * npg / (ndg + 1))): k for k in range(ndg)} \
                    if npg else {}
                for i in range(max(npg, 1)):
                    if i < npg:
                        pgroups[i]()
                    if i == 3 and 1 <= c <= NCHUNK:
                        pool_chunk(c - 1)
                    if i in dpos:
                        dgroups[dpos[i]]()
                if not pgroups:
                    if 1 <= c <= NCHUNK:
                        pool_chunk(c - 1)
                    for g in dgroups:
                        g()
                if ax is not None:
                    store_ax(c - 2, ax)


        # ================= phase 2 =================
        with ExitStack() as ctx:
            p2 = ctx.enter_context(tc.tile_pool(name="p2", bufs=1))

            pv = p2.tile([128, 4, WIN, D], BF16, tag="pv")
            wtn = p2.tile([128, 4, WN], BF16, tag="wtn")
            pqT = p2.tile([128, 4, WN], BF16, tag="pqT")
            pkT = p2.tile([128, 4, WN], BF16, tag="pkT")
            esA = p2.tile([128, H, 4, WN], BF16, tag="esA")
            zt = p2.tile([128, 4, QLEN], BF16, tag="zt")
            recip_sb = p2.tile([128, H, 4], F32, tag="recips")

            # issue axt transposes + pv gathers in dependency-arrival order
            srcv = axd.ap().rearrange("(cc p w) d -> cc p w d", p=128, w=PW)
            for c in range(NCHUNK):
                issue_axt(c)
                if c % 2 == 1:
                    cc = c // 2
                    gi = nc.sync.dma_start(out=pv[:, cc, :, :], in_=srcv[cc, :, 1:PW, :])
                    add_dep_helper(gi.ins, axd_writers[2 * cc].ins, reason="pv gather")
                    add_dep_helper(gi.ins, axd_writers[2 * cc + 1].ins, reason="pv gather")

            # ---- win_tok (+bv) LN + GELU + pq/pk, per 128-window group ----
            with ExitStack() as lctx:
                ps_ln = lctx.enter_context(
                    tc.tile_pool(name="ps_ln", bufs=3, space="PSUM"))
                lnp = lctx.enter_context(tc.tile_pool(name="lnp", bufs=2))

                lnA = {}

                def ln_phase_a(g):
                    """Moments + rstd; ACT funcs all within one table set
                    (Identity/Square/Copy/Sqrt)."""
                    wt_g = axt[:, :, g * GW * PW:(g + 1) * GW * PW:PW]
                    wtb = lnp.tile([128, 4, GW], BF16, tag="wtb", bufs=4,
                                   name=f"wtb_{g}")
                    for j in range(4):
                        nc.scalar.activation(wtb[:, j, :], wt_g[:, j, :],
                                             AF.Identity,
                                             bias=bias_cols["bv_c"][:, j:j + 1],
                                             scale=1.0)
                    wsq = lnp.tile([128, 4, GW], BF16, tag="wsq", name=f"wsq_{g}")
                    nc.scalar.activation(wsq[:], wtb[:], AF.Square)
                    ps_mu = ps_ln.tile([128, GW], F32, tag="psln", name=f"psmu_{g}")
                    ps_var = ps_ln.tile([128, GW], F32, tag="psln", name=f"psvar_{g}")
                    for j in range(4):
                        nc.tensor.matmul(ps_mu[:], ones_full[:], wtb[:, j, :],
                                         start=(j == 0), stop=(j == 3),
                                         skip_group_check=True)
                        nc.tensor.matmul(ps_var[:], ones_full[:], wsq[:, j, :],
                                         start=(j == 0), stop=(j == 3),
                                         skip_group_check=True)
                    mu = lnp.tile([128, GW], F32, tag="mu", bufs=4, name=f"mu_{g}")
                    nc.scalar.mul(mu[:], ps_mu[:], 1.0 / D)
                    ex2 = lnp.tile([128, GW], F32, tag="ex2", bufs=1,
                                   name=f"ex2_{g}")
                    nc.scalar.mul(ex2[:], ps_var[:], 1.0 / D)
                    var = lnp.tile([128, GW], F32, tag="var", bufs=1,
                                   name=f"var_{g}")
                    nc.vector.tensor_mul(var[:], mu[:], mu[:])
                    nc.vector.tensor_sub(var[:], ex2[:], var[:])
                    sd = lnp.tile([128, GW], F32, tag="sd", bufs=1, name=f"sd_{g}")
                    nc.scalar.activation(sd[:], var[:], AF.Sqrt, bias=eps_sb[:])
                    rstd = lnp.tile([128, GW], F32, tag="rstd", bufs=4,
                                    name=f"rstd_{g}")
                    nc.vector.reciprocal(rstd[:], sd[:])
                    lnA[g] = (wtb, mu, rstd)

                def ln_phase_b(g):
                    """GELU + pq/pk projections (Gelu/Identity table set)."""
                    gs = g * GW
                    wtb, mu, rstd = lnA.pop(g)
                    for j in range(4):
                        tmp = lnp.tile([128, GW], F32, tag="lnt", name=f"lnt_{g}_{j}")
                        nc.vector.tensor_sub(tmp[:], wtb[:, j, :], mu[:])
                        nc.vector.tensor_mul(tmp[:], tmp[:], rstd[:])
                        nc.scalar.activation(wtn[:, j, gs:gs + GW], tmp[:],
                                             AF.Gelu,
                                             bias=bias_cols["ln_b_c"][:, j:j + 1],
                                             scale=bias_cols["ln_g_c"][:, j:j + 1])
                    for dst, wname, bname in ((pqT, "wpq", "bpq_c"),
                                              (pkT, "wpk", "bpk_c")):
                        for j in range(4):
                            ps = ps_ln.tile([128, GW], F32, tag="psln",
                                            name=f"pp_{wname}_{g}_{j}")
                            for dk in range(4):
                                nc.tensor.matmul(
                                    ps[:], W[wname][:, dk, j * 128:(j + 1) * 128],
                                    wtn[:, dk, gs:gs + GW],
                                    start=(dk == 0), stop=(dk == 3))
                            nc.vector.tensor_scalar_add(
                                dst[:, j, gs:gs + GW], ps[:],
                                bias_cols[bname][:, j:j + 1])

                # A0..A2 then B0..B2 (one Sqrt->Gelu table switch), then the
                # last group's A3+B3 pair on the critical path (one more
                # switch pair).
                for g in range(NG - 1):
                    ln_phase_a(g)
                for g in range(NG - 1):
                    ln_phase_b(g)
                ln_phase_a(NG - 1)
                ln_phase_b(NG - 1)

            # ---- PSA: raw exp scores; den via N=1 matmuls; window-major pout
            with ExitStack() as pctx:
                # PSUM budget (8 banks): es/fin share slots (disjoint
                # lifetimes, same shape) 2 + po 2 + ztps 2 + den 1 = 7.
                ps_es = pctx.enter_context(
                    tc.tile_pool(name="ps_es", bufs=3, space="PSUM"))
                ps_po = pctx.enter_context(
                    tc.tile_pool(name="ps_po", bufs=2, space="PSUM"))
                ps_ztden = pctx.enter_context(
                    tc.tile_pool(name="ps_ztden", bufs=2, space="PSUM"))
                ps_fin = ps_es
                zwp = pctx.enter_context(tc.tile_pool(name="zwp", bufs=2))
                ztp = pctx.enter_context(tc.tile_pool(name="ztp", bufs=2))
                osb = pctx.enter_context(tc.tile_pool(name="osb", bufs=3))

                def psa_scores(h):
                    base = (h % 2) * 64
                    for cc in range(4):
                        ps = ps_es.tile([128, WN], F32, tag="es",
                                    name=f"es_{h}_{cc}")
                        nc.tensor.matmul(
                            ps[:], pkT[base:base + 64, h // 2,
                                       cc * 128:(cc + 1) * 128],
                            pqT[base:base + 64, h // 2, :], start=True, stop=True)
                        nc.scalar.activation(esA[:, h, cc, :], ps[:], AF.Exp,
                                             scale=SCALE)

                den_ps = None

                def psa_den(h):
                    for qt in range(4):
                        idx = h * 4 + qt
                        for cc in range(4):
                            nc.tensor.matmul(
                                den_ps[:, idx:idx + 1],
                                esA[:, h, cc, qt * 128:(qt + 1) * 128],
                                ones_col[:], start=(cc == 0), stop=(cc == 3),
                                skip_group_check=True)
                    nc.vector.reciprocal(recip_sb[:, h, :],
                                         den_ps[:, h * 4:(h + 1) * 4])

                def pout_one(h, qt, zwin):
                    po = ps_po.tile([128, WIN, HD], F32, tag="po",
                                    name=f"po_{h}_{qt}")
                    for cc in range(4):
                        nc.tensor.matmul(
                            po[:], esA[:, h, cc, qt * 128:(qt + 1) * 128],
                            pv[:, cc, :, h * 64:(h + 1) * 64],
                            start=(cc == 0), stop=(cc == 3),
                            skip_group_check=True)
                    ztmp = ztp.tile([128, WIN, HD], BF16, tag="ztmp",
                                    name=f"ztmp_{h}_{qt}")
                    nc.vector.tensor_scalar_mul(ztmp[:], po[:],
                                                recip_sb[:, h, qt:qt + 1])
                    nc.vector.tensor_add(zwin[:, :, h * 64:(h + 1) * 64], ztmp[:],
                                         pv[:, qt, :, h * 64:(h + 1) * 64])

                def ztrans_one(qt, ii, zwin):
                    """Transpose payload slots ii..ii+1 (or just ii at the
                    tail) of group qt into feature-major zt."""
                    ni = min(2, WIN - ii)
                    zt_ps = ps_ztden.tile([128, 4, 2, 128], BF16, tag="ztps",
                                          name=f"ztps_{qt}_{ii}")
                    for di in range(ni):
                        for fg in range(4):
                            nc.tensor.transpose(
                                zt_ps[:, fg, di, :],
                                zwin[:, ii + di, fg * 128:(fg + 1) * 128],
                                ident_sb[:])
                    base = qt * GW * WIN
                    dst = zt[:, :, base + ii:base + GW * WIN:WIN]
                    dst = bass.AP(dst.tensor, dst.offset,
                                  [dst.ap[0], dst.ap[1], [1, ni], [WIN, 128]])
                    src_ap = zt_ps[:, :, 0:ni, :]
                    nc.scalar.copy(dst, src_ap)

                osb_tiles = {}

                def fin_one(tt):
                    ps = ps_fin.tile([128, D], F32, tag="es", name=f"fin_{tt}")
                    for dk in range(4):
                        nc.tensor.matmul(ps[:], zt[:, dk, tt * 128:(tt + 1) * 128],
                                         W["wo"][:, dk, :], start=(dk == 0),
                                         stop=(dk == 3), skip_group_check=True)
                    g = tt // 2
                    if tt % 2 == 0:
                        osb_tiles[g] = osb.tile([128, 2, D], F32, tag="osb",
                                                name=f"osb_{g}")
                    nc.vector.tensor_add(osb_tiles[g][:, tt % 2, :], ps[:],
                                         bo_sb[:])
                    if tt % 2 == 1:
                        outv = out.ap().rearrange("(g tt p) d -> g p tt d",
                                                  tt=2, p=128)
                        nc.sync.dma_start(out=outv[g], in_=osb_tiles.pop(g)[:])

                # head-outer pipeline: as soon as head h's exp-scores are
                # done, its denominators and all four pout groups flow; the
                # transposes + final projections drain afterwards per group.
                for h in range(H):
                    psa_scores(h)
                den_ps = ps_ztden.tile([128, H * 4], F32, tag="den", name="den",
                                       bufs=1)
                for h in range(H):
                    psa_den(h)

                zw = {}
                prev = None

                def tail_items(qt):
                    items = []
                    zwin_p = zw[qt]
                    for ii in range(0, WIN, 2):
                        items.append(lambda ii=ii: ztrans_one(qt, ii, zwin_p))
                    for j in range(WIN):
                        items.append(lambda j=j: fin_one(qt * WIN + j))
                    return items

                for qt in range(NG):
                    zw[qt] = zwp.tile([128, WIN, D], BF16, tag="zwin",
                                      name=f"zwin_{qt}")
                    titems = tail_items(prev) if prev is not None else []
                    ti = 0
                    for h in range(H):
                        pout_one(h, qt, zw[qt])
                        for _ in range(2):
                            if ti < len(titems) and (h * 12) // H >= ti:
                                titems[ti]()
                                ti += 1
                    while ti < len(titems):
                        titems[ti]()
                        ti += 1
                    if prev is not None:
                        zw.pop(prev)
                    prev = qt
                for it in tail_items(prev):
                    it()


_NC_CACHE = None


def _get_program():
    global _NC_CACHE
    if _NC_CACHE is None:
        _NC_CACHE = build_program()
    return _NC_CACHE


def _host_consts(Wk, bk, Wv, bv, Wq, bq, ln_g, ln_b, Wpq, bpq, Wpk, bpk, Wo, bo):
    bf = ml_dtypes.bfloat16
    col = lambda b: np.asarray(b, np.float32).reshape(4, 128).T.copy()
    bo2 = np.asarray(bo, np.float32) + 2.0 * (
        np.asarray(bv, np.float32) @ np.asarray(Wo, np.float32))
    consts = {
        "wq": np.asarray(Wq, np.float32).astype(bf),
        "wk": np.asarray(Wk, np.float32).astype(bf),
        "wv": np.asarray(Wv, np.float32).astype(bf),
        "wpq": np.asarray(Wpq, np.float32).astype(bf),
        "wpk": np.asarray(Wpk, np.float32).astype(bf),
        "wo": np.asarray(Wo, np.float32).astype(bf),
        "bq_c": col(bq), "bk_c": col(bk),
        "bpq_c": col(bpq), "bpk_c": col(bpk),
        "ln_g_c": col(ln_g), "ln_b_c": col(ln_b),
        "bv_c": col(bv),
        "bo_r": bo2.reshape(1, D).astype(bf),
        "ident": np.eye(128, dtype=np.float32).astype(bf),
    }
    m = np.zeros((128, 128), np.float32)
    for g in range(16):
        m[g * PW:(g + 1) * PW, g * PW:(g + 1) * PW] = 1.0
    consts["bmask"] = m.astype(bf)
    return consts


def kernel(k, v, q, query_len, Wk, bk, Wv, bv, Wq, bq, ln_g, ln_b,
           Wpq, bpq, Wpk, bpk, Wo, bo):
    nc = _get_program()
    consts = _host_consts(Wk, bk, Wv, bv, Wq, bq, ln_g, ln_b,
                          Wpq, bpq, Wpk, bpk, Wo, bo)
    k = np.asarray(k, np.float32)
    v = np.asarray(v, np.float32)
    q = np.asarray(q, np.float32)
    in_maps = []
    for b in range(B):
        m = {"q": np.ascontiguousarray(q[b]), "k": np.ascontiguousarray(k[b]),
             "v": np.ascontiguousarray(v[b])}
        m.update(consts)
        in_maps.append(m)
    res = run_bass_kernel_spmd(nc, in_maps, core_ids=list(range(B)))
    return np.stack([res.results[b]["out"] for b in range(B)], axis=0)


if __name__ == "__main__":
    nc = build_program()
    print("program built ok")


# revision 52
# speedup vs baseline: 1.5193x; 1.0011x over previous
"""Trainium2 Bass kernel for DeformableMultiHeadedAttention.

Data-parallel over batch B=8 across 8 NeuronCores (one batch element per
core, identical programs, no collectives).

Per-core pipeline (matmuls bf16 with f32 accumulate):
  1. q,k,v [4096,512] f32 -> SWDGE cast-DMA -> DRAM bf16 -> batched HWDGE
     DMA-transpose (one [512,512] xbar transpose per chunk) -> feature-major
     XT [128,4,tok] chunks in SBUF.
  2. Projections on PE: K'T/Q'T feature-major (lhsT=W, rhs=XT); V' token-major
     (lhsT=XT tile, rhs=W), bv folded out on host (bo' = bo + 2*bv@Wo, LN
     input gets +bv on chip).
  3. Q pooling (AvgPool k=5, stride 1, zero pad) as 3 shifted adds; the 1/5
     is folded into the softmax exp scale.
  4. DSA (windows of 8 tokens): per 128-token tile, 8 heads: S_T[k,q] on PE,
     exp on ACT, block-diag mask mul on DVE, attn@V plus ones-col denominator
     sharing the lhsT, per-partition 1/den scale on DVE. Token-major DSA
     output -> DRAM (bf16).
  5. DRAM round-trips: batched DMA-transpose -> attn_xT feature-major;
     strided gather -> PV window-major [kw, (slot, head, hd)].
  6. Incremental (per 128-window group, overlapping phase 1 tail): win_tok
     +bv, LayerNorm moments via ones-matmuls, exact GELU, pq/pk projections.
  7. PSA restructured: raw exp-scores kept unnormalized; denominators via
     N=1 matmuls against a ones column (per-partition 1/den on DVE); pout
     window-major [wq, (slot,hd)] at M=128 (half the PE rows of the
     feature-major form); z = pout*recip + attn_x in window-major form;
     PE identity-transposes + strided ACT copies build feature-major zT.
  8. out = Z @ Wo + bo' with Z as the stationary operand -> token-major f32
     output, streamed per window-group.
"""

import sys
from contextlib import ExitStack

for _p in ("/opt/trn_rl_repo/concourse", "/opt/trn_rl_repo"):
    if _p not in sys.path:
        sys.path.insert(0, _p)

import numpy as np
import ml_dtypes

import concourse.bass as bass
import concourse.mybir as mybir
import concourse.tile as tile
from concourse import bacc
from concourse.tile import add_dep_helper
from concourse.bass_utils import run_bass_kernel_spmd

BF16 = mybir.dt.bfloat16
F32 = mybir.dt.float32
AF = mybir.ActivationFunctionType
ALU = mybir.AluOpType

B, M, D = 8, 4096, 512
H, HD = 8, 64
WIN = 7
PW = WIN + 1
QNB = 5
QLEN = 3584
WN = M // PW
SCALE = D ** -0.5
EPS = 1e-5
NCHUNK = 8
CH = 512
NG = 4                   # window groups of 128 for phase 2
GW = WN // NG            # 128 windows per group
PERM = [(h % 2) * 4 + h // 2 for h in range(H)]  # head -> DSA psum slot


def build_program():
    nc = bacc.Bacc("TRN2", target_bir_lowering=False, debug=False, num_devices=8)

    t = {}
    t["q_in"] = nc.dram_tensor("q", [M, D], F32, kind="ExternalInput")
    t["k_in"] = nc.dram_tensor("k", [M, D], F32, kind="ExternalInput")
    t["v_in"] = nc.dram_tensor("v", [M, D], F32, kind="ExternalInput")
    for nm in ("wq", "wk", "wv", "wpq", "wpk", "wo"):
        t[nm] = nc.dram_tensor(nm, [D, D], BF16, kind="ExternalInput")
    for nm in ("bq_c", "bk_c", "bpq_c", "bpk_c", "ln_g_c", "ln_b_c", "bv_c"):
        t[nm] = nc.dram_tensor(nm, [128, 4], F32, kind="ExternalInput")
    t["bo_r"] = nc.dram_tensor("bo_r", [1, D], BF16, kind="ExternalInput")
    t["bmask"] = nc.dram_tensor("bmask", [128, 128], BF16, kind="ExternalInput")
    t["ident"] = nc.dram_tensor("ident", [128, 128], BF16, kind="ExternalInput")
    t["out"] = nc.dram_tensor("out", [QLEN, D], F32, kind="ExternalOutput")
    t["axd"] = nc.dram_tensor("axd_s", [M, D], BF16, kind="Internal")
    t["zd"] = nc.dram_tensor("zd_s", [QLEN, D], BF16, kind="Internal")
    t["qb"] = nc.dram_tensor("qb_s", [M, D], BF16, kind="Internal")
    t["kb"] = nc.dram_tensor("kb_s", [M, D], BF16, kind="Internal")
    t["vb"] = nc.dram_tensor("vb_s", [M, D], BF16, kind="Internal")

    with tile.TileContext(nc) as tc:
        _build(nc, tc, t)
    nc.compile()
    return nc


def _build(nc, tc, t):
    qb, kb, vb = t["qb"], t["kb"], t["vb"]
    axd, out = t["axd"], t["out"]
    zd = t["zd"]

    with ExitStack() as octx:
        singles = octx.enter_context(tc.tile_pool(name="singles", bufs=1))

        # phase-1 weights first (needed by the first projections), then the
        # input casts, then everything else so the casts win the DMA engines.
        cast_insts = {"q": [], "k": [], "v": []}

        def issue_casts(lo, hi):
            for nm, srcd, dst in (("q", t["q_in"], qb), ("k", t["k_in"], kb),
                                  ("v", t["v_in"], vb)):
                ci = nc.gpsimd.dma_start(
                    out=dst[lo * CH:hi * CH, :],
                    in_=srcd[lo * CH:hi * CH, :])
                cast_insts[nm].append(((lo, hi), ci))

        issue_casts(0, 1)
        W = {}
        for nm in ("wq", "wk", "wv"):
            W[nm] = singles.tile([128, 4, D], BF16, tag=nm, name=f"w_{nm}")
            nc.scalar.dma_start(out=W[nm][:],
                                in_=t[nm].ap().rearrange("(c p) d -> p c d", p=128))
        issue_casts(1, 2)

        bias_cols = {}
        for nm in ("bq_c", "bk_c"):
            bias_cols[nm] = singles.tile([128, 4], F32, tag=nm, name=f"bc_{nm}")
            nc.scalar.dma_start(out=bias_cols[nm][:], in_=t[nm][:, :])
        mask_sb = singles.tile([128, 128], BF16)
        nc.scalar.dma_start(out=mask_sb[:], in_=t["bmask"][:, :])
        ones_row = singles.tile([1, 128], BF16)
        nc.vector.memset(ones_row[:], 1.0)
        ones_col = singles.tile([128, 1], BF16)
        nc.vector.memset(ones_col[:], 1.0)
        ones_full = singles.tile([128, 128], BF16)
        nc.vector.memset(ones_full[:], 1.0)
        eps_sb = singles.tile([128, 1], F32)
        nc.vector.memset(eps_sb[:], EPS)

        issue_casts(2, 4)
        issue_casts(4, 6)
        issue_casts(6, 8)

        axd_writers = {}
        p2a = octx.enter_context(tc.tile_pool(name="p2a", bufs=1))
        axt = p2a.tile([128, 4, M], BF16, tag="axt")

        # ================= phase 2 weights ==============================
        # deferred behind the early input casts so they don't hog the DMA
        # engines during the pipeline ramp
        # Allocated here; DMAs issued mid-phase-1 (see chunk loop, c==3)
        # on the sync queue so they neither hog the DMA engines at startup
        # nor get scheduled into the phase boundary.
        for nm in ("wpq", "wpk", "wo"):
            W[nm] = singles.tile([128, 4, D], BF16, tag=nm, name=f"w_{nm}")
        for nm in ("bpq_c", "bpk_c", "ln_g_c", "ln_b_c", "bv_c"):
            bias_cols[nm] = singles.tile([128, 4], F32, tag=nm, name=f"bc_{nm}")
        bo_sb = singles.tile([128, D], BF16)
        ident_sb = singles.tile([128, 128], BF16)

        def load_p2_consts(gate):
            dis = []
            for nm in ("wpq", "wpk", "wo"):
                dis.append(nc.sync.dma_start(
                    out=W[nm][:],
                    in_=t[nm].ap().rearrange("(c p) d -> p c d", p=128)))
            for nm in ("bpq_c", "bpk_c", "ln_g_c", "ln_b_c", "bv_c"):
                dis.append(nc.sync.dma_start(out=bias_cols[nm][:], in_=t[nm][:, :]))
            dis.append(nc.sync.dma_start(
                out=bo_sb[:],
                in_=t["bo_r"].ap().to_broadcast((128, D))))
            dis.append(nc.sync.dma_start(out=ident_sb[:], in_=t["ident"][:, :]))
            for di in dis:
                add_dep_helper(di.ins, gate.ins,
                               reason="const loads after startup transposes")

        # ================= phase 1 =================
        with ExitStack() as ctx:
            p1 = ctx.enter_context(tc.tile_pool(name="p1", bufs=1))
            kT = p1.tile([128, 4, 3, CH], BF16, tag="kT")        # ring of 3 chunks
            qpT = p1.tile([128, 4, 3, CH], BF16, tag="qpT")      # ring of 3 chunks
            vtm = p1.tile([128, 12, 8, 65], BF16, tag="vtm")     # ring of 12 tiles, 65-col/head
            nc.vector.memset(vtm[:, :, :, 64:65], 1.0)           # ones col for denominators
            qraw = p1.tile([128, 4, M + 4], BF16, tag="qraw")    # full, padded +-2
            nc.vector.memset(qraw[:, :, 0:2], 0.0)
            nc.vector.memset(qraw[:, :, M + 2:M + 4], 0.0)

            xtp = ctx.enter_context(tc.tile_pool(name="xtp", bufs=2))
            ps_proj = ctx.enter_context(tc.tile_pool(name="ps_proj", bufs=2, space="PSUM"))
            ps_st = ctx.enter_context(tc.tile_pool(name="ps_st", bufs=2, space="PSUM"))
            ps_out = ctx.enter_context(tc.tile_pool(name="ps_out", bufs=1, space="PSUM"))
            dsa_sb = ctx.enter_context(tc.tile_pool(name="dsa_sb", bufs=2))
            pool_tmp = ctx.enter_context(tc.tile_pool(name="pool_tmp", bufs=2))
            ax_pool = ctx.enter_context(tc.tile_pool(name="ax_sb", bufs=2))

            xt_gate = {}

            def load_xt(nm, dram, c):
                xt = xtp.tile([128, 4, CH], BF16, tag=f"xt_{nm}", name=f"xt_{nm}_{c}")
                ti = nc.sync.dma_start(out=xt[:],
                                       in_=dram[c * CH:(c + 1) * CH, :],
                                       transpose=True)
                for (lo, hi), ci in cast_insts[nm]:
                    if lo <= c < hi:
                        add_dep_helper(ti.ins, ci.ins,
                                       reason="transpose reads cast output")
                xt_gate[(nm, c)] = ti
                return xt

            def proj_fm_group(xt, wname, bname, dst_fn, j):
                ps = ps_proj.tile([128, CH], F32, tag="proj",
                                  name=f"ps_{wname}_{j}")
                for dk in range(4):
                    nc.tensor.matmul(ps[:], W[wname][:, dk, j * 128:(j + 1) * 128],
                                     xt[:, dk, :], start=(dk == 0), stop=(dk == 3))
                nc.scalar.activation(dst_fn(j), ps[:], AF.Identity,
                                     bias=bias_cols[bname][:, j:j + 1], scale=1.0)

            def proj_v_group(xt, c, tt):
                ps = ps_proj.tile([128, D], F32, tag="proj", name=f"ps_v_{tt}")
                for dk in range(4):
                    nc.tensor.matmul(ps[:], xt[:, dk, tt * 128:(tt + 1) * 128],
                                     W["wv"][:, dk, :], start=(dk == 0),
                                     stop=(dk == 3), skip_group_check=True)
                nc.scalar.copy(vtm[:, (c * 4 + tt) % 12, :, 0:64],
                               ps[:].rearrange("p (h d) -> p h d", h=H))

            def pool_chunk(c):
                base = c * CH
                ta = pool_tmp.tile([128, 4, CH + 2], BF16, tag="ta")
                nc.vector.tensor_add(ta[:], qraw[:, :, base:base + CH + 2],
                                     qraw[:, :, base + 1:base + CH + 3])
                tb = pool_tmp.tile([128, 4, CH], BF16, tag="tb")
                nc.vector.tensor_add(tb[:], ta[:, :, 0:CH], ta[:, :, 2:CH + 2])
                nc.vector.tensor_add(qpT[:, :, c % 3, :], tb[:],
                                     qraw[:, :, base + 4:base + CH + 4])

            def dsa_scores(c, lt):
                """MM1 + exp + mask for tile lt of chunk c -> masked sbuf tile."""
                st = ps_st.tile([128, 8, 128], F32, tag="st", name=f"st_{c}_{lt}")
                for h in range(H):
                    hp = PERM[h]
                    base = (h % 2) * 64
                    lhsT = kT[base:base + 64, h // 2, c % 3, lt * 128:(lt + 1) * 128]
                    rhs = qpT[base:base + 64, h // 2, c % 3, lt * 128:(lt + 1) * 128]
                    nc.tensor.matmul(st[:, hp, :], lhsT, rhs, start=True, stop=True,
                                     skip_group_check=True)
                expS = dsa_sb.tile([128, 8, 128], BF16, tag="expS",
                                   name=f"expS_{c}_{lt}")
                nc.scalar.activation(expS[:], st[:], AF.Exp, scale=SCALE / QNB)
                masked = dsa_sb.tile([128, 8, 128], BF16, tag="masked",
                                     name=f"masked_{c}_{lt}")
                nc.vector.tensor_mul(masked[:], expS[:],
                                     mask_sb[:].unsqueeze(1).to_broadcast((128, 8, 128)))
                return masked

            def dsa_out(c, lt, masked, ax_out):
                """attn@V with ones-col denominators, then normalize."""
                outp = ps_out.tile([128, 2, 512], F32, tag="outp",
                                   name=f"outp_{c}_{lt}")
                for h in range(H):
                    hp = PERM[h]
                    nc.tensor.matmul(outp[:, h // 4, (h % 4) * 65:(h % 4) * 65 + 65],
                                     masked[:, hp, :],
                                     vtm[:, (c * 4 + lt) % 12, h, :],
                                     start=True, stop=True, skip_group_check=True)
                recip = dsa_sb.tile([128, 2, 4], F32, tag="recip",
                                    name=f"recip_{c}_{lt}")
                den_view = bass.AP(outp.tensor, outp[:].offset + 64,
                                   [outp[:].ap[0], [512, 2], [65, 4]])
                nc.vector.reciprocal(recip[:], den_view)
                av_view = bass.AP(outp.tensor, outp[:].offset,
                                  [outp[:].ap[0], [512, 2], [65, 4], [1, 64]])
                nc.vector.tensor_mul(
                    ax_out.rearrange("p (a b d) -> p a b d", a=2, b=4),
                    av_view,
                    recip[:].unsqueeze(3).to_broadcast((128, 2, 4, 64)))

            def dsa_group_list(c, ax):
                masked = {}
                g = []
                g.append(lambda: masked.__setitem__(0, dsa_scores(c, 0)))
                g.append(lambda: masked.__setitem__(1, dsa_scores(c, 1)))
                g.append(lambda: dsa_out(c, 0, masked.pop(0), ax[:, 0, :]))
                g.append(lambda: masked.__setitem__(2, dsa_scores(c, 2)))
                g.append(lambda: dsa_out(c, 1, masked.pop(1), ax[:, 1, :]))
                g.append(lambda: masked.__setitem__(3, dsa_scores(c, 3)))
                g.append(lambda: dsa_out(c, 2, masked.pop(2), ax[:, 2, :]))
                g.append(lambda: dsa_out(c, 3, masked.pop(3), ax[:, 3, :]))
                return g

            def store_ax(c, ax):
                dst = axd.ap().rearrange("(cc lt p) d -> cc p lt d", lt=4, p=128)[c]
                wi = nc.gpsimd.dma_start(out=dst, in_=ax[:])
                axd_writers[c] = wi

            def issue_axt(c):
                ti = nc.sync.dma_start(
                    out=axt[:, :, c * CH:(c + 1) * CH],
                    in_=axd[c * CH:(c + 1) * CH, :],
                    transpose=True)
                add_dep_helper(ti.ins, axd_writers[c].ins,
                               reason="axt transpose reads axd chunk")

            for c in range(NCHUNK + 2):
                pgroups = []
                if c < NCHUNK:
                    qxt = load_xt("q", qb, c)
                    kxt = load_xt("k", kb, c)
                    vxt = load_xt("v", vb, c)
                    if c == 3:
                        load_p2_consts(xt_gate[("q", 2)])
                    for j in range(4):
                        pgroups.append(lambda j=j, x=qxt, c=c: proj_fm_group(
                            x, "wq", "bq_c",
                            lambda jj, c=c: qraw[:, jj, 2 + c * CH:2 + (c + 1) * CH], j))
                    for j in range(4):
                        pgroups.append(lambda j=j, x=kxt, c=c: proj_fm_group(
                            x, "wk", "bk_c", lambda jj, c=c: kT[:, jj, c % 3, :], j))
                    for tt in range(4):
                        pgroups.append(lambda tt=tt, x=vxt, c=c: proj_v_group(x, c, tt))
                dgroups = []
                ax = None
                if c >= 2:
                    ax = ax_pool.tile([128, 4, D], BF16, tag="ax", name=f"ax_{c - 2}")
                    dgroups = dsa_group_list(c - 2, ax)
                # weave: spread D groups evenly through the P stream;
                # pool(c-1) after the 4 Q-projection groups
                npg, ndg = len(pgroups), len(dgroups)
                dpos = {int(round((k + 1) * npg / (ndg + 1))): k for k in range(ndg)} \
                    if npg else {}
                for i in range(max(npg, 1)):
                    if i < npg:
                        pgroups[i]()
                    if i == 3 and 1 <= c <= NCHUNK:
                        pool_chunk(c - 1)
                    if i in dpos:
                        dgroups[dpos[i]]()
                if not pgroups:
                    if 1 <= c <= NCHUNK:
                        pool_chunk(c - 1)
                    for g in dgroups:
                        g()
                if ax is not None:
                    store_ax(c - 2, ax)


        # ================= phase 2 =================
        with ExitStack() as ctx:
            p2 = ctx.enter_context(tc.tile_pool(name="p2", bufs=1))

            pv = p2.tile([128, 4, WIN, D], BF16, tag="pv")
            wtn = p2.tile([128, 4, WN], BF16, tag="wtn")
            pqT = p2.tile([128, 4, WN], BF16, tag="pqT")
            pkT = p2.tile([128, 4, WN], BF16, tag="pkT")
            esA = p2.tile([128, H, 4, WN], BF16, tag="esA")
            zt = p2.tile([128, 4, QLEN], BF16, tag="zt")
            recip_sb = p2.tile([128, H, 4], F32, tag="recips")

            # issue axt transposes + pv gathers in dependency-arrival order
            srcv = axd.ap().rearrange("(cc p w) d -> cc p w d", p=128, w=PW)
            for c in range(NCHUNK):
                issue_axt(c)
                if c % 2 == 1:
                    cc = c // 2
                    gi = nc.sync.dma_start(out=pv[:, cc, :, :], in_=srcv[cc, :, 1:PW, :])
                    add_dep_helper(gi.ins, axd_writers[2 * cc].ins, reason="pv gather")
                    add_dep_helper(gi.ins, axd_writers[2 * cc + 1].ins, reason="pv gather")

            # ---- win_tok (+bv) LN + GELU + pq/pk, per 128-window group ----
            with ExitStack() as lctx:
                ps_ln = lctx.enter_context(
                    tc.tile_pool(name="ps_ln", bufs=3, space="PSUM"))
                lnp = lctx.enter_context(tc.tile_pool(name="lnp", bufs=2))

                lnA = {}

                def ln_phase_a(g):
                    """Moments + rstd; ACT funcs all within one table set
                    (Identity/Square/Copy/Sqrt)."""
                    wt_g = axt[:, :, g * GW * PW:(g + 1) * GW * PW:PW]
                    wtb = lnp.tile([128, 4, GW], BF16, tag="wtb", bufs=4,
                                   name=f"wtb_{g}")
                    for j in range(4):
                        nc.scalar.activation(wtb[:, j, :], wt_g[:, j, :],
                                             AF.Identity,
                                             bias=bias_cols["bv_c"][:, j:j + 1],
                                             scale=1.0)
                    wsq = lnp.tile([128, 4, GW], BF16, tag="wsq", name=f"wsq_{g}")
                    nc.scalar.activation(wsq[:], wtb[:], AF.Square)
                    ps_mu = ps_ln.tile([128, GW], F32, tag="psln", name=f"psmu_{g}")
                    ps_var = ps_ln.tile([128, GW], F32, tag="psln", name=f"psvar_{g}")
                    for j in range(4):
                        nc.tensor.matmul(ps_mu[:], ones_full[:], wtb[:, j, :],
                                         start=(j == 0), stop=(j == 3),
                                         skip_group_check=True)
                        nc.tensor.matmul(ps_var[:], ones_full[:], wsq[:, j, :],
                                         start=(j == 0), stop=(j == 3),
                                         skip_group_check=True)
                    mu = lnp.tile([128, GW], F32, tag="mu", bufs=4, name=f"mu_{g}")
                    nc.scalar.mul(mu[:], ps_mu[:], 1.0 / D)
                    ex2 = lnp.tile([128, GW], F32, tag="ex2", bufs=1,
                                   name=f"ex2_{g}")
                    nc.scalar.mul(ex2[:], ps_var[:], 1.0 / D)
                    var = lnp.tile([128, GW], F32, tag="var", bufs=1,
                                   name=f"var_{g}")
                    nc.vector.tensor_mul(var[:], mu[:], mu[:])
                    nc.vector.tensor_sub(var[:], ex2[:], var[:])
                    sd = lnp.tile([128, GW], F32, tag="sd", bufs=1, name=f"sd_{g}")
                    nc.scalar.activation(sd[:], var[:], AF.Sqrt, bias=eps_sb[:])
                    rstd = lnp.tile([128, GW], F32, tag="rstd", bufs=4,
                                    name=f"rstd_{g}")
                    nc.vector.reciprocal(rstd[:], sd[:])
                    lnA[g] = (wtb, mu, rstd)

                def ln_phase_b(g):
                    """GELU + pq/pk projections (Gelu/Identity table set)."""
                    gs = g * GW
                    wtb, mu, rstd = lnA.pop(g)
                    for j in range(4):
                        tmp = lnp.tile([128, GW], F32, tag="lnt", name=f"lnt_{g}_{j}")
                        nc.vector.tensor_sub(tmp[:], wtb[:, j, :], mu[:])
                        nc.vector.tensor_mul(tmp[:], tmp[:], rstd[:])
                        nc.scalar.activation(wtn[:, j, gs:gs + GW], tmp[:],
                                             AF.Gelu,
                                             bias=bias_cols["ln_b_c"][:, j:j + 1],
                                             scale=bias_cols["ln_g_c"][:, j:j + 1])
                    for dst, wname, bname in ((pqT, "wpq", "bpq_c"),
                                              (pkT, "wpk", "bpk_c")):
                        for j in range(4):
                            ps = ps_ln.tile([128, GW], F32, tag="psln",
                                            name=f"pp_{wname}_{g}_{j}")
                            for dk in range(4):
                                nc.tensor.matmul(
                                    ps[:], W[wname][:, dk, j * 128:(j + 1) * 128],
                                    wtn[:, dk, gs:gs + GW],
                                    start=(dk == 0), stop=(dk == 3))
                            nc.vector.tensor_scalar_add(
                                dst[:, j, gs:gs + GW], ps[:],
                                bias_cols[bname][:, j:j + 1])

                # A0..A2 then B0..B2 (one Sqrt->Gelu table switch), then the
                # last group's A3+B3 pair on the critical path (one more
                # switch pair).
                for g in range(NG - 1):
                    ln_phase_a(g)
                for g in range(NG - 1):
                    ln_phase_b(g)
                ln_phase_a(NG - 1)
                ln_phase_b(NG - 1)

            # ---- PSA: raw exp scores; den via N=1 matmuls; window-major pout
            with ExitStack() as pctx:
                # PSUM budget (8 banks): es/fin share slots (disjoint
                # lifetimes, same shape) 2 + po 2 + ztps 2 + den 1 = 7.
                ps_es = pctx.enter_context(
                    tc.tile_pool(name="ps_es", bufs=3, space="PSUM"))
                ps_po = pctx.enter_context(
                    tc.tile_pool(name="ps_po", bufs=2, space="PSUM"))
                ps_ztden = pctx.enter_context(
                    tc.tile_pool(name="ps_ztden", bufs=2, space="PSUM"))
                ps_fin = ps_es
                zwp = pctx.enter_context(tc.tile_pool(name="zwp", bufs=3))
                ztp = pctx.enter_context(tc.tile_pool(name="ztp", bufs=2))
                osb = pctx.enter_context(tc.tile_pool(name="osb", bufs=4))

                def psa_scores(h):
                    base = (h % 2) * 64
                    for cc in range(4):
                        ps = ps_es.tile([128, WN], F32, tag="es",
                                    name=f"es_{h}_{cc}")
                        nc.tensor.matmul(
                            ps[:], pkT[base:base + 64, h // 2,
                                       cc * 128:(cc + 1) * 128],
                            pqT[base:base + 64, h // 2, :], start=True, stop=True)
                        nc.scalar.activation(esA[:, h, cc, :], ps[:], AF.Exp,
                                             scale=SCALE)

                den_ps = None

                def psa_den(h):
                    for qt in range(4):
                        idx = h * 4 + qt
                        for cc in range(4):
                            nc.tensor.matmul(
                                den_ps[:, idx:idx + 1],
                                esA[:, h, cc, qt * 128:(qt + 1) * 128],
                                ones_col[:], start=(cc == 0), stop=(cc == 3),
                                skip_group_check=True)
                    nc.vector.reciprocal(recip_sb[:, h, :],
                                         den_ps[:, h * 4:(h + 1) * 4])

                def pout_one(h, qt, zwin):
                    po = ps_po.tile([128, WIN, HD], F32, tag="po",
                                    name=f"po_{h}_{qt}")
                    for cc in range(4):
                        nc.tensor.matmul(
                            po[:], esA[:, h, cc, qt * 128:(qt + 1) * 128],
                            pv[:, cc, :, h * 64:(h + 1) * 64],
                            start=(cc == 0), stop=(cc == 3),
                            skip_group_check=True)
                    ztmp = ztp.tile([128, WIN, HD], BF16, tag="ztmp",
                                    name=f"ztmp_{h}_{qt}")
                    nc.vector.tensor_scalar_mul(ztmp[:], po[:],
                                                recip_sb[:, h, qt:qt + 1])
                    nc.vector.tensor_add(zwin[:, :, h * 64:(h + 1) * 64], ztmp[:],
                                         pv[:, qt, :, h * 64:(h + 1) * 64])

                def ztrans_one(qt, ii, zwin):
                    """Transpose payload slots ii..ii+1 (or just ii at the
                    tail) of group qt into feature-major zt."""
                    ni = min(2, WIN - ii)
                    zt_ps = ps_ztden.tile([128, 4, 2, 128], BF16, tag="ztps",
                                          name=f"ztps_{qt}_{ii}")
                    for di in range(ni):
                        for fg in range(4):
                            nc.tensor.transpose(
                                zt_ps[:, fg, di, :],
                                zwin[:, ii + di, fg * 128:(fg + 1) * 128],
                                ident_sb[:])
                    base = qt * GW * WIN
                    dst = zt[:, :, base + ii:base + GW * WIN:WIN]
                    dst = bass.AP(dst.tensor, dst.offset,
                                  [dst.ap[0], dst.ap[1], [1, ni], [WIN, 128]])
                    src_ap = zt_ps[:, :, 0:ni, :]
                    nc.scalar.copy(dst, src_ap)

                osb_tiles = {}

                def fin_one(tt):
                    ps = ps_fin.tile([128, D], F32, tag="es", name=f"fin_{tt}")
                    for dk in range(4):
                        nc.tensor.matmul(ps[:], zt[:, dk, tt * 128:(tt + 1) * 128],
                                         W["wo"][:, dk, :], start=(dk == 0),
                                         stop=(dk == 3), skip_group_check=True)
                    g = tt // 2
                    if tt % 2 == 0:
                        osb_tiles[g] = osb.tile([128, 2, D], F32, tag="osb",
                                                name=f"osb_{g}")
                    nc.vector.tensor_add(osb_tiles[g][:, tt % 2, :], ps[:],
                                         bo_sb[:])
                    if tt % 2 == 1:
                        outv = out.ap().rearrange("(g tt p) d -> g p tt d",
                                                  tt=2, p=128)
                        nc.sync.dma_start(out=outv[g], in_=osb_tiles.pop(g)[:])

                # head-outer pipeline: as soon as head h's exp-scores are
                # done, its denominators and all four pout groups flow; the
                # transposes + final projections drain afterwards per group.
                for h in range(H):
                    psa_scores(h)
                den_ps = ps_ztden.tile([128, H * 4], F32, tag="den", name="den",
                                       bufs=1)
                for h in range(H):
                    psa_den(h)

                zw = {}
                prev = None

                def tail_items(qt):
                    items = []
                    zwin_p = zw[qt]
                    for ii in range(0, WIN, 2):
                        items.append(lambda ii=ii: ztrans_one(qt, ii, zwin_p))
                    for j in range(WIN):
                        items.append(lambda j=j: fin_one(qt * WIN + j))
                    return items

                for qt in range(NG):
                    zw[qt] = zwp.tile([128, WIN, D], BF16, tag="zwin",
                                      name=f"zwin_{qt}")
                    titems = tail_items(prev) if prev is not None else []
                    ti = 0
                    for h in range(H):
                        pout_one(h, qt, zw[qt])
                        for _ in range(2):
                            if ti < len(titems) and (h * 8) // H >= ti:
                                titems[ti]()
                                ti += 1
                    while ti < len(titems):
                        titems[ti]()
                        ti += 1
                    if prev is not None:
                        zw.pop(prev)
                    prev = qt
                for it in tail_items(prev):
                    it()


_NC_CACHE = None


def _get_program():
    global _NC_CACHE
    if _NC_CACHE is None:
        _NC_CACHE = build_program()
    return _NC_CACHE


def _host_consts(Wk, bk, Wv, bv, Wq, bq, ln_g, ln_b, Wpq, bpq, Wpk, bpk, Wo, bo):
    bf = ml_dtypes.bfloat16
    col = lambda b: np.asarray(b, np.float32).reshape(4, 128).T.copy()
    bo2 = np.asarray(bo, np.float32) + 2.0 * (
        np.asarray(bv, np.float32) @ np.asarray(Wo, np.float32))
    consts = {
        "wq": np.asarray(Wq, np.float32).astype(bf),
        "wk": np.asarray(Wk, np.float32).astype(bf),
        "wv": np.asarray(Wv, np.float32).astype(bf),
        "wpq": np.asarray(Wpq, np.float32).astype(bf),
        "wpk": np.asarray(Wpk, np.float32).astype(bf),
        "wo": np.asarray(Wo, np.float32).astype(bf),
        "bq_c": col(bq), "bk_c": col(bk),
        "bpq_c": col(bpq), "bpk_c": col(bpk),
        "ln_g_c": col(ln_g), "ln_b_c": col(ln_b),
        "bv_c": col(bv),
        "bo_r": bo2.reshape(1, D).astype(bf),
        "ident": np.eye(128, dtype=np.float32).astype(bf),
    }
    m = np.zeros((128, 128), np.float32)
    for g in range(16):
        m[g * PW:(g + 1) * PW, g * PW:(g + 1) * PW] = 1.0
    consts["bmask"] = m.astype(bf)
    return consts


def kernel(k, v, q, query_len, Wk, bk, Wv, bv, Wq, bq, ln_g, ln_b,
           Wpq, bpq, Wpk, bpk, Wo, bo):
    nc = _get_program()
    consts = _host_consts(Wk, bk, Wv, bv, Wq, bq, ln_g, ln_b,
                          Wpq, bpq, Wpk, bpk, Wo, bo)
    k = np.asarray(k, np.float32)
    v = np.asarray(v, np.float32)
    q = np.asarray(q, np.float32)
    in_maps = []
    for b in range(B):
        m = {"q": np.ascontiguousarray(q[b]), "k": np.ascontiguousarray(k[b]),
             "v": np.ascontiguousarray(v[b])}
        m.update(consts)
        in_maps.append(m)
    res = run_bass_kernel_spmd(nc, in_maps, core_ids=list(range(B)))
    return np.stack([res.results[b]["out"] for b in range(B)], axis=0)


if __name__ == "__main__":
    nc = build_program()
    print("program built ok")


# revision 60
# speedup vs baseline: 1.5208x; 1.0010x over previous
"""Trainium2 Bass kernel for DeformableMultiHeadedAttention.

Data-parallel over batch B=8 across 8 NeuronCores (one batch element per
core, identical programs, no collectives).

Per-core pipeline (matmuls bf16 with f32 accumulate):
  1. q,k,v [4096,512] f32 -> SWDGE cast-DMA -> DRAM bf16 -> batched HWDGE
     DMA-transpose (one [512,512] xbar transpose per chunk) -> feature-major
     XT [128,4,tok] chunks in SBUF.
  2. Projections on PE: K'T/Q'T feature-major (lhsT=W, rhs=XT); V' token-major
     (lhsT=XT tile, rhs=W), bv folded out on host (bo' = bo + 2*bv@Wo, LN
     input gets +bv on chip).
  3. Q pooling (AvgPool k=5, stride 1, zero pad) as 3 shifted adds; the 1/5
     is folded into the softmax exp scale.
  4. DSA (windows of 8 tokens): per 128-token tile, 8 heads: S_T[k,q] on PE,
     exp on ACT, block-diag mask mul on DVE, attn@V plus ones-col denominator
     sharing the lhsT, per-partition 1/den scale on DVE. Token-major DSA
     output -> DRAM (bf16).
  5. DRAM round-trips: batched DMA-transpose -> attn_xT feature-major;
     strided gather -> PV window-major [kw, (slot, head, hd)].
  6. Incremental (per 128-window group, overlapping phase 1 tail): win_tok
     +bv, LayerNorm moments via ones-matmuls, exact GELU, pq/pk projections.
  7. PSA restructured: raw exp-scores kept unnormalized; denominators via
     N=1 matmuls against a ones column (per-partition 1/den on DVE); pout
     window-major [wq, (slot,hd)] at M=128 (half the PE rows of the
     feature-major form); z = pout*recip + attn_x in window-major form;
     PE identity-transposes + strided ACT copies build feature-major zT.
  8. out = Z @ Wo + bo' with Z as the stationary operand -> token-major f32
     output, streamed per window-group.
"""

import sys
from contextlib import ExitStack

for _p in ("/opt/trn_rl_repo/concourse", "/opt/trn_rl_repo"):
    if _p not in sys.path:
        sys.path.insert(0, _p)

import numpy as np
import ml_dtypes

import concourse.bass as bass
import concourse.mybir as mybir
import concourse.tile as tile
from concourse import bacc
from concourse.tile import add_dep_helper
from concourse.bass_utils import run_bass_kernel_spmd

BF16 = mybir.dt.bfloat16
F32 = mybir.dt.float32
AF = mybir.ActivationFunctionType
ALU = mybir.AluOpType

B, M, D = 8, 4096, 512
H, HD = 8, 64
WIN = 7
PW = WIN + 1
QNB = 5
QLEN = 3584
WN = M // PW
SCALE = D ** -0.5
EPS = 1e-5
NCHUNK = 8
CH = 512
NG = 4                   # window groups of 128 for phase 2
GW = WN // NG            # 128 windows per group
PERM = [(h % 2) * 4 + h // 2 for h in range(H)]  # head -> DSA psum slot


def build_program():
    nc = bacc.Bacc("TRN2", target_bir_lowering=False, debug=False, num_devices=8)

    t = {}
    t["q_in"] = nc.dram_tensor("q", [M, D], F32, kind="ExternalInput")
    t["k_in"] = nc.dram_tensor("k", [M, D], F32, kind="ExternalInput")
    t["v_in"] = nc.dram_tensor("v", [M, D], F32, kind="ExternalInput")
    for nm in ("wq", "wk", "wv", "wpq", "wpk", "wo"):
        t[nm] = nc.dram_tensor(nm, [D, D], BF16, kind="ExternalInput")
    for nm in ("bq_c", "bk_c", "bpq_c", "bpk_c", "ln_g_c", "ln_b_c", "bv_c"):
        t[nm] = nc.dram_tensor(nm, [128, 4], F32, kind="ExternalInput")
    t["bo_r"] = nc.dram_tensor("bo_r", [1, D], BF16, kind="ExternalInput")
    t["bmask"] = nc.dram_tensor("bmask", [128, 128], BF16, kind="ExternalInput")
    t["ident"] = nc.dram_tensor("ident", [128, 128], BF16, kind="ExternalInput")
    t["out"] = nc.dram_tensor("out", [QLEN, D], F32, kind="ExternalOutput")
    t["axd"] = nc.dram_tensor("axd_s", [M, D], BF16, kind="Internal")
    t["zd"] = nc.dram_tensor("zd_s", [QLEN, D], BF16, kind="Internal")
    t["qb"] = nc.dram_tensor("qb_s", [M, D], BF16, kind="Internal")
    t["kb"] = nc.dram_tensor("kb_s", [M, D], BF16, kind="Internal")
    t["vb"] = nc.dram_tensor("vb_s", [M, D], BF16, kind="Internal")

    with tile.TileContext(nc) as tc:
        _build(nc, tc, t)
    nc.compile()
    return nc


def _build(nc, tc, t):
    qb, kb, vb = t["qb"], t["kb"], t["vb"]
    axd, out = t["axd"], t["out"]
    zd = t["zd"]

    with ExitStack() as octx:
        singles = octx.enter_context(tc.tile_pool(name="singles", bufs=1))

        # phase-1 weights first (needed by the first projections), then the
        # input casts, then everything else so the casts win the DMA engines.
        cast_insts = {"q": [], "k": [], "v": []}

        def issue_casts(lo, hi):
            for nm, srcd, dst in (("q", t["q_in"], qb), ("k", t["k_in"], kb),
                                  ("v", t["v_in"], vb)):
                ci = nc.gpsimd.dma_start(
                    out=dst[lo * CH:hi * CH, :],
                    in_=srcd[lo * CH:hi * CH, :])
                cast_insts[nm].append(((lo, hi), ci))

        issue_casts(0, 1)
        W = {}
        for nm in ("wq", "wk", "wv"):
            W[nm] = singles.tile([128, 4, D], BF16, tag=nm, name=f"w_{nm}")
            nc.scalar.dma_start(out=W[nm][:],
                                in_=t[nm].ap().rearrange("(c p) d -> p c d", p=128))
        issue_casts(1, 2)

        bias_cols = {}
        for nm in ("bq_c", "bk_c"):
            bias_cols[nm] = singles.tile([128, 4], F32, tag=nm, name=f"bc_{nm}")
            nc.scalar.dma_start(out=bias_cols[nm][:], in_=t[nm][:, :])
        mask_sb = singles.tile([128, 128], BF16)
        nc.scalar.dma_start(out=mask_sb[:], in_=t["bmask"][:, :])
        ones_row = singles.tile([1, 128], BF16)
        nc.vector.memset(ones_row[:], 1.0)
        ones_col = singles.tile([128, 1], BF16)
        nc.vector.memset(ones_col[:], 1.0)
        ones_full = singles.tile([128, 128], BF16)
        nc.vector.memset(ones_full[:], 1.0)
        eps_sb = singles.tile([128, 1], F32)
        nc.vector.memset(eps_sb[:], EPS)

        issue_casts(2, 4)
        issue_casts(4, 6)
        issue_casts(6, 8)

        axd_writers = {}
        p2a = octx.enter_context(tc.tile_pool(name="p2a", bufs=1))
        axt = p2a.tile([128, 4, M], BF16, tag="axt")

        # ================= phase 2 weights ==============================
        # deferred behind the early input casts so they don't hog the DMA
        # engines during the pipeline ramp
        # Allocated here; DMAs issued mid-phase-1 (see chunk loop, c==3)
        # on the sync queue so they neither hog the DMA engines at startup
        # nor get scheduled into the phase boundary.
        for nm in ("wpq", "wpk", "wo"):
            W[nm] = singles.tile([128, 4, D], BF16, tag=nm, name=f"w_{nm}")
        for nm in ("bpq_c", "bpk_c", "ln_g_c", "ln_b_c", "bv_c"):
            bias_cols[nm] = singles.tile([128, 4], F32, tag=nm, name=f"bc_{nm}")
        bo_sb = singles.tile([128, D], BF16)
        ident_sb = singles.tile([128, 128], BF16)

        def load_p2_consts(gate):
            dis = []
            for nm in ("wpq", "wpk", "wo"):
                dis.append(nc.sync.dma_start(
                    out=W[nm][:],
                    in_=t[nm].ap().rearrange("(c p) d -> p c d", p=128)))
            for nm in ("bpq_c", "bpk_c", "ln_g_c", "ln_b_c", "bv_c"):
                dis.append(nc.sync.dma_start(out=bias_cols[nm][:], in_=t[nm][:, :]))
            dis.append(nc.sync.dma_start(
                out=bo_sb[:],
                in_=t["bo_r"].ap().to_broadcast((128, D))))
            dis.append(nc.sync.dma_start(out=ident_sb[:], in_=t["ident"][:, :]))
            for di in dis:
                add_dep_helper(di.ins, gate.ins,
                               reason="const loads after startup transposes")

        # ================= phase 1 =================
        with ExitStack() as ctx:
            p1 = ctx.enter_context(tc.tile_pool(name="p1", bufs=1))
            kT = p1.tile([128, 4, 3, CH], BF16, tag="kT")        # ring of 3 chunks
            qpT = p1.tile([128, 4, 3, CH], BF16, tag="qpT")      # ring of 3 chunks
            vtm = p1.tile([128, 12, 8, 65], BF16, tag="vtm")     # ring of 12 tiles, 65-col/head
            nc.vector.memset(vtm[:, :, :, 64:65], 1.0)           # ones col for denominators
            qraw = p1.tile([128, 4, M + 4], BF16, tag="qraw")    # full, padded +-2
            nc.vector.memset(qraw[:, :, 0:2], 0.0)
            nc.vector.memset(qraw[:, :, M + 2:M + 4], 0.0)

            xtp = ctx.enter_context(tc.tile_pool(name="xtp", bufs=2))
            ps_proj = ctx.enter_context(tc.tile_pool(name="ps_proj", bufs=2, space="PSUM"))
            ps_st = ctx.enter_context(tc.tile_pool(name="ps_st", bufs=2, space="PSUM"))
            ps_out = ctx.enter_context(tc.tile_pool(name="ps_out", bufs=1, space="PSUM"))
            dsa_sb = ctx.enter_context(tc.tile_pool(name="dsa_sb", bufs=2))
            pool_tmp = ctx.enter_context(tc.tile_pool(name="pool_tmp", bufs=2))
            ax_pool = ctx.enter_context(tc.tile_pool(name="ax_sb", bufs=2))

            xt_gate = {}

            def load_xt(nm, dram, c):
                xt = xtp.tile([128, 4, CH], BF16, tag=f"xt_{nm}", name=f"xt_{nm}_{c}")
                ti = nc.sync.dma_start(out=xt[:],
                                       in_=dram[c * CH:(c + 1) * CH, :],
                                       transpose=True)
                for (lo, hi), ci in cast_insts[nm]:
                    if lo <= c < hi:
                        add_dep_helper(ti.ins, ci.ins,
                                       reason="transpose reads cast output")
                xt_gate[(nm, c)] = ti
                return xt

            def proj_fm_group(xt, wname, bname, dst_fn, j):
                ps = ps_proj.tile([128, CH], F32, tag="proj",
                                  name=f"ps_{wname}_{j}")
                for dk in range(4):
                    nc.tensor.matmul(ps[:], W[wname][:, dk, j * 128:(j + 1) * 128],
                                     xt[:, dk, :], start=(dk == 0), stop=(dk == 3))
                nc.scalar.activation(dst_fn(j), ps[:], AF.Identity,
                                     bias=bias_cols[bname][:, j:j + 1], scale=1.0)

            def proj_v_group(xt, c, tt):
                ps = ps_proj.tile([128, D], F32, tag="proj", name=f"ps_v_{tt}")
                for dk in range(4):
                    nc.tensor.matmul(ps[:], xt[:, dk, tt * 128:(tt + 1) * 128],
                                     W["wv"][:, dk, :], start=(dk == 0),
                                     stop=(dk == 3), skip_group_check=True)
                nc.scalar.copy(vtm[:, (c * 4 + tt) % 12, :, 0:64],
                               ps[:].rearrange("p (h d) -> p h d", h=H))

            def pool_chunk(c):
                base = c * CH
                ta = pool_tmp.tile([128, 4, CH + 2], BF16, tag="ta")
                nc.vector.tensor_add(ta[:], qraw[:, :, base:base + CH + 2],
                                     qraw[:, :, base + 1:base + CH + 3])
                tb = pool_tmp.tile([128, 4, CH], BF16, tag="tb")
                nc.vector.tensor_add(tb[:], ta[:, :, 0:CH], ta[:, :, 2:CH + 2])
                nc.vector.tensor_add(qpT[:, :, c % 3, :], tb[:],
                                     qraw[:, :, base + 4:base + CH + 4])

            def dsa_scores(c, lt):
                """MM1 + exp + mask for tile lt of chunk c -> masked sbuf tile."""
                st = ps_st.tile([128, 8, 128], F32, tag="st", name=f"st_{c}_{lt}")
                for h in range(H):
                    hp = PERM[h]
                    base = (h % 2) * 64
                    lhsT = kT[base:base + 64, h // 2, c % 3, lt * 128:(lt + 1) * 128]
                    rhs = qpT[base:base + 64, h // 2, c % 3, lt * 128:(lt + 1) * 128]
                    nc.tensor.matmul(st[:, hp, :], lhsT, rhs, start=True, stop=True,
                                     skip_group_check=True)
                expS = dsa_sb.tile([128, 8, 128], BF16, tag="expS",
                                   name=f"expS_{c}_{lt}")
                nc.scalar.activation(expS[:], st[:], AF.Exp, scale=SCALE / QNB)
                masked = dsa_sb.tile([128, 8, 128], BF16, tag="masked",
                                     name=f"masked_{c}_{lt}")
                nc.vector.tensor_mul(masked[:], expS[:],
                                     mask_sb[:].unsqueeze(1).to_broadcast((128, 8, 128)))
                return masked

            def dsa_out(c, lt, masked, ax_out):
                """attn@V with ones-col denominators, then normalize."""
                outp = ps_out.tile([128, 2, 512], F32, tag="outp",
                                   name=f"outp_{c}_{lt}")
                for h in range(H):
                    hp = PERM[h]
                    nc.tensor.matmul(outp[:, h // 4, (h % 4) * 65:(h % 4) * 65 + 65],
                                     masked[:, hp, :],
                                     vtm[:, (c * 4 + lt) % 12, h, :],
                                     start=True, stop=True, skip_group_check=True)
                recip = dsa_sb.tile([128, 2, 4], F32, tag="recip",
                                    name=f"recip_{c}_{lt}")
                den_view = bass.AP(outp.tensor, outp[:].offset + 64,
                                   [outp[:].ap[0], [512, 2], [65, 4]])
                nc.vector.reciprocal(recip[:], den_view)
                av_view = bass.AP(outp.tensor, outp[:].offset,
                                  [outp[:].ap[0], [512, 2], [65, 4], [1, 64]])
                nc.vector.tensor_mul(
                    ax_out.rearrange("p (a b d) -> p a b d", a=2, b=4),
                    av_view,
                    recip[:].unsqueeze(3).to_broadcast((128, 2, 4, 64)))

            def dsa_group_list(c, ax):
                masked = {}
                g = []
                g.append(lambda: masked.__setitem__(0, dsa_scores(c, 0)))
                g.append(lambda: masked.__setitem__(1, dsa_scores(c, 1)))
                g.append(lambda: dsa_out(c, 0, masked.pop(0), ax[:, 0, :]))
                g.append(lambda: masked.__setitem__(2, dsa_scores(c, 2)))
                g.append(lambda: dsa_out(c, 1, masked.pop(1), ax[:, 1, :]))
                g.append(lambda: masked.__setitem__(3, dsa_scores(c, 3)))
                g.append(lambda: dsa_out(c, 2, masked.pop(2), ax[:, 2, :]))
                g.append(lambda: dsa_out(c, 3, masked.pop(3), ax[:, 3, :]))
                return g

            def store_ax(c, ax):
                dst = axd.ap().rearrange("(cc lt p) d -> cc p lt d", lt=4, p=128)[c]
                wi = nc.gpsimd.dma_start(out=dst, in_=ax[:])
                axd_writers[c] = wi

            def issue_axt(c):
                ti = nc.sync.dma_start(
                    out=axt[:, :, c * CH:(c + 1) * CH],
                    in_=axd[c * CH:(c + 1) * CH, :],
                    transpose=True)
                add_dep_helper(ti.ins, axd_writers[c].ins,
                               reason="axt transpose reads axd chunk")

            for c in range(NCHUNK + 2):
                pgroups = []
                if c < NCHUNK:
                    qxt = load_xt("q", qb, c)
                    kxt = load_xt("k", kb, c)
                    vxt = load_xt("v", vb, c)
                    if c == 3:
                        load_p2_consts(xt_gate[("q", 2)])
                    for j in range(4):
                        pgroups.append(lambda j=j, x=qxt, c=c: proj_fm_group(
                            x, "wq", "bq_c",
                            lambda jj, c=c: qraw[:, jj, 2 + c * CH:2 + (c + 1) * CH], j))
                    for j in range(4):
                        pgroups.append(lambda j=j, x=kxt, c=c: proj_fm_group(
                            x, "wk", "bk_c", lambda jj, c=c: kT[:, jj, c % 3, :], j))
                    for tt in range(4):
                        pgroups.append(lambda tt=tt, x=vxt, c=c: proj_v_group(x, c, tt))
                dgroups = []
                ax = None
                if c >= 2:
                    ax = ax_pool.tile([128, 4, D], BF16, tag="ax", name=f"ax_{c - 2}")
                    dgroups = dsa_group_list(c - 2, ax)
                # weave: spread D groups evenly through the P stream;
                # pool(c-1) after the 4 Q-projection groups
                npg, ndg = len(pgroups), len(dgroups)
                dpos = {int(round((k + 1) * npg / (ndg + 1))): k for k in range(ndg)} \
                    if npg else {}
                for i in range(max(npg, 1)):
                    if i < npg:
                        pgroups[i]()
                    if i == 3 and 1 <= c <= NCHUNK:
                        pool_chunk(c - 1)
                    if i in dpos:
                        dgroups[dpos[i]]()
                if not pgroups:
                    if 1 <= c <= NCHUNK:
                        pool_chunk(c - 1)
                    for g in dgroups:
                        g()
                if ax is not None:
                    store_ax(c - 2, ax)


        # ================= phase 2 =================
        with ExitStack() as ctx:
            p2 = ctx.enter_context(tc.tile_pool(name="p2", bufs=1))

            pv = p2.tile([128, 4, WIN, D], BF16, tag="pv")
            wtn = p2.tile([128, 4, WN], BF16, tag="wtn")
            pqT = p2.tile([128, 4, WN], BF16, tag="pqT")
            pkT = p2.tile([128, 4, WN], BF16, tag="pkT")
            esA = p2.tile([128, H, 4, WN], BF16, tag="esA")
            zt = p2.tile([128, 4, QLEN], BF16, tag="zt")
            recip_sb = p2.tile([128, H, 4], F32, tag="recips")

            # issue axt transposes + pv gathers in dependency-arrival order
            srcv = axd.ap().rearrange("(cc p w) d -> cc p w d", p=128, w=PW)
            for c in range(NCHUNK):
                issue_axt(c)
                if c % 2 == 1:
                    cc = c // 2
                    gi = nc.sync.dma_start(out=pv[:, cc, :, :], in_=srcv[cc, :, 1:PW, :])
                    add_dep_helper(gi.ins, axd_writers[2 * cc].ins, reason="pv gather")
                    add_dep_helper(gi.ins, axd_writers[2 * cc + 1].ins, reason="pv gather")

            # ---- win_tok (+bv) LN + GELU + pq/pk, per 128-window group ----
            with ExitStack() as lctx:
                ps_ln = lctx.enter_context(
                    tc.tile_pool(name="ps_ln", bufs=3, space="PSUM"))
                lnp = lctx.enter_context(tc.tile_pool(name="lnp", bufs=2))

                lnA = {}

                def ln_phase_a(g):
                    """Moments + rstd; ACT funcs all within one table set
                    (Identity/Square/Copy/Sqrt)."""
                    wt_g = axt[:, :, g * GW * PW:(g + 1) * GW * PW:PW]
                    wtb = lnp.tile([128, 4, GW], BF16, tag="wtb", bufs=4,
                                   name=f"wtb_{g}")
                    for j in range(4):
                        nc.scalar.activation(wtb[:, j, :], wt_g[:, j, :],
                                             AF.Identity,
                                             bias=bias_cols["bv_c"][:, j:j + 1],
                                             scale=1.0)
                    wsq = lnp.tile([128, 4, GW], BF16, tag="wsq", name=f"wsq_{g}")
                    nc.scalar.activation(wsq[:], wtb[:], AF.Square)
                    ps_mu = ps_ln.tile([128, GW], F32, tag="psln", name=f"psmu_{g}")
                    ps_var = ps_ln.tile([128, GW], F32, tag="psln", name=f"psvar_{g}")
                    for j in range(4):
                        nc.tensor.matmul(ps_mu[:], ones_full[:], wtb[:, j, :],
                                         start=(j == 0), stop=(j == 3),
                                         skip_group_check=True)
                        nc.tensor.matmul(ps_var[:], ones_full[:], wsq[:, j, :],
                                         start=(j == 0), stop=(j == 3),
                                         skip_group_check=True)
                    mu = lnp.tile([128, GW], F32, tag="mu", bufs=4, name=f"mu_{g}")
                    nc.scalar.mul(mu[:], ps_mu[:], 1.0 / D)
                    ex2 = lnp.tile([128, GW], F32, tag="ex2", bufs=1,
                                   name=f"ex2_{g}")
                    nc.scalar.mul(ex2[:], ps_var[:], 1.0 / D)
                    var = lnp.tile([128, GW], F32, tag="var", bufs=1,
                                   name=f"var_{g}")
                    nc.vector.tensor_mul(var[:], mu[:], mu[:])
                    nc.vector.tensor_sub(var[:], ex2[:], var[:])
                    sd = lnp.tile([128, GW], F32, tag="sd", bufs=1, name=f"sd_{g}")
                    nc.scalar.activation(sd[:], var[:], AF.Sqrt, bias=eps_sb[:])
                    rstd = lnp.tile([128, GW], F32, tag="rstd", bufs=4,
                                    name=f"rstd_{g}")
                    nc.vector.reciprocal(rstd[:], sd[:])
                    lnA[g] = (wtb, mu, rstd)

                def ln_phase_b(g):
                    """GELU + pq/pk projections (Gelu/Identity table set)."""
                    gs = g * GW
                    wtb, mu, rstd = lnA.pop(g)
                    for j in range(4):
                        tmp = lnp.tile([128, GW], F32, tag="lnt", name=f"lnt_{g}_{j}")
                        nc.vector.tensor_sub(tmp[:], wtb[:, j, :], mu[:])
                        nc.vector.tensor_mul(tmp[:], tmp[:], rstd[:])
                        nc.scalar.activation(wtn[:, j, gs:gs + GW], tmp[:],
                                             AF.Gelu,
                                             bias=bias_cols["ln_b_c"][:, j:j + 1],
                                             scale=bias_cols["ln_g_c"][:, j:j + 1])
                    for dst, wname, bname in ((pqT, "wpq", "bpq_c"),
                                              (pkT, "wpk", "bpk_c")):
                        for j in range(4):
                            ps = ps_ln.tile([128, GW], F32, tag="psln",
                                            name=f"pp_{wname}_{g}_{j}")
                            for dk in range(4):
                                nc.tensor.matmul(
                                    ps[:], W[wname][:, dk, j * 128:(j + 1) * 128],
                                    wtn[:, dk, gs:gs + GW],
                                    start=(dk == 0), stop=(dk == 3))
                            nc.vector.tensor_scalar_add(
                                dst[:, j, gs:gs + GW], ps[:],
                                bias_cols[bname][:, j:j + 1])

                # A0..A2 then B0..B2 (one Sqrt->Gelu table switch), then the
                # last group's A3+B3 pair on the critical path (one more
                # switch pair).
                for g in range(NG - 1):
                    ln_phase_a(g)
                for g in range(NG - 1):
                    ln_phase_b(g)
                ln_phase_a(NG - 1)
                ln_phase_b(NG - 1)

            # ---- PSA: raw exp scores; den via N=1 matmuls; window-major pout
            with ExitStack() as pctx:
                # PSUM budget (8 banks): es/fin share slots (disjoint
                # lifetimes, same shape) 2 + po 2 + ztps 2 + den 1 = 7.
                ps_es = pctx.enter_context(
                    tc.tile_pool(name="ps_es", bufs=2, space="PSUM"))
                ps_po = pctx.enter_context(
                    tc.tile_pool(name="ps_po", bufs=3, space="PSUM"))
                ps_ztden = pctx.enter_context(
                    tc.tile_pool(name="ps_ztden", bufs=2, space="PSUM"))
                ps_fin = ps_es
                zwp = pctx.enter_context(tc.tile_pool(name="zwp", bufs=3))
                ztp = pctx.enter_context(tc.tile_pool(name="ztp", bufs=2))
                osb = pctx.enter_context(tc.tile_pool(name="osb", bufs=4))

                def psa_scores(h):
                    base = (h % 2) * 64
                    for cc in range(4):
                        ps = ps_es.tile([128, WN], F32, tag="es",
                                    name=f"es_{h}_{cc}")
                        nc.tensor.matmul(
                            ps[:], pkT[base:base + 64, h // 2,
                                       cc * 128:(cc + 1) * 128],
                            pqT[base:base + 64, h // 2, :], start=True, stop=True)
                        nc.scalar.activation(esA[:, h, cc, :], ps[:], AF.Exp,
                                             scale=SCALE)

                den_ps = None

                def psa_den(h):
                    for qt in range(4):
                        idx = h * 4 + qt
                        for cc in range(4):
                            nc.tensor.matmul(
                                den_ps[:, idx:idx + 1],
                                esA[:, h, cc, qt * 128:(qt + 1) * 128],
                                ones_col[:], start=(cc == 0), stop=(cc == 3),
                                skip_group_check=True)
                    nc.vector.reciprocal(recip_sb[:, h, :],
                                         den_ps[:, h * 4:(h + 1) * 4])

                def pout_one(h, qt, zwin):
                    po = ps_po.tile([128, WIN, HD], F32, tag="po",
                                    name=f"po_{h}_{qt}")
                    for cc in range(4):
                        nc.tensor.matmul(
                            po[:], esA[:, h, cc, qt * 128:(qt + 1) * 128],
                            pv[:, cc, :, h * 64:(h + 1) * 64],
                            start=(cc == 0), stop=(cc == 3),
                            skip_group_check=True)
                    ztmp = ztp.tile([128, WIN, HD], BF16, tag="ztmp",
                                    name=f"ztmp_{h}_{qt}")
                    nc.vector.tensor_scalar_mul(ztmp[:], po[:],
                                                recip_sb[:, h, qt:qt + 1])
                    nc.vector.tensor_add(zwin[:, :, h * 64:(h + 1) * 64], ztmp[:],
                                         pv[:, qt, :, h * 64:(h + 1) * 64])

                def ztrans_one(qt, ii, zwin):
                    """Transpose payload slots ii..ii+1 (or just ii at the
                    tail) of group qt into feature-major zt."""
                    ni = min(2, WIN - ii)
                    zt_ps = ps_ztden.tile([128, 4, 2, 128], BF16, tag="ztps",
                                          name=f"ztps_{qt}_{ii}")
                    for di in range(ni):
                        for fg in range(4):
                            nc.tensor.transpose(
                                zt_ps[:, fg, di, :],
                                zwin[:, ii + di, fg * 128:(fg + 1) * 128],
                                ident_sb[:])
                    base = qt * GW * WIN
                    dst = zt[:, :, base + ii:base + GW * WIN:WIN]
                    dst = bass.AP(dst.tensor, dst.offset,
                                  [dst.ap[0], dst.ap[1], [1, ni], [WIN, 128]])
                    src_ap = zt_ps[:, :, 0:ni, :]
                    nc.scalar.copy(dst, src_ap)

                osb_tiles = {}

                def fin_one(tt):
                    ps = ps_fin.tile([128, D], F32, tag="es", name=f"fin_{tt}")
                    for dk in range(4):
                        nc.tensor.matmul(ps[:], zt[:, dk, tt * 128:(tt + 1) * 128],
                                         W["wo"][:, dk, :], start=(dk == 0),
                                         stop=(dk == 3), skip_group_check=True)
                    g = tt // 2
                    if tt % 2 == 0:
                        osb_tiles[g] = osb.tile([128, 2, D], F32, tag="osb",
                                                name=f"osb_{g}")
                    nc.vector.tensor_add(osb_tiles[g][:, tt % 2, :], ps[:],
                                         bo_sb[:])
                    if tt % 2 == 1:
                        outv = out.ap().rearrange("(g tt p) d -> g p tt d",
                                                  tt=2, p=128)
                        nc.sync.dma_start(out=outv[g], in_=osb_tiles.pop(g)[:])

                # head-outer pipeline: as soon as head h's exp-scores are
                # done, its denominators and all four pout groups flow; the
                # transposes + final projections drain afterwards per group.
                for h in range(H):
                    psa_scores(h)
                den_ps = ps_ztden.tile([128, H * 4], F32, tag="den", name="den",
                                       bufs=1)
                for h in range(H):
                    psa_den(h)

                zw = {}
                prev = None

                def tail_items(qt):
                    items = []
                    zwin_p = zw[qt]
                    for ii in range(0, WIN, 2):
                        items.append(lambda ii=ii: ztrans_one(qt, ii, zwin_p))
                    for j in range(WIN):
                        items.append(lambda j=j: fin_one(qt * WIN + j))
                    return items

                for qt in range(NG):
                    zw[qt] = zwp.tile([128, WIN, D], BF16, tag="zwin",
                                      name=f"zwin_{qt}")
                    titems = tail_items(prev) if prev is not None else []
                    ti = 0
                    for h in range(H):
                        pout_one(h, qt, zw[qt])
                        for _ in range(2):
                            if ti < len(titems) and (h * 8) // H >= ti:
                                titems[ti]()
                                ti += 1
                    while ti < len(titems):
                        titems[ti]()
                        ti += 1
                    if prev is not None:
                        zw.pop(prev)
                    prev = qt
                for it in tail_items(prev):
                    it()


_NC_CACHE = None


def _get_program():
    global _NC_CACHE
    if _NC_CACHE is None:
        _NC_CACHE = build_program()
    return _NC_CACHE


def _host_consts(Wk, bk, Wv, bv, Wq, bq, ln_g, ln_b, Wpq, bpq, Wpk, bpk, Wo, bo):
    bf = ml_dtypes.bfloat16
    col = lambda b: np.asarray(b, np.float32).reshape(4, 128).T.copy()
    bo2 = np.asarray(bo, np.float32) + 2.0 * (
        np.asarray(bv, np.float32) @ np.asarray(Wo, np.float32))
    consts = {
        "wq": np.asarray(Wq, np.float32).astype(bf),
        "wk": np.asarray(Wk, np.float32).astype(bf),
        "wv": np.asarray(Wv, np.float32).astype(bf),
        "wpq": np.asarray(Wpq, np.float32).astype(bf),
        "wpk": np.asarray(Wpk, np.float32).astype(bf),
        "wo": np.asarray(Wo, np.float32).astype(bf),
        "bq_c": col(bq), "bk_c": col(bk),
        "bpq_c": col(bpq), "bpk_c": col(bpk),
        "ln_g_c": col(ln_g), "ln_b_c": col(ln_b),
        "bv_c": col(bv),
        "bo_r": bo2.reshape(1, D).astype(bf),
        "ident": np.eye(128, dtype=np.float32).astype(bf),
    }
    m = np.zeros((128, 128), np.float32)
    for g in range(16):
        m[g * PW:(g + 1) * PW, g * PW:(g + 1) * PW] = 1.0
    consts["bmask"] = m.astype(bf)
    return consts


def kernel(k, v, q, query_len, Wk, bk, Wv, bv, Wq, bq, ln_g, ln_b,
           Wpq, bpq, Wpk, bpk, Wo, bo):
    nc = _get_program()
    consts = _host_consts(Wk, bk, Wv, bv, Wq, bq, ln_g, ln_b,
                          Wpq, bpq, Wpk, bpk, Wo, bo)
    k = np.asarray(k, np.float32)
    v = np.asarray(v, np.float32)
    q = np.asarray(q, np.float32)
    in_maps = []
    for b in range(B):
        m = {"q": np.ascontiguousarray(q[b]), "k": np.ascontiguousarray(k[b]),
             "v": np.ascontiguousarray(v[b])}
        m.update(consts)
        in_maps.append(m)
    res = run_bass_kernel_spmd(nc, in_maps, core_ids=list(range(B)))
    return np.stack([res.results[b]["out"] for b in range(B)], axis=0)


if __name__ == "__main__":
    nc = build_program()
    print("program built ok")


# revision 64
# speedup vs baseline: 1.5279x; 1.0047x over previous
"""Trainium2 Bass kernel for DeformableMultiHeadedAttention.

Data-parallel over batch B=8 across 8 NeuronCores (one batch element per
core, identical programs, no collectives).

Per-core pipeline (matmuls bf16 with f32 accumulate):
  1. q,k,v [4096,512] f32 -> SWDGE cast-DMA -> DRAM bf16 -> batched HWDGE
     DMA-transpose (one [512,512] xbar transpose per chunk) -> feature-major
     XT [128,4,tok] chunks in SBUF.
  2. Projections on PE: K'T/Q'T feature-major (lhsT=W, rhs=XT); V' token-major
     (lhsT=XT tile, rhs=W), bv folded out on host (bo' = bo + 2*bv@Wo, LN
     input gets +bv on chip).
  3. Q pooling (AvgPool k=5, stride 1, zero pad) as 3 shifted adds; the 1/5
     is folded into the softmax exp scale.
  4. DSA (windows of 8 tokens): per 128-token tile, 8 heads: S_T[k,q] on PE,
     exp on ACT, block-diag mask mul on DVE, attn@V plus ones-col denominator
     sharing the lhsT, per-partition 1/den scale on DVE. Token-major DSA
     output -> DRAM (bf16).
  5. DRAM round-trips: batched DMA-transpose -> attn_xT feature-major;
     strided gather -> PV window-major [kw, (slot, head, hd)].
  6. Incremental (per 128-window group, overlapping phase 1 tail): win_tok
     +bv, LayerNorm moments via ones-matmuls, exact GELU, pq/pk projections.
  7. PSA restructured: raw exp-scores kept unnormalized; denominators via
     N=1 matmuls against a ones column (per-partition 1/den on DVE); pout
     window-major [wq, (slot,hd)] at M=128 (half the PE rows of the
     feature-major form); z = pout*recip + attn_x in window-major form;
     PE identity-transposes + strided ACT copies build feature-major zT.
  8. out = Z @ Wo + bo' with Z as the stationary operand -> token-major f32
     output, streamed per window-group.
"""

import sys
from contextlib import ExitStack

for _p in ("/opt/trn_rl_repo/concourse", "/opt/trn_rl_repo"):
    if _p not in sys.path:
        sys.path.insert(0, _p)

import numpy as np
import ml_dtypes

import concourse.bass as bass
import concourse.mybir as mybir
import concourse.tile as tile
from concourse import bacc
from concourse.tile import add_dep_helper
from concourse.bass_utils import run_bass_kernel_spmd

BF16 = mybir.dt.bfloat16
F32 = mybir.dt.float32
AF = mybir.ActivationFunctionType
ALU = mybir.AluOpType

B, M, D = 8, 4096, 512
H, HD = 8, 64
WIN = 7
PW = WIN + 1
QNB = 5
QLEN = 3584
WN = M // PW
SCALE = D ** -0.5
EPS = 1e-5
NCHUNK = 8
CH = 512
NG = 4                   # window groups of 128 for phase 2
GW = WN // NG            # 128 windows per group
PERM = [(h % 2) * 4 + h // 2 for h in range(H)]  # head -> DSA psum slot


def build_program():
    nc = bacc.Bacc("TRN2", target_bir_lowering=False, debug=False, num_devices=8)

    t = {}
    t["q_in"] = nc.dram_tensor("q", [M, D], F32, kind="ExternalInput")
    t["k_in"] = nc.dram_tensor("k", [M, D], F32, kind="ExternalInput")
    t["v_in"] = nc.dram_tensor("v", [M, D], F32, kind="ExternalInput")
    for nm in ("wq", "wk", "wv", "wpq", "wpk", "wo"):
        t[nm] = nc.dram_tensor(nm, [D, D], BF16, kind="ExternalInput")
    for nm in ("bq_c", "bk_c", "bpq_c", "bpk_c", "ln_g_c", "ln_b_c", "bv_c"):
        t[nm] = nc.dram_tensor(nm, [128, 4], F32, kind="ExternalInput")
    t["bo_r"] = nc.dram_tensor("bo_r", [1, D], BF16, kind="ExternalInput")
    t["bmask"] = nc.dram_tensor("bmask", [128, 128], BF16, kind="ExternalInput")
    t["ident"] = nc.dram_tensor("ident", [128, 128], BF16, kind="ExternalInput")
    t["out"] = nc.dram_tensor("out", [QLEN, D], F32, kind="ExternalOutput")
    t["axd"] = nc.dram_tensor("axd_s", [M, D], BF16, kind="Internal")
    t["zd"] = nc.dram_tensor("zd_s", [QLEN, D], BF16, kind="Internal")
    t["qb"] = nc.dram_tensor("qb_s", [M, D], BF16, kind="Internal")
    t["kb"] = nc.dram_tensor("kb_s", [M, D], BF16, kind="Internal")
    t["vb"] = nc.dram_tensor("vb_s", [M, D], BF16, kind="Internal")

    with tile.TileContext(nc) as tc:
        _build(nc, tc, t)
    nc.compile()
    return nc


def _build(nc, tc, t):
    qb, kb, vb = t["qb"], t["kb"], t["vb"]
    axd, out = t["axd"], t["out"]
    zd = t["zd"]

    with ExitStack() as octx:
        singles = octx.enter_context(tc.tile_pool(name="singles", bufs=1))

        # phase-1 weights first (needed by the first projections), then the
        # input casts, then everything else so the casts win the DMA engines.
        cast_insts = {"q": [], "k": [], "v": []}

        def issue_casts(lo, hi):
            for nm, srcd, dst in (("q", t["q_in"], qb), ("k", t["k_in"], kb),
                                  ("v", t["v_in"], vb)):
                ci = nc.gpsimd.dma_start(
                    out=dst[lo * CH:hi * CH, :],
                    in_=srcd[lo * CH:hi * CH, :])
                cast_insts[nm].append(((lo, hi), ci))

        issue_casts(0, 1)
        W = {}
        for nm in ("wq", "wk", "wv"):
            W[nm] = singles.tile([128, 4, D], BF16, tag=nm, name=f"w_{nm}")
            nc.scalar.dma_start(out=W[nm][:],
                                in_=t[nm].ap().rearrange("(c p) d -> p c d", p=128))
        issue_casts(1, 2)

        bias_cols = {}
        for nm in ("bq_c", "bk_c"):
            bias_cols[nm] = singles.tile([128, 4], F32, tag=nm, name=f"bc_{nm}")
            nc.scalar.dma_start(out=bias_cols[nm][:], in_=t[nm][:, :])
        mask_sb = singles.tile([128, 128], BF16)
        nc.scalar.dma_start(out=mask_sb[:], in_=t["bmask"][:, :])
        ones_row = singles.tile([1, 128], BF16)
        nc.vector.memset(ones_row[:], 1.0)
        ones_col = singles.tile([128, 1], BF16)
        nc.vector.memset(ones_col[:], 1.0)
        ones_full = singles.tile([128, 128], BF16)
        nc.vector.memset(ones_full[:], 1.0)
        eps_sb = singles.tile([128, 1], F32)
        nc.vector.memset(eps_sb[:], EPS)

        issue_casts(2, 4)
        issue_casts(4, 6)
        issue_casts(6, 8)

        axd_writers = {}
        p2a = octx.enter_context(tc.tile_pool(name="p2a", bufs=1))
        axt = p2a.tile([128, 4, M], BF16, tag="axt")

        # ================= phase 2 weights ==============================
        # deferred behind the early input casts so they don't hog the DMA
        # engines during the pipeline ramp
        # Allocated here; DMAs issued mid-phase-1 (see chunk loop, c==3)
        # on the sync queue so they neither hog the DMA engines at startup
        # nor get scheduled into the phase boundary.
        for nm in ("wpq", "wpk", "wo"):
            W[nm] = singles.tile([128, 4, D], BF16, tag=nm, name=f"w_{nm}")
        for nm in ("bpq_c", "bpk_c", "ln_g_c", "ln_b_c", "bv_c"):
            bias_cols[nm] = singles.tile([128, 4], F32, tag=nm, name=f"bc_{nm}")
        bo_sb = singles.tile([128, D], BF16)
        ident_sb = singles.tile([128, 128], BF16)

        def load_p2_consts(gate):
            dis = []
            for nm in ("wpq", "wpk", "wo"):
                dis.append(nc.sync.dma_start(
                    out=W[nm][:],
                    in_=t[nm].ap().rearrange("(c p) d -> p c d", p=128)))
            for nm in ("bpq_c", "bpk_c", "ln_g_c", "ln_b_c", "bv_c"):
                dis.append(nc.sync.dma_start(out=bias_cols[nm][:], in_=t[nm][:, :]))
            dis.append(nc.sync.dma_start(
                out=bo_sb[:],
                in_=t["bo_r"].ap().to_broadcast((128, D))))
            dis.append(nc.sync.dma_start(out=ident_sb[:], in_=t["ident"][:, :]))
            for di in dis:
                add_dep_helper(di.ins, gate.ins,
                               reason="const loads after startup transposes")

        # ================= phase 1 =================
        with ExitStack() as ctx:
            p1 = ctx.enter_context(tc.tile_pool(name="p1", bufs=1))
            kT = p1.tile([128, 4, 3, CH], BF16, tag="kT")        # ring of 3 chunks
            qpT = p1.tile([128, 4, 3, CH], BF16, tag="qpT")      # ring of 3 chunks
            vtm = p1.tile([128, 12, 8, 65], BF16, tag="vtm")     # ring of 12 tiles, 65-col/head
            nc.vector.memset(vtm[:, :, :, 64:65], 1.0)           # ones col for denominators
            qraw = p1.tile([128, 4, M + 4], BF16, tag="qraw")    # full, padded +-2
            nc.vector.memset(qraw[:, :, 0:2], 0.0)
            nc.vector.memset(qraw[:, :, M + 2:M + 4], 0.0)

            xtp = ctx.enter_context(tc.tile_pool(name="xtp", bufs=2))
            ps_proj = ctx.enter_context(tc.tile_pool(name="ps_proj", bufs=2, space="PSUM"))
            ps_st = ctx.enter_context(tc.tile_pool(name="ps_st", bufs=2, space="PSUM"))
            ps_out = ctx.enter_context(tc.tile_pool(name="ps_out", bufs=1, space="PSUM"))
            dsa_sb = ctx.enter_context(tc.tile_pool(name="dsa_sb", bufs=2))
            pool_tmp = ctx.enter_context(tc.tile_pool(name="pool_tmp", bufs=2))
            ax_pool = ctx.enter_context(tc.tile_pool(name="ax_sb", bufs=2))

            xt_gate = {}

            def load_xt(nm, dram, c):
                xt = xtp.tile([128, 4, CH], BF16, tag=f"xt_{nm}", name=f"xt_{nm}_{c}")
                ti = nc.sync.dma_start(out=xt[:],
                                       in_=dram[c * CH:(c + 1) * CH, :],
                                       transpose=True)
                for (lo, hi), ci in cast_insts[nm]:
                    if lo <= c < hi:
                        add_dep_helper(ti.ins, ci.ins,
                                       reason="transpose reads cast output")
                xt_gate[(nm, c)] = ti
                return xt

            def proj_fm_group(xt, wname, bname, dst_fn, j):
                ps = ps_proj.tile([128, CH], F32, tag="proj",
                                  name=f"ps_{wname}_{j}")
                for dk in range(4):
                    nc.tensor.matmul(ps[:], W[wname][:, dk, j * 128:(j + 1) * 128],
                                     xt[:, dk, :], start=(dk == 0), stop=(dk == 3))
                nc.scalar.activation(dst_fn(j), ps[:], AF.Identity,
                                     bias=bias_cols[bname][:, j:j + 1], scale=1.0)

            def proj_v_group(xt, c, tt):
                ps = ps_proj.tile([128, D], F32, tag="proj", name=f"ps_v_{tt}")
                for dk in range(4):
                    nc.tensor.matmul(ps[:], xt[:, dk, tt * 128:(tt + 1) * 128],
                                     W["wv"][:, dk, :], start=(dk == 0),
                                     stop=(dk == 3), skip_group_check=True)
                nc.scalar.copy(vtm[:, (c * 4 + tt) % 12, :, 0:64],
                               ps[:].rearrange("p (h d) -> p h d", h=H))

            def pool_chunk(c):
                base = c * CH
                ta = pool_tmp.tile([128, 4, CH + 2], BF16, tag="ta")
                nc.vector.tensor_add(ta[:], qraw[:, :, base:base + CH + 2],
                                     qraw[:, :, base + 1:base + CH + 3])
                tb = pool_tmp.tile([128, 4, CH], BF16, tag="tb")
                nc.vector.tensor_add(tb[:], ta[:, :, 0:CH], ta[:, :, 2:CH + 2])
                nc.vector.tensor_add(qpT[:, :, c % 3, :], tb[:],
                                     qraw[:, :, base + 4:base + CH + 4])

            def dsa_scores(c, lt):
                """MM1 + exp + mask for tile lt of chunk c -> masked sbuf tile."""
                st = ps_st.tile([128, 8, 128], F32, tag="st", name=f"st_{c}_{lt}")
                for h in range(H):
                    hp = PERM[h]
                    base = (h % 2) * 64
                    lhsT = kT[base:base + 64, h // 2, c % 3, lt * 128:(lt + 1) * 128]
                    rhs = qpT[base:base + 64, h // 2, c % 3, lt * 128:(lt + 1) * 128]
                    nc.tensor.matmul(st[:, hp, :], lhsT, rhs, start=True, stop=True,
                                     skip_group_check=True)
                expS = dsa_sb.tile([128, 8, 128], BF16, tag="expS",
                                   name=f"expS_{c}_{lt}")
                nc.scalar.activation(expS[:], st[:], AF.Exp, scale=SCALE / QNB)
                masked = dsa_sb.tile([128, 8, 128], BF16, tag="masked",
                                     name=f"masked_{c}_{lt}")
                nc.vector.tensor_mul(masked[:], expS[:],
                                     mask_sb[:].unsqueeze(1).to_broadcast((128, 8, 128)))
                return masked

            def dsa_out(c, lt, masked, ax_out):
                """attn@V with ones-col denominators, then normalize."""
                outp = ps_out.tile([128, 2, 512], F32, tag="outp",
                                   name=f"outp_{c}_{lt}")
                for h in range(H):
                    hp = PERM[h]
                    nc.tensor.matmul(outp[:, h // 4, (h % 4) * 65:(h % 4) * 65 + 65],
                                     masked[:, hp, :],
                                     vtm[:, (c * 4 + lt) % 12, h, :],
                                     start=True, stop=True, skip_group_check=True)
                recip = dsa_sb.tile([128, 2, 4], F32, tag="recip",
                                    name=f"recip_{c}_{lt}")
                den_view = bass.AP(outp.tensor, outp[:].offset + 64,
                                   [outp[:].ap[0], [512, 2], [65, 4]])
                nc.vector.reciprocal(recip[:], den_view)
                av_view = bass.AP(outp.tensor, outp[:].offset,
                                  [outp[:].ap[0], [512, 2], [65, 4], [1, 64]])
                nc.vector.tensor_mul(
                    ax_out.rearrange("p (a b d) -> p a b d", a=2, b=4),
                    av_view,
                    recip[:].unsqueeze(3).to_broadcast((128, 2, 4, 64)))

            def dsa_group_list(c, ax):
                masked = {}
                g = []
                g.append(lambda: masked.__setitem__(0, dsa_scores(c, 0)))
                g.append(lambda: masked.__setitem__(1, dsa_scores(c, 1)))
                g.append(lambda: dsa_out(c, 0, masked.pop(0), ax[:, 0, :]))
                g.append(lambda: masked.__setitem__(2, dsa_scores(c, 2)))
                g.append(lambda: dsa_out(c, 1, masked.pop(1), ax[:, 1, :]))
                g.append(lambda: masked.__setitem__(3, dsa_scores(c, 3)))
                g.append(lambda: dsa_out(c, 2, masked.pop(2), ax[:, 2, :]))
                g.append(lambda: dsa_out(c, 3, masked.pop(3), ax[:, 3, :]))
                return g

            def store_ax(c, ax):
                dst = axd.ap().rearrange("(cc lt p) d -> cc p lt d", lt=4, p=128)[c]
                wi = nc.gpsimd.dma_start(out=dst, in_=ax[:])
                axd_writers[c] = wi

            def issue_axt(c):
                ti = nc.sync.dma_start(
                    out=axt[:, :, c * CH:(c + 1) * CH],
                    in_=axd[c * CH:(c + 1) * CH, :],
                    transpose=True)
                add_dep_helper(ti.ins, axd_writers[c].ins,
                               reason="axt transpose reads axd chunk")

            for c in range(NCHUNK + 2):
                pgroups = []
                if c < NCHUNK:
                    qxt = load_xt("q", qb, c)
                    kxt = load_xt("k", kb, c)
                    vxt = load_xt("v", vb, c)
                    if c == 3:
                        load_p2_consts(xt_gate[("q", 2)])
                    for j in range(4):
                        pgroups.append(lambda j=j, x=qxt, c=c: proj_fm_group(
                            x, "wq", "bq_c",
                            lambda jj, c=c: qraw[:, jj, 2 + c * CH:2 + (c + 1) * CH], j))
                    for j in range(4):
                        pgroups.append(lambda j=j, x=kxt, c=c: proj_fm_group(
                            x, "wk", "bk_c", lambda jj, c=c: kT[:, jj, c % 3, :], j))
                    for tt in range(4):
                        pgroups.append(lambda tt=tt, x=vxt, c=c: proj_v_group(x, c, tt))
                dgroups = []
                ax = None
                if c >= 2:
                    ax = ax_pool.tile([128, 4, D], BF16, tag="ax", name=f"ax_{c - 2}")
                    dgroups = dsa_group_list(c - 2, ax)
                # weave: spread D groups evenly through the P stream;
                # pool(c-1) after the 4 Q-projection groups
                npg, ndg = len(pgroups), len(dgroups)
                dpos = {int(round((k + 1) * npg / (ndg + 1))): k for k in range(ndg)} \
                    if npg else {}
                for i in range(max(npg, 1)):
                    if i < npg:
                        pgroups[i]()
                    if i == 3 and 1 <= c <= NCHUNK:
                        pool_chunk(c - 1)
                    if i in dpos:
                        dgroups[dpos[i]]()
                if not pgroups:
                    if 1 <= c <= NCHUNK:
                        pool_chunk(c - 1)
                    for g in dgroups:
                        g()
                if ax is not None:
                    store_ax(c - 2, ax)


        # ================= phase 2 =================
        with ExitStack() as ctx:
            p2 = ctx.enter_context(tc.tile_pool(name="p2", bufs=1))

            pv = p2.tile([128, 4, WIN, D], BF16, tag="pv")
            wtn = p2.tile([128, 4, WN], BF16, tag="wtn")
            pqT = p2.tile([128, 4, WN], BF16, tag="pqT")
            pkT = p2.tile([128, 4, WN], BF16, tag="pkT")
            esA = p2.tile([128, H, 4, WN], BF16, tag="esA")
            zt = p2.tile([128, 4, QLEN], BF16, tag="zt")
            recip_sb = p2.tile([128, H, 4], F32, tag="recips")

            # issue axt transposes + pv gathers in dependency-arrival order
            srcv = axd.ap().rearrange("(cc p w) d -> cc p w d", p=128, w=PW)
            for c in range(NCHUNK):
                issue_axt(c)
                if c % 2 == 1:
                    cc = c // 2
                    gi = nc.sync.dma_start(out=pv[:, cc, :, :], in_=srcv[cc, :, 1:PW, :])
                    add_dep_helper(gi.ins, axd_writers[2 * cc].ins, reason="pv gather")
                    add_dep_helper(gi.ins, axd_writers[2 * cc + 1].ins, reason="pv gather")

            # ---- win_tok (+bv) LN + GELU + pq/pk, per 128-window group ----
            with ExitStack() as lctx:
                ps_ln = lctx.enter_context(
                    tc.tile_pool(name="ps_ln", bufs=3, space="PSUM"))
                lnp = lctx.enter_context(tc.tile_pool(name="lnp", bufs=2))

                lnA = {}

                def ln_phase_a(g):
                    """Moments + rstd; ACT funcs all within one table set
                    (Identity/Square/Copy/Sqrt)."""
                    wt_g = axt[:, :, g * GW * PW:(g + 1) * GW * PW:PW]
                    wtb = lnp.tile([128, 4, GW], BF16, tag="wtb", bufs=4,
                                   name=f"wtb_{g}")
                    for j in range(4):
                        nc.scalar.activation(wtb[:, j, :], wt_g[:, j, :],
                                             AF.Identity,
                                             bias=bias_cols["bv_c"][:, j:j + 1],
                                             scale=1.0)
                    wsq = lnp.tile([128, 4, GW], BF16, tag="wsq", name=f"wsq_{g}")
                    nc.scalar.activation(wsq[:], wtb[:], AF.Square)
                    ps_mu = ps_ln.tile([128, GW], F32, tag="psln", name=f"psmu_{g}")
                    ps_var = ps_ln.tile([128, GW], F32, tag="psln", name=f"psvar_{g}")
                    for j in range(4):
                        nc.tensor.matmul(ps_mu[:], ones_full[:], wtb[:, j, :],
                                         start=(j == 0), stop=(j == 3),
                                         skip_group_check=True)
                        nc.tensor.matmul(ps_var[:], ones_full[:], wsq[:, j, :],
                                         start=(j == 0), stop=(j == 3),
                                         skip_group_check=True)
                    mu = lnp.tile([128, GW], F32, tag="mu", bufs=4, name=f"mu_{g}")
                    nc.scalar.mul(mu[:], ps_mu[:], 1.0 / D)
                    ex2 = lnp.tile([128, GW], F32, tag="ex2", bufs=1,
                                   name=f"ex2_{g}")
                    nc.scalar.mul(ex2[:], ps_var[:], 1.0 / D)
                    var = lnp.tile([128, GW], F32, tag="var", bufs=1,
                                   name=f"var_{g}")
                    nc.vector.tensor_mul(var[:], mu[:], mu[:])
                    nc.vector.tensor_sub(var[:], ex2[:], var[:])
                    sd = lnp.tile([128, GW], F32, tag="sd", bufs=1, name=f"sd_{g}")
                    nc.scalar.activation(sd[:], var[:], AF.Sqrt, bias=eps_sb[:])
                    rstd = lnp.tile([128, GW], F32, tag="rstd", bufs=4,
                                    name=f"rstd_{g}")
                    nc.vector.reciprocal(rstd[:], sd[:])
                    lnA[g] = (wtb, mu, rstd)

                def ln_phase_b(g):
                    """GELU + pq/pk projections (Gelu/Identity table set)."""
                    gs = g * GW
                    wtb, mu, rstd = lnA.pop(g)
                    for j in range(4):
                        tmp = lnp.tile([128, GW], F32, tag="lnt", name=f"lnt_{g}_{j}")
                        nc.vector.tensor_sub(tmp[:], wtb[:, j, :], mu[:])
                        nc.vector.tensor_mul(tmp[:], tmp[:], rstd[:])
                        nc.scalar.activation(wtn[:, j, gs:gs + GW], tmp[:],
                                             AF.Gelu,
                                             bias=bias_cols["ln_b_c"][:, j:j + 1],
                                             scale=bias_cols["ln_g_c"][:, j:j + 1])
                    for dst, wname, bname in ((pqT, "wpq", "bpq_c"),
                                              (pkT, "wpk", "bpk_c")):
                        for j in range(4):
                            ps = ps_ln.tile([128, GW], F32, tag="psln",
                                            name=f"pp_{wname}_{g}_{j}")
                            for dk in range(4):
                                nc.tensor.matmul(
                                    ps[:], W[wname][:, dk, j * 128:(j + 1) * 128],
                                    wtn[:, dk, gs:gs + GW],
                                    start=(dk == 0), stop=(dk == 3))
                            nc.vector.tensor_scalar_add(
                                dst[:, j, gs:gs + GW], ps[:],
                                bias_cols[bname][:, j:j + 1])

                # A0..A2 then B0..B2 (one Sqrt->Gelu table switch), then the
                # last group's A3+B3 pair on the critical path (one more
                # switch pair).
                for g in range(NG - 1):
                    ln_phase_a(g)
                for g in range(NG - 1):
                    ln_phase_b(g)
                ln_phase_a(NG - 1)
                ln_phase_b(NG - 1)

            # ---- PSA: raw exp scores; den via N=1 matmuls; window-major pout
            with ExitStack() as pctx:
                # PSUM budget (8 banks): es/fin share slots (disjoint
                # lifetimes, same shape) 2 + po 2 + ztps 2 + den 1 = 7.
                ps_es = pctx.enter_context(
                    tc.tile_pool(name="ps_es", bufs=2, space="PSUM"))
                ps_po = pctx.enter_context(
                    tc.tile_pool(name="ps_po", bufs=2, space="PSUM"))
                ps_ztden = pctx.enter_context(
                    tc.tile_pool(name="ps_ztden", bufs=2, space="PSUM"))
                ps_fin = ps_es
                zwp = pctx.enter_context(tc.tile_pool(name="zwp", bufs=3))
                ztp = pctx.enter_context(tc.tile_pool(name="ztp", bufs=2))
                osb = pctx.enter_context(tc.tile_pool(name="osb", bufs=4))

                def psa_scores(h):
                    base = (h % 2) * 64
                    for cp in range(2):
                        ps = ps_es.tile([128, 2, WN], F32, tag="es",
                                        name=f"es_{h}_{cp}")
                        for k in range(2):
                            cc = cp * 2 + k
                            nc.tensor.matmul(
                                ps[:, k, :], pkT[base:base + 64, h // 2,
                                                 cc * 128:(cc + 1) * 128],
                                pqT[base:base + 64, h // 2, :],
                                start=True, stop=True, skip_group_check=True)
                        nc.scalar.activation(esA[:, h, 2 * cp:2 * cp + 2, :],
                                             ps[:], AF.Exp, scale=SCALE)

                def pout_one(h, qt, zwin):
                    # [pout | den] share one PSUM bank: cols 0:448 accumulate
                    # raw-exp attn @ pv, col 448 accumulates the softmax
                    # denominator against a ones column.
                    po = ps_po.tile([128, WIN * HD + 1], F32, tag="po",
                                    name=f"po_{h}_{qt}")
                    pov = po[:, 0:WIN * HD].rearrange("p (i d) -> p i d", i=WIN)
                    for cc in range(4):
                        nc.tensor.matmul(
                            pov, esA[:, h, cc, qt * 128:(qt + 1) * 128],
                            pv[:, cc, :, h * 64:(h + 1) * 64],
                            start=(cc == 0), stop=(cc == 3),
                            skip_group_check=True)
                    for cc in range(4):
                        nc.tensor.matmul(
                            po[:, WIN * HD:WIN * HD + 1],
                            esA[:, h, cc, qt * 128:(qt + 1) * 128],
                            ones_col[:], start=(cc == 0), stop=(cc == 3),
                            skip_group_check=True)
                    rc = recip_sb[:, h, qt:qt + 1]
                    nc.vector.reciprocal(rc, po[:, WIN * HD:WIN * HD + 1])
                    ztmp = ztp.tile([128, WIN, HD], BF16, tag="ztmp",
                                    name=f"ztmp_{h}_{qt}")
                    nc.vector.tensor_scalar_mul(ztmp[:], pov, rc)
                    nc.vector.tensor_add(zwin[:, :, h * 64:(h + 1) * 64], ztmp[:],
                                         pv[:, qt, :, h * 64:(h + 1) * 64])

                def ztrans_one(qt, ii, zwin):
                    """Transpose payload slots ii..ii+1 (or just ii at the
                    tail) of group qt into feature-major zt."""
                    ni = min(2, WIN - ii)
                    zt_ps = ps_ztden.tile([128, 4, 2, 128], BF16, tag="ztps",
                                          name=f"ztps_{qt}_{ii}")
                    for di in range(ni):
                        for fg in range(4):
                            nc.tensor.transpose(
                                zt_ps[:, fg, di, :],
                                zwin[:, ii + di, fg * 128:(fg + 1) * 128],
                                ident_sb[:])
                    base = qt * GW * WIN
                    dst = zt[:, :, base + ii:base + GW * WIN:WIN]
                    dst = bass.AP(dst.tensor, dst.offset,
                                  [dst.ap[0], dst.ap[1], [1, ni], [WIN, 128]])
                    src_ap = zt_ps[:, :, 0:ni, :]
                    nc.scalar.copy(dst, src_ap)

                osb_tiles = {}

                def fin_one(tt):
                    psf = ps_fin.tile([128, 2, WN], F32, tag="es",
                                      name=f"fin_{tt}")
                    ps = psf[:, 0, :]
                    for dk in range(4):
                        nc.tensor.matmul(ps, zt[:, dk, tt * 128:(tt + 1) * 128],
                                         W["wo"][:, dk, :], start=(dk == 0),
                                         stop=(dk == 3), skip_group_check=True)
                    g = tt // 2
                    if tt % 2 == 0:
                        osb_tiles[g] = osb.tile([128, 2, D], F32, tag="osb",
                                                name=f"osb_{g}")
                    nc.vector.tensor_add(osb_tiles[g][:, tt % 2, :], ps,
                                         bo_sb[:])
                    if tt % 2 == 1:
                        outv = out.ap().rearrange("(g tt p) d -> g p tt d",
                                                  tt=2, p=128)
                        nc.sync.dma_start(out=outv[g], in_=osb_tiles.pop(g)[:])

                # head-outer pipeline: as soon as head h's exp-scores are
                # done, its denominators and all four pout groups flow; the
                # transposes + final projections drain afterwards per group.
                for h in range(H):
                    psa_scores(h)

                zw = {}
                prev = None

                def tail_items(qt):
                    items = []
                    zwin_p = zw[qt]
                    for ii in range(0, WIN, 2):
                        items.append(lambda ii=ii: ztrans_one(qt, ii, zwin_p))
                    for j in range(WIN):
                        items.append(lambda j=j: fin_one(qt * WIN + j))
                    return items

                for qt in range(NG):
                    zw[qt] = zwp.tile([128, WIN, D], BF16, tag="zwin",
                                      name=f"zwin_{qt}")
                    titems = tail_items(prev) if prev is not None else []
                    ti = 0
                    for h in range(H):
                        pout_one(h, qt, zw[qt])
                        for _ in range(2):
                            if ti < len(titems) and (h * 8) // H >= ti:
                                titems[ti]()
                                ti += 1
                    while ti < len(titems):
                        titems[ti]()
                        ti += 1
                    if prev is not None:
                        zw.pop(prev)
                    prev = qt
                for it in tail_items(prev):
                    it()


_NC_CACHE = None


def _get_program():
    global _NC_CACHE
    if _NC_CACHE is None:
        _NC_CACHE = build_program()
    return _NC_CACHE


def _host_consts(Wk, bk, Wv, bv, Wq, bq, ln_g, ln_b, Wpq, bpq, Wpk, bpk, Wo, bo):
    bf = ml_dtypes.bfloat16
    col = lambda b: np.asarray(b, np.float32).reshape(4, 128).T.copy()
    bo2 = np.asarray(bo, np.float32) + 2.0 * (
        np.asarray(bv, np.float32) @ np.asarray(Wo, np.float32))
    consts = {
        "wq": np.asarray(Wq, np.float32).astype(bf),
        "wk": np.asarray(Wk, np.float32).astype(bf),
        "wv": np.asarray(Wv, np.float32).astype(bf),
        "wpq": np.asarray(Wpq, np.float32).astype(bf),
        "wpk": np.asarray(Wpk, np.float32).astype(bf),
        "wo": np.asarray(Wo, np.float32).astype(bf),
        "bq_c": col(bq), "bk_c": col(bk),
        "bpq_c": col(bpq), "bpk_c": col(bpk),
        "ln_g_c": col(ln_g), "ln_b_c": col(ln_b),
        "bv_c": col(bv),
        "bo_r": bo2.reshape(1, D).astype(bf),
        "ident": np.eye(128, dtype=np.float32).astype(bf),
    }
    m = np.zeros((128, 128), np.float32)
    for g in range(16):
        m[g * PW:(g + 1) * PW, g * PW:(g + 1) * PW] = 1.0
    consts["bmask"] = m.astype(bf)
    return consts


def kernel(k, v, q, query_len, Wk, bk, Wv, bv, Wq, bq, ln_g, ln_b,
           Wpq, bpq, Wpk, bpk, Wo, bo):
    nc = _get_program()
    consts = _host_consts(Wk, bk, Wv, bv, Wq, bq, ln_g, ln_b,
                          Wpq, bpq, Wpk, bpk, Wo, bo)
    k = np.asarray(k, np.float32)
    v = np.asarray(v, np.float32)
    q = np.asarray(q, np.float32)
    in_maps = []
    for b in range(B):
        m = {"q": np.ascontiguousarray(q[b]), "k": np.ascontiguousarray(k[b]),
             "v": np.ascontiguousarray(v[b])}
        m.update(consts)
        in_maps.append(m)
    res = run_bass_kernel_spmd(nc, in_maps, core_ids=list(range(B)))
    return np.stack([res.results[b]["out"] for b in range(B)], axis=0)


if __name__ == "__main__":
    nc = build_program()
    print("program built ok")


# revision 66
# speedup vs baseline: 1.5294x; 1.0010x over previous
"""Trainium2 Bass kernel for DeformableMultiHeadedAttention.

Data-parallel over batch B=8 across 8 NeuronCores (one batch element per
core, identical programs, no collectives).

Per-core pipeline (matmuls bf16 with f32 accumulate):
  1. q,k,v [4096,512] f32 -> SWDGE cast-DMA -> DRAM bf16 -> batched HWDGE
     DMA-transpose (one [512,512] xbar transpose per chunk) -> feature-major
     XT [128,4,tok] chunks in SBUF.
  2. Projections on PE: K'T/Q'T feature-major (lhsT=W, rhs=XT); V' token-major
     (lhsT=XT tile, rhs=W), bv folded out on host (bo' = bo + 2*bv@Wo, LN
     input gets +bv on chip).
  3. Q pooling (AvgPool k=5, stride 1, zero pad) as 3 shifted adds; the 1/5
     is folded into the softmax exp scale.
  4. DSA (windows of 8 tokens): per 128-token tile, 8 heads: S_T[k,q] on PE,
     exp on ACT, block-diag mask mul on DVE, attn@V plus ones-col denominator
     sharing the lhsT, per-partition 1/den scale on DVE. Token-major DSA
     output -> DRAM (bf16).
  5. DRAM round-trips: batched DMA-transpose -> attn_xT feature-major;
     strided gather -> PV window-major [kw, (slot, head, hd)].
  6. Incremental (per 128-window group, overlapping phase 1 tail): win_tok
     +bv, LayerNorm moments via ones-matmuls, exact GELU, pq/pk projections.
  7. PSA restructured: raw exp-scores kept unnormalized; denominators via
     N=1 matmuls against a ones column (per-partition 1/den on DVE); pout
     window-major [wq, (slot,hd)] at M=128 (half the PE rows of the
     feature-major form); z = pout*recip + attn_x in window-major form;
     PE identity-transposes + strided ACT copies build feature-major zT.
  8. out = Z @ Wo + bo' with Z as the stationary operand -> token-major f32
     output, streamed per window-group.
"""

import sys
from contextlib import ExitStack

for _p in ("/opt/trn_rl_repo/concourse", "/opt/trn_rl_repo"):
    if _p not in sys.path:
        sys.path.insert(0, _p)

import numpy as np
import ml_dtypes

import concourse.bass as bass
import concourse.mybir as mybir
import concourse.tile as tile
from concourse import bacc
from concourse.tile import add_dep_helper
from concourse.bass_utils import run_bass_kernel_spmd

BF16 = mybir.dt.bfloat16
F32 = mybir.dt.float32
AF = mybir.ActivationFunctionType
ALU = mybir.AluOpType

B, M, D = 8, 4096, 512
H, HD = 8, 64
WIN = 7
PW = WIN + 1
QNB = 5
QLEN = 3584
WN = M // PW
SCALE = D ** -0.5
EPS = 1e-5
NCHUNK = 8
CH = 512
NG = 4                   # window groups of 128 for phase 2
GW = WN // NG            # 128 windows per group
PERM = [(h % 2) * 4 + h // 2 for h in range(H)]  # head -> DSA psum slot


def build_program():
    nc = bacc.Bacc("TRN2", target_bir_lowering=False, debug=False, num_devices=8)

    t = {}
    t["q_in"] = nc.dram_tensor("q", [M, D], F32, kind="ExternalInput")
    t["k_in"] = nc.dram_tensor("k", [M, D], F32, kind="ExternalInput")
    t["v_in"] = nc.dram_tensor("v", [M, D], F32, kind="ExternalInput")
    for nm in ("wq", "wk", "wv", "wpq", "wpk", "wo"):
        t[nm] = nc.dram_tensor(nm, [D, D], BF16, kind="ExternalInput")
    for nm in ("bq_c", "bk_c", "bpq_c", "bpk_c", "ln_g_c", "ln_b_c", "bv_c"):
        t[nm] = nc.dram_tensor(nm, [128, 4], F32, kind="ExternalInput")
    t["bo_r"] = nc.dram_tensor("bo_r", [1, D], BF16, kind="ExternalInput")
    t["bmask"] = nc.dram_tensor("bmask", [128, 128], BF16, kind="ExternalInput")
    t["ident"] = nc.dram_tensor("ident", [128, 128], BF16, kind="ExternalInput")
    t["out"] = nc.dram_tensor("out", [QLEN, D], F32, kind="ExternalOutput")
    t["axd"] = nc.dram_tensor("axd_s", [M, D], BF16, kind="Internal")
    t["zd"] = nc.dram_tensor("zd_s", [QLEN, D], BF16, kind="Internal")
    t["qb"] = nc.dram_tensor("qb_s", [M, D], BF16, kind="Internal")
    t["kb"] = nc.dram_tensor("kb_s", [M, D], BF16, kind="Internal")
    t["vb"] = nc.dram_tensor("vb_s", [M, D], BF16, kind="Internal")

    with tile.TileContext(nc) as tc:
        _build(nc, tc, t)
    nc.compile()
    return nc


def _build(nc, tc, t):
    qb, kb, vb = t["qb"], t["kb"], t["vb"]
    axd, out = t["axd"], t["out"]
    zd = t["zd"]

    with ExitStack() as octx:
        singles = octx.enter_context(tc.tile_pool(name="singles", bufs=1))

        # phase-1 weights first (needed by the first projections), then the
        # input casts, then everything else so the casts win the DMA engines.
        cast_insts = {"q": [], "k": [], "v": []}

        def issue_casts(lo, hi):
            for nm, srcd, dst in (("q", t["q_in"], qb), ("k", t["k_in"], kb),
                                  ("v", t["v_in"], vb)):
                ci = nc.gpsimd.dma_start(
                    out=dst[lo * CH:hi * CH, :],
                    in_=srcd[lo * CH:hi * CH, :])
                cast_insts[nm].append(((lo, hi), ci))

        issue_casts(0, 1)
        W = {}
        for nm in ("wq", "wk", "wv"):
            W[nm] = singles.tile([128, 4, D], BF16, tag=nm, name=f"w_{nm}")
            nc.scalar.dma_start(out=W[nm][:],
                                in_=t[nm].ap().rearrange("(c p) d -> p c d", p=128))
        issue_casts(1, 2)

        bias_cols = {}
        for nm in ("bq_c", "bk_c"):
            bias_cols[nm] = singles.tile([128, 4], F32, tag=nm, name=f"bc_{nm}")
            nc.scalar.dma_start(out=bias_cols[nm][:], in_=t[nm][:, :])
        mask_sb = singles.tile([128, 128], BF16)
        nc.scalar.dma_start(out=mask_sb[:], in_=t["bmask"][:, :])
        ones_row = singles.tile([1, 128], BF16)
        nc.vector.memset(ones_row[:], 1.0)
        ones_col = singles.tile([128, 1], BF16)
        nc.vector.memset(ones_col[:], 1.0)
        ones_full = singles.tile([128, 128], BF16)
        nc.vector.memset(ones_full[:], 1.0)
        eps_sb = singles.tile([128, 1], F32)
        nc.vector.memset(eps_sb[:], EPS)

        issue_casts(2, 4)
        issue_casts(4, 6)
        issue_casts(6, 8)

        axd_writers = {}
        p2a = octx.enter_context(tc.tile_pool(name="p2a", bufs=1))
        axt = p2a.tile([128, 4, M], BF16, tag="axt")

        # ================= phase 2 weights ==============================
        # deferred behind the early input casts so they don't hog the DMA
        # engines during the pipeline ramp
        # Allocated here; DMAs issued mid-phase-1 (see chunk loop, c==3)
        # on the sync queue so they neither hog the DMA engines at startup
        # nor get scheduled into the phase boundary.
        for nm in ("wpq", "wpk", "wo"):
            W[nm] = singles.tile([128, 4, D], BF16, tag=nm, name=f"w_{nm}")
        for nm in ("bpq_c", "bpk_c", "ln_g_c", "ln_b_c", "bv_c"):
            bias_cols[nm] = singles.tile([128, 4], F32, tag=nm, name=f"bc_{nm}")
        bo_sb = singles.tile([128, D], BF16)
        ident_sb = singles.tile([128, 128], BF16)

        def load_p2_consts(gate):
            dis = []
            for nm in ("wpq", "wpk", "wo"):
                dis.append(nc.sync.dma_start(
                    out=W[nm][:],
                    in_=t[nm].ap().rearrange("(c p) d -> p c d", p=128)))
            for nm in ("bpq_c", "bpk_c", "ln_g_c", "ln_b_c", "bv_c"):
                dis.append(nc.sync.dma_start(out=bias_cols[nm][:], in_=t[nm][:, :]))
            dis.append(nc.sync.dma_start(
                out=bo_sb[:],
                in_=t["bo_r"].ap().to_broadcast((128, D))))
            dis.append(nc.sync.dma_start(out=ident_sb[:], in_=t["ident"][:, :]))
            for di in dis:
                add_dep_helper(di.ins, gate.ins,
                               reason="const loads after startup transposes")

        # ================= phase 1 =================
        with ExitStack() as ctx:
            p1 = ctx.enter_context(tc.tile_pool(name="p1", bufs=1))
            kT = p1.tile([128, 4, 3, CH], BF16, tag="kT")        # ring of 3 chunks
            qpT = p1.tile([128, 4, 3, CH], BF16, tag="qpT")      # ring of 3 chunks
            vtm = p1.tile([128, 12, 8, 65], BF16, tag="vtm")     # ring of 12 tiles, 65-col/head
            nc.vector.memset(vtm[:, :, :, 64:65], 1.0)           # ones col for denominators
            qraw = p1.tile([128, 4, M + 4], BF16, tag="qraw")    # full, padded +-2
            nc.vector.memset(qraw[:, :, 0:2], 0.0)
            nc.vector.memset(qraw[:, :, M + 2:M + 4], 0.0)

            xtp = ctx.enter_context(tc.tile_pool(name="xtp", bufs=2))
            ps_proj = ctx.enter_context(tc.tile_pool(name="ps_proj", bufs=2, space="PSUM"))
            ps_st = ctx.enter_context(tc.tile_pool(name="ps_st", bufs=2, space="PSUM"))
            ps_out = ctx.enter_context(tc.tile_pool(name="ps_out", bufs=1, space="PSUM"))
            dsa_sb = ctx.enter_context(tc.tile_pool(name="dsa_sb", bufs=2))
            pool_tmp = ctx.enter_context(tc.tile_pool(name="pool_tmp", bufs=2))
            ax_pool = ctx.enter_context(tc.tile_pool(name="ax_sb", bufs=2))

            xt_gate = {}

            def load_xt(nm, dram, c):
                xt = xtp.tile([128, 4, CH], BF16, tag=f"xt_{nm}", name=f"xt_{nm}_{c}")
                ti = nc.sync.dma_start(out=xt[:],
                                       in_=dram[c * CH:(c + 1) * CH, :],
                                       transpose=True)
                for (lo, hi), ci in cast_insts[nm]:
                    if lo <= c < hi:
                        add_dep_helper(ti.ins, ci.ins,
                                       reason="transpose reads cast output")
                xt_gate[(nm, c)] = ti
                return xt

            def proj_fm_group(xt, wname, bname, dst_fn, j):
                ps = ps_proj.tile([128, CH], F32, tag="proj",
                                  name=f"ps_{wname}_{j}")
                for dk in range(4):
                    nc.tensor.matmul(ps[:], W[wname][:, dk, j * 128:(j + 1) * 128],
                                     xt[:, dk, :], start=(dk == 0), stop=(dk == 3))
                nc.scalar.activation(dst_fn(j), ps[:], AF.Identity,
                                     bias=bias_cols[bname][:, j:j + 1], scale=1.0)

            def proj_v_group(xt, c, tt):
                ps = ps_proj.tile([128, D], F32, tag="proj", name=f"ps_v_{tt}")
                for dk in range(4):
                    nc.tensor.matmul(ps[:], xt[:, dk, tt * 128:(tt + 1) * 128],
                                     W["wv"][:, dk, :], start=(dk == 0),
                                     stop=(dk == 3), skip_group_check=True)
                nc.scalar.copy(vtm[:, (c * 4 + tt) % 12, :, 0:64],
                               ps[:].rearrange("p (h d) -> p h d", h=H))

            def pool_chunk(c):
                base = c * CH
                ta = pool_tmp.tile([128, 4, CH + 2], BF16, tag="ta")
                nc.vector.tensor_add(ta[:], qraw[:, :, base:base + CH + 2],
                                     qraw[:, :, base + 1:base + CH + 3])
                tb = pool_tmp.tile([128, 4, CH], BF16, tag="tb")
                nc.vector.tensor_add(tb[:], ta[:, :, 0:CH], ta[:, :, 2:CH + 2])
                nc.vector.tensor_add(qpT[:, :, c % 3, :], tb[:],
                                     qraw[:, :, base + 4:base + CH + 4])

            def dsa_scores(c, lt):
                """MM1 + exp + mask for tile lt of chunk c -> masked sbuf tile."""
                st = ps_st.tile([128, 8, 128], F32, tag="st", name=f"st_{c}_{lt}")
                for h in range(H):
                    hp = PERM[h]
                    base = (h % 2) * 64
                    lhsT = kT[base:base + 64, h // 2, c % 3, lt * 128:(lt + 1) * 128]
                    rhs = qpT[base:base + 64, h // 2, c % 3, lt * 128:(lt + 1) * 128]
                    nc.tensor.matmul(st[:, hp, :], lhsT, rhs, start=True, stop=True,
                                     skip_group_check=True)
                expS = dsa_sb.tile([128, 8, 128], BF16, tag="expS",
                                   name=f"expS_{c}_{lt}")
                nc.scalar.activation(expS[:], st[:], AF.Exp, scale=SCALE / QNB)
                masked = dsa_sb.tile([128, 8, 128], BF16, tag="masked",
                                     name=f"masked_{c}_{lt}")
                nc.vector.tensor_mul(masked[:], expS[:],
                                     mask_sb[:].unsqueeze(1).to_broadcast((128, 8, 128)))
                return masked

            def dsa_out(c, lt, masked, ax_out):
                """attn@V with ones-col denominators, then normalize."""
                outp = ps_out.tile([128, 2, 512], F32, tag="outp",
                                   name=f"outp_{c}_{lt}")
                for h in range(H):
                    hp = PERM[h]
                    nc.tensor.matmul(outp[:, h // 4, (h % 4) * 65:(h % 4) * 65 + 65],
                                     masked[:, hp, :],
                                     vtm[:, (c * 4 + lt) % 12, h, :],
                                     start=True, stop=True, skip_group_check=True)
                recip = dsa_sb.tile([128, 2, 4], F32, tag="recip",
                                    name=f"recip_{c}_{lt}")
                den_view = bass.AP(outp.tensor, outp[:].offset + 64,
                                   [outp[:].ap[0], [512, 2], [65, 4]])
                nc.vector.reciprocal(recip[:], den_view)
                av_view = bass.AP(outp.tensor, outp[:].offset,
                                  [outp[:].ap[0], [512, 2], [65, 4], [1, 64]])
                nc.vector.tensor_mul(
                    ax_out.rearrange("p (a b d) -> p a b d", a=2, b=4),
                    av_view,
                    recip[:].unsqueeze(3).to_broadcast((128, 2, 4, 64)))

            def dsa_group_list(c, ax):
                masked = {}
                g = []
                g.append(lambda: masked.__setitem__(0, dsa_scores(c, 0)))
                g.append(lambda: masked.__setitem__(1, dsa_scores(c, 1)))
                g.append(lambda: dsa_out(c, 0, masked.pop(0), ax[:, 0, :]))
                g.append(lambda: masked.__setitem__(2, dsa_scores(c, 2)))
                g.append(lambda: dsa_out(c, 1, masked.pop(1), ax[:, 1, :]))
                g.append(lambda: masked.__setitem__(3, dsa_scores(c, 3)))
                g.append(lambda: dsa_out(c, 2, masked.pop(2), ax[:, 2, :]))
                g.append(lambda: dsa_out(c, 3, masked.pop(3), ax[:, 3, :]))
                return g

            def store_ax(c, ax):
                dst = axd.ap().rearrange("(cc lt p) d -> cc p lt d", lt=4, p=128)[c]
                wi = nc.gpsimd.dma_start(out=dst, in_=ax[:])
                axd_writers[c] = wi

            def issue_axt(c):
                ti = nc.sync.dma_start(
                    out=axt[:, :, c * CH:(c + 1) * CH],
                    in_=axd[c * CH:(c + 1) * CH, :],
                    transpose=True)
                add_dep_helper(ti.ins, axd_writers[c].ins,
                               reason="axt transpose reads axd chunk")

            for c in range(NCHUNK + 2):
                pgroups = []
                if c < NCHUNK:
                    qxt = load_xt("q", qb, c)
                    kxt = load_xt("k", kb, c)
                    vxt = load_xt("v", vb, c)
                    if c == 3:
                        load_p2_consts(xt_gate[("q", 2)])
                    for j in range(4):
                        pgroups.append(lambda j=j, x=qxt, c=c: proj_fm_group(
                            x, "wq", "bq_c",
                            lambda jj, c=c: qraw[:, jj, 2 + c * CH:2 + (c + 1) * CH], j))
                    for j in range(4):
                        pgroups.append(lambda j=j, x=kxt, c=c: proj_fm_group(
                            x, "wk", "bk_c", lambda jj, c=c: kT[:, jj, c % 3, :], j))
                    for tt in range(4):
                        pgroups.append(lambda tt=tt, x=vxt, c=c: proj_v_group(x, c, tt))
                dgroups = []
                ax = None
                if c >= 2:
                    ax = ax_pool.tile([128, 4, D], BF16, tag="ax", name=f"ax_{c - 2}")
                    dgroups = dsa_group_list(c - 2, ax)
                # weave: spread D groups evenly through the P stream;
                # pool(c-1) after the 4 Q-projection groups
                npg, ndg = len(pgroups), len(dgroups)
                dpos = {int(round((k + 1) * npg / (ndg + 1))): k for k in range(ndg)} \
                    if npg else {}
                for i in range(max(npg, 1)):
                    if i < npg:
                        pgroups[i]()
                    if i == 3 and 1 <= c <= NCHUNK:
                        pool_chunk(c - 1)
                    if i in dpos:
                        dgroups[dpos[i]]()
                if not pgroups:
                    if 1 <= c <= NCHUNK:
                        pool_chunk(c - 1)
                    for g in dgroups:
                        g()
                if ax is not None:
                    store_ax(c - 2, ax)


        # ================= phase 2 =================
        with ExitStack() as ctx:
            p2 = ctx.enter_context(tc.tile_pool(name="p2", bufs=1))

            pv = p2.tile([128, 4, WIN, D], BF16, tag="pv")
            wtn = p2.tile([128, 4, WN], BF16, tag="wtn")
            pqT = p2.tile([128, 4, WN], BF16, tag="pqT")
            pkT = p2.tile([128, 4, WN], BF16, tag="pkT")
            esA = p2.tile([128, H, 4, WN], BF16, tag="esA")
            zt = p2.tile([128, 4, QLEN], BF16, tag="zt")
            recip_sb = p2.tile([128, H, 4], F32, tag="recips")

            # issue axt transposes + pv gathers in dependency-arrival order
            srcv = axd.ap().rearrange("(cc p w) d -> cc p w d", p=128, w=PW)
            for c in range(NCHUNK):
                issue_axt(c)
                if c % 2 == 1:
                    cc = c // 2
                    gi = nc.sync.dma_start(out=pv[:, cc, :, :], in_=srcv[cc, :, 1:PW, :])
                    add_dep_helper(gi.ins, axd_writers[2 * cc].ins, reason="pv gather")
                    add_dep_helper(gi.ins, axd_writers[2 * cc + 1].ins, reason="pv gather")

            # ---- win_tok (+bv) LN + GELU + pq/pk, per 128-window group ----
            with ExitStack() as lctx:
                ps_ln = lctx.enter_context(
                    tc.tile_pool(name="ps_ln", bufs=3, space="PSUM"))
                lnp = lctx.enter_context(tc.tile_pool(name="lnp", bufs=2))

                lnA = {}

                def ln_phase_a(g):
                    """Moments + rstd; ACT funcs all within one table set
                    (Identity/Square/Copy/Sqrt)."""
                    wt_g = axt[:, :, g * GW * PW:(g + 1) * GW * PW:PW]
                    wtb = lnp.tile([128, 4, GW], BF16, tag="wtb", bufs=4,
                                   name=f"wtb_{g}")
                    for j in range(4):
                        nc.scalar.activation(wtb[:, j, :], wt_g[:, j, :],
                                             AF.Identity,
                                             bias=bias_cols["bv_c"][:, j:j + 1],
                                             scale=1.0)
                    wsq = lnp.tile([128, 4, GW], BF16, tag="wsq", name=f"wsq_{g}")
                    nc.scalar.activation(wsq[:], wtb[:], AF.Square)
                    ps_mu = ps_ln.tile([128, GW], F32, tag="psln", name=f"psmu_{g}")
                    ps_var = ps_ln.tile([128, GW], F32, tag="psln", name=f"psvar_{g}")
                    for j in range(4):
                        nc.tensor.matmul(ps_mu[:], ones_full[:], wtb[:, j, :],
                                         start=(j == 0), stop=(j == 3),
                                         skip_group_check=True)
                        nc.tensor.matmul(ps_var[:], ones_full[:], wsq[:, j, :],
                                         start=(j == 0), stop=(j == 3),
                                         skip_group_check=True)
                    mu = lnp.tile([128, GW], F32, tag="mu", bufs=4, name=f"mu_{g}")
                    nc.scalar.mul(mu[:], ps_mu[:], 1.0 / D)
                    ex2 = lnp.tile([128, GW], F32, tag="ex2", bufs=1,
                                   name=f"ex2_{g}")
                    nc.scalar.mul(ex2[:], ps_var[:], 1.0 / D)
                    var = lnp.tile([128, GW], F32, tag="var", bufs=1,
                                   name=f"var_{g}")
                    nc.vector.tensor_mul(var[:], mu[:], mu[:])
                    nc.vector.tensor_sub(var[:], ex2[:], var[:])
                    sd = lnp.tile([128, GW], F32, tag="sd", bufs=1, name=f"sd_{g}")
                    nc.scalar.activation(sd[:], var[:], AF.Sqrt, bias=eps_sb[:])
                    rstd = lnp.tile([128, GW], F32, tag="rstd", bufs=4,
                                    name=f"rstd_{g}")
                    nc.vector.reciprocal(rstd[:], sd[:])
                    lnA[g] = (wtb, mu, rstd)

                def ln_phase_b(g):
                    """GELU + pq/pk projections (Gelu/Identity table set)."""
                    gs = g * GW
                    wtb, mu, rstd = lnA.pop(g)
                    for j in range(4):
                        tmp = lnp.tile([128, GW], F32, tag="lnt", name=f"lnt_{g}_{j}")
                        nc.vector.tensor_sub(tmp[:], wtb[:, j, :], mu[:])
                        nc.vector.tensor_mul(tmp[:], tmp[:], rstd[:])
                        nc.scalar.activation(wtn[:, j, gs:gs + GW], tmp[:],
                                             AF.Gelu,
                                             bias=bias_cols["ln_b_c"][:, j:j + 1],
                                             scale=bias_cols["ln_g_c"][:, j:j + 1])
                    for dst, wname, bname in ((pqT, "wpq", "bpq_c"),
                                              (pkT, "wpk", "bpk_c")):
                        for j in range(4):
                            ps = ps_ln.tile([128, GW], F32, tag="psln",
                                            name=f"pp_{wname}_{g}_{j}")
                            for dk in range(4):
                                nc.tensor.matmul(
                                    ps[:], W[wname][:, dk, j * 128:(j + 1) * 128],
                                    wtn[:, dk, gs:gs + GW],
                                    start=(dk == 0), stop=(dk == 3))
                            nc.vector.tensor_scalar_add(
                                dst[:, j, gs:gs + GW], ps[:],
                                bias_cols[bname][:, j:j + 1])

                # A0..A2 then B0..B2 (one Sqrt->Gelu table switch), then the
                # last group's A3+B3 pair on the critical path (one more
                # switch pair).
                for g in range(NG - 1):
                    ln_phase_a(g)
                for g in range(NG - 1):
                    ln_phase_b(g)
                ln_phase_a(NG - 1)
                ln_phase_b(NG - 1)

            # ---- PSA: raw exp scores; den via N=1 matmuls; window-major pout
            with ExitStack() as pctx:
                # PSUM budget (8 banks): es/fin share slots (disjoint
                # lifetimes, same shape) 2 + po 2 + ztps 2 + den 1 = 7.
                ps_es = pctx.enter_context(
                    tc.tile_pool(name="ps_es", bufs=2, space="PSUM"))
                ps_po = pctx.enter_context(
                    tc.tile_pool(name="ps_po", bufs=2, space="PSUM"))
                ps_ztden = pctx.enter_context(
                    tc.tile_pool(name="ps_ztden", bufs=2, space="PSUM"))
                ps_fin = ps_es
                zwp = pctx.enter_context(tc.tile_pool(name="zwp", bufs=3))
                ztp = pctx.enter_context(tc.tile_pool(name="ztp", bufs=2))
                osb = pctx.enter_context(tc.tile_pool(name="osb", bufs=4))

                def psa_scores(h):
                    base = (h % 2) * 64
                    for cp in range(2):
                        ps = ps_es.tile([128, 2, WN], F32, tag="es",
                                        name=f"es_{h}_{cp}")
                        for k in range(2):
                            cc = cp * 2 + k
                            nc.tensor.matmul(
                                ps[:, k, :], pkT[base:base + 64, h // 2,
                                                 cc * 128:(cc + 1) * 128],
                                pqT[base:base + 64, h // 2, :],
                                start=True, stop=True, skip_group_check=True)
                        nc.scalar.activation(esA[:, h, 2 * cp:2 * cp + 2, :],
                                             ps[:], AF.Exp, scale=SCALE)

                def pout_one(h, qt, zwin):
                    # [pout | den] share one PSUM bank: cols 0:448 accumulate
                    # raw-exp attn @ pv, col 448 accumulates the softmax
                    # denominator against a ones column.
                    po = ps_po.tile([128, WIN * HD + 1], F32, tag="po",
                                    name=f"po_{h}_{qt}")
                    pov = po[:, 0:WIN * HD].rearrange("p (i d) -> p i d", i=WIN)
                    for cc in range(4):
                        nc.tensor.matmul(
                            pov, esA[:, h, cc, qt * 128:(qt + 1) * 128],
                            pv[:, cc, :, h * 64:(h + 1) * 64],
                            start=(cc == 0), stop=(cc == 3),
                            skip_group_check=True)
                    for cc in range(4):
                        nc.tensor.matmul(
                            po[:, WIN * HD:WIN * HD + 1],
                            esA[:, h, cc, qt * 128:(qt + 1) * 128],
                            ones_col[:], start=(cc == 0), stop=(cc == 3),
                            skip_group_check=True)
                    rc = recip_sb[:, h, qt:qt + 1]
                    nc.vector.reciprocal(rc, po[:, WIN * HD:WIN * HD + 1])
                    ztmp = ztp.tile([128, WIN, HD], BF16, tag="ztmp",
                                    name=f"ztmp_{h}_{qt}")
                    nc.vector.tensor_scalar_mul(ztmp[:], pov, rc)
                    nc.vector.tensor_add(zwin[:, :, h * 64:(h + 1) * 64], ztmp[:],
                                         pv[:, qt, :, h * 64:(h + 1) * 64])

                def ztrans_one(qt, ii, zwin):
                    """Transpose payload slots ii..ii+1 (or just ii at the
                    tail) of group qt into feature-major zt."""
                    ni = min(2, WIN - ii)
                    zt_ps = ps_ztden.tile([128, 4, 2, 128], BF16, tag="ztps",
                                          name=f"ztps_{qt}_{ii}")
                    for di in range(ni):
                        for fg in range(4):
                            nc.tensor.transpose(
                                zt_ps[:, fg, di, :],
                                zwin[:, ii + di, fg * 128:(fg + 1) * 128],
                                ident_sb[:])
                    base = qt * GW * WIN
                    dst = zt[:, :, base + ii:base + GW * WIN:WIN]
                    dst = bass.AP(dst.tensor, dst.offset,
                                  [dst.ap[0], dst.ap[1], [1, ni], [WIN, 128]])
                    src_ap = zt_ps[:, :, 0:ni, :]
                    nc.scalar.copy(dst, src_ap)

                osb_tiles = {}

                def fin_one(tt):
                    psf = ps_fin.tile([128, 2, WN], F32, tag="es",
                                      name=f"fin_{tt}")
                    ps = psf[:, 0, :]
                    for dk in range(4):
                        nc.tensor.matmul(ps, zt[:, dk, tt * 128:(tt + 1) * 128],
                                         W["wo"][:, dk, :], start=(dk == 0),
                                         stop=(dk == 3), skip_group_check=True)
                    g = tt // 2
                    if tt % 2 == 0:
                        osb_tiles[g] = osb.tile([128, 2, D], F32, tag="osb",
                                                name=f"osb_{g}")
                    nc.vector.tensor_add(osb_tiles[g][:, tt % 2, :], ps,
                                         bo_sb[:])
                    if tt % 2 == 1:
                        outv = out.ap().rearrange("(g tt p) d -> g p tt d",
                                                  tt=2, p=128)
                        nc.sync.dma_start(out=outv[g], in_=osb_tiles.pop(g)[:])

                # head-outer pipeline: as soon as head h's exp-scores are
                # done, its denominators and all four pout groups flow; the
                # transposes + final projections drain afterwards per group.
                for h in range(H):
                    psa_scores(h)

                zw = {}
                prev = None

                def tail_items(qt):
                    items = []
                    zwin_p = zw[qt]
                    for ii in range(0, WIN, 2):
                        items.append(lambda ii=ii: ztrans_one(qt, ii, zwin_p))
                    for j in range(WIN):
                        items.append(lambda j=j: fin_one(qt * WIN + j))
                    return items

                for qt in range(NG):
                    zw[qt] = zwp.tile([128, WIN, D], BF16, tag="zwin",
                                      name=f"zwin_{qt}")
                    titems = tail_items(prev) if prev is not None else []
                    ti = 0
                    for h in range(H):
                        pout_one(h, qt, zw[qt])
                        for _ in range(2):
                            if ti < len(titems) and h >= 3 and \
                                    ((h - 3) * 16) // H >= ti:
                                titems[ti]()
                                ti += 1
                    while ti < len(titems):
                        titems[ti]()
                        ti += 1
                    if prev is not None:
                        zw.pop(prev)
                    prev = qt
                for it in tail_items(prev):
                    it()


_NC_CACHE = None


def _get_program():
    global _NC_CACHE
    if _NC_CACHE is None:
        _NC_CACHE = build_program()
    return _NC_CACHE


def _host_consts(Wk, bk, Wv, bv, Wq, bq, ln_g, ln_b, Wpq, bpq, Wpk, bpk, Wo, bo):
    bf = ml_dtypes.bfloat16
    col = lambda b: np.asarray(b, np.float32).reshape(4, 128).T.copy()
    bo2 = np.asarray(bo, np.float32) + 2.0 * (
        np.asarray(bv, np.float32) @ np.asarray(Wo, np.float32))
    consts = {
        "wq": np.asarray(Wq, np.float32).astype(bf),
        "wk": np.asarray(Wk, np.float32).astype(bf),
        "wv": np.asarray(Wv, np.float32).astype(bf),
        "wpq": np.asarray(Wpq, np.float32).astype(bf),
        "wpk": np.asarray(Wpk, np.float32).astype(bf),
        "wo": np.asarray(Wo, np.float32).astype(bf),
        "bq_c": col(bq), "bk_c": col(bk),
        "bpq_c": col(bpq), "bpk_c": col(bpk),
        "ln_g_c": col(ln_g), "ln_b_c": col(ln_b),
        "bv_c": col(bv),
        "bo_r": bo2.reshape(1, D).astype(bf),
        "ident": np.eye(128, dtype=np.float32).astype(bf),
    }
    m = np.zeros((128, 128), np.float32)
    for g in range(16):
        m[g * PW:(g + 1) * PW, g * PW:(g + 1) * PW] = 1.0
    consts["bmask"] = m.astype(bf)
    return consts


def kernel(k, v, q, query_len, Wk, bk, Wv, bv, Wq, bq, ln_g, ln_b,
           Wpq, bpq, Wpk, bpk, Wo, bo):
    nc = _get_program()
    consts = _host_consts(Wk, bk, Wv, bv, Wq, bq, ln_g, ln_b,
                          Wpq, bpq, Wpk, bpk, Wo, bo)
    k = np.asarray(k, np.float32)
    v = np.asarray(v, np.float32)
    q = np.asarray(q, np.float32)
    in_maps = []
    for b in range(B):
        m = {"q": np.ascontiguousarray(q[b]), "k": np.ascontiguousarray(k[b]),
             "v": np.ascontiguousarray(v[b])}
        m.update(consts)
        in_maps.append(m)
    res = run_bass_kernel_spmd(nc, in_maps, core_ids=list(range(B)))
    return np.stack([res.results[b]["out"] for b in range(B)], axis=0)


if __name__ == "__main__":
    nc = build_program()
    print("program built ok")


# revision 68
# speedup vs baseline: 1.5326x; 1.0021x over previous
"""Trainium2 Bass kernel for DeformableMultiHeadedAttention.

Data-parallel over batch B=8 across 8 NeuronCores (one batch element per
core, identical programs, no collectives).

Per-core pipeline (matmuls bf16 with f32 accumulate):
  1. q,k,v [4096,512] f32 -> SWDGE cast-DMA -> DRAM bf16 -> batched HWDGE
     DMA-transpose (one [512,512] xbar transpose per chunk) -> feature-major
     XT [128,4,tok] chunks in SBUF.
  2. Projections on PE: K'T/Q'T feature-major (lhsT=W, rhs=XT); V' token-major
     (lhsT=XT tile, rhs=W), bv folded out on host (bo' = bo + 2*bv@Wo, LN
     input gets +bv on chip).
  3. Q pooling (AvgPool k=5, stride 1, zero pad) as 3 shifted adds; the 1/5
     is folded into the softmax exp scale.
  4. DSA (windows of 8 tokens): per 128-token tile, 8 heads: S_T[k,q] on PE,
     exp on ACT, block-diag mask mul on DVE, attn@V plus ones-col denominator
     sharing the lhsT, per-partition 1/den scale on DVE. Token-major DSA
     output -> DRAM (bf16).
  5. DRAM round-trips: batched DMA-transpose -> attn_xT feature-major;
     strided gather -> PV window-major [kw, (slot, head, hd)].
  6. Incremental (per 128-window group, overlapping phase 1 tail): win_tok
     +bv, LayerNorm moments via ones-matmuls, exact GELU, pq/pk projections.
  7. PSA restructured: raw exp-scores kept unnormalized; denominators via
     N=1 matmuls against a ones column (per-partition 1/den on DVE); pout
     window-major [wq, (slot,hd)] at M=128 (half the PE rows of the
     feature-major form); z = pout*recip + attn_x in window-major form;
     PE identity-transposes + strided ACT copies build feature-major zT.
  8. out = Z @ Wo + bo' with Z as the stationary operand -> token-major f32
     output, streamed per window-group.
"""

import sys
from contextlib import ExitStack

for _p in ("/opt/trn_rl_repo/concourse", "/opt/trn_rl_repo"):
    if _p not in sys.path:
        sys.path.insert(0, _p)

import numpy as np
import ml_dtypes

import concourse.bass as bass
import concourse.mybir as mybir
import concourse.tile as tile
from concourse import bacc
from concourse.tile import add_dep_helper
from concourse.bass_utils import run_bass_kernel_spmd

BF16 = mybir.dt.bfloat16
F32 = mybir.dt.float32
AF = mybir.ActivationFunctionType
ALU = mybir.AluOpType

B, M, D = 8, 4096, 512
H, HD = 8, 64
WIN = 7
PW = WIN + 1
QNB = 5
QLEN = 3584
WN = M // PW
SCALE = D ** -0.5
EPS = 1e-5
NCHUNK = 8
CH = 512
NG = 4                   # window groups of 128 for phase 2
GW = WN // NG            # 128 windows per group
PERM = [(h % 2) * 4 + h // 2 for h in range(H)]  # head -> DSA psum slot


def build_program():
    nc = bacc.Bacc("TRN2", target_bir_lowering=False, debug=False, num_devices=8)

    t = {}
    t["q_in"] = nc.dram_tensor("q", [M, D], F32, kind="ExternalInput")
    t["k_in"] = nc.dram_tensor("k", [M, D], F32, kind="ExternalInput")
    t["v_in"] = nc.dram_tensor("v", [M, D], F32, kind="ExternalInput")
    for nm in ("wq", "wk", "wv", "wpq", "wpk", "wo"):
        t[nm] = nc.dram_tensor(nm, [D, D], BF16, kind="ExternalInput")
    for nm in ("bq_c", "bk_c", "bpq_c", "bpk_c", "ln_g_c", "ln_b_c", "bv_c"):
        t[nm] = nc.dram_tensor(nm, [128, 4], F32, kind="ExternalInput")
    t["bo_r"] = nc.dram_tensor("bo_r", [1, D], BF16, kind="ExternalInput")
    t["bmask"] = nc.dram_tensor("bmask", [128, 128], BF16, kind="ExternalInput")
    t["ident"] = nc.dram_tensor("ident", [128, 128], BF16, kind="ExternalInput")
    t["out"] = nc.dram_tensor("out", [QLEN, D], F32, kind="ExternalOutput")
    t["axd"] = nc.dram_tensor("axd_s", [M, D], BF16, kind="Internal")
    t["zd"] = nc.dram_tensor("zd_s", [QLEN, D], BF16, kind="Internal")
    t["qb"] = nc.dram_tensor("qb_s", [M, D], BF16, kind="Internal")
    t["kb"] = nc.dram_tensor("kb_s", [M, D], BF16, kind="Internal")
    t["vb"] = nc.dram_tensor("vb_s", [M, D], BF16, kind="Internal")

    with tile.TileContext(nc) as tc:
        _build(nc, tc, t)
    nc.compile()
    return nc


def _build(nc, tc, t):
    qb, kb, vb = t["qb"], t["kb"], t["vb"]
    axd, out = t["axd"], t["out"]
    zd = t["zd"]

    with ExitStack() as octx:
        singles = octx.enter_context(tc.tile_pool(name="singles", bufs=1))

        # phase-1 weights first (needed by the first projections), then the
        # input casts, then everything else so the casts win the DMA engines.
        cast_insts = {"q": [], "k": [], "v": []}

        def issue_casts(lo, hi):
            for nm, srcd, dst in (("q", t["q_in"], qb), ("k", t["k_in"], kb),
                                  ("v", t["v_in"], vb)):
                ci = nc.gpsimd.dma_start(
                    out=dst[lo * CH:hi * CH, :],
                    in_=srcd[lo * CH:hi * CH, :])
                cast_insts[nm].append(((lo, hi), ci))

        issue_casts(0, 1)
        W = {}

        def load_w(nm):
            W[nm] = singles.tile([128, 4, D], BF16, tag=nm, name=f"w_{nm}")
            nc.scalar.dma_start(out=W[nm][:],
                                in_=t[nm].ap().rearrange("(c p) d -> p c d", p=128))

        load_w("wq")
        issue_casts(1, 2)
        load_w("wk")
        load_w("wv")

        bias_cols = {}
        for nm in ("bq_c", "bk_c"):
            bias_cols[nm] = singles.tile([128, 4], F32, tag=nm, name=f"bc_{nm}")
            nc.scalar.dma_start(out=bias_cols[nm][:], in_=t[nm][:, :])
        mask_sb = singles.tile([128, 128], BF16)
        nc.scalar.dma_start(out=mask_sb[:], in_=t["bmask"][:, :])
        ones_row = singles.tile([1, 128], BF16)
        nc.vector.memset(ones_row[:], 1.0)
        ones_col = singles.tile([128, 1], BF16)
        nc.vector.memset(ones_col[:], 1.0)
        ones_full = singles.tile([128, 128], BF16)
        nc.vector.memset(ones_full[:], 1.0)
        eps_sb = singles.tile([128, 1], F32)
        nc.vector.memset(eps_sb[:], EPS)

        issue_casts(2, 4)
        issue_casts(4, 6)
        issue_casts(6, 8)

        axd_writers = {}
        p2a = octx.enter_context(tc.tile_pool(name="p2a", bufs=1))
        axt = p2a.tile([128, 4, M], BF16, tag="axt")

        # ================= phase 2 weights ==============================
        # deferred behind the early input casts so they don't hog the DMA
        # engines during the pipeline ramp
        # Allocated here; DMAs issued mid-phase-1 (see chunk loop, c==3)
        # on the sync queue so they neither hog the DMA engines at startup
        # nor get scheduled into the phase boundary.
        for nm in ("wpq", "wpk", "wo"):
            W[nm] = singles.tile([128, 4, D], BF16, tag=nm, name=f"w_{nm}")
        for nm in ("bpq_c", "bpk_c", "ln_g_c", "ln_b_c", "bv_c"):
            bias_cols[nm] = singles.tile([128, 4], F32, tag=nm, name=f"bc_{nm}")
        bo_sb = singles.tile([128, D], BF16)
        ident_sb = singles.tile([128, 128], BF16)

        def load_p2_consts(gate):
            dis = []
            for nm in ("wpq", "wpk", "wo"):
                dis.append(nc.sync.dma_start(
                    out=W[nm][:],
                    in_=t[nm].ap().rearrange("(c p) d -> p c d", p=128)))
            for nm in ("bpq_c", "bpk_c", "ln_g_c", "ln_b_c", "bv_c"):
                dis.append(nc.sync.dma_start(out=bias_cols[nm][:], in_=t[nm][:, :]))
            dis.append(nc.sync.dma_start(
                out=bo_sb[:],
                in_=t["bo_r"].ap().to_broadcast((128, D))))
            dis.append(nc.sync.dma_start(out=ident_sb[:], in_=t["ident"][:, :]))
            for di in dis:
                add_dep_helper(di.ins, gate.ins,
                               reason="const loads after startup transposes")

        # ================= phase 1 =================
        with ExitStack() as ctx:
            p1 = ctx.enter_context(tc.tile_pool(name="p1", bufs=1))
            kT = p1.tile([128, 4, 3, CH], BF16, tag="kT")        # ring of 3 chunks
            qpT = p1.tile([128, 4, 3, CH], BF16, tag="qpT")      # ring of 3 chunks
            vtm = p1.tile([128, 12, 8, 65], BF16, tag="vtm")     # ring of 12 tiles, 65-col/head
            nc.vector.memset(vtm[:, :, :, 64:65], 1.0)           # ones col for denominators
            qraw = p1.tile([128, 4, M + 4], BF16, tag="qraw")    # full, padded +-2
            nc.vector.memset(qraw[:, :, 0:2], 0.0)
            nc.vector.memset(qraw[:, :, M + 2:M + 4], 0.0)

            xtp = ctx.enter_context(tc.tile_pool(name="xtp", bufs=2))
            ps_proj = ctx.enter_context(tc.tile_pool(name="ps_proj", bufs=2, space="PSUM"))
            ps_st = ctx.enter_context(tc.tile_pool(name="ps_st", bufs=2, space="PSUM"))
            ps_out = ctx.enter_context(tc.tile_pool(name="ps_out", bufs=1, space="PSUM"))
            dsa_sb = ctx.enter_context(tc.tile_pool(name="dsa_sb", bufs=2))
            pool_tmp = ctx.enter_context(tc.tile_pool(name="pool_tmp", bufs=2))
            ax_pool = ctx.enter_context(tc.tile_pool(name="ax_sb", bufs=2))

            xt_gate = {}

            def load_xt(nm, dram, c):
                xt = xtp.tile([128, 4, CH], BF16, tag=f"xt_{nm}", name=f"xt_{nm}_{c}")
                ti = nc.sync.dma_start(out=xt[:],
                                       in_=dram[c * CH:(c + 1) * CH, :],
                                       transpose=True)
                for (lo, hi), ci in cast_insts[nm]:
                    if lo <= c < hi:
                        add_dep_helper(ti.ins, ci.ins,
                                       reason="transpose reads cast output")
                xt_gate[(nm, c)] = ti
                return xt

            def proj_fm_group(xt, wname, bname, dst_fn, j):
                ps = ps_proj.tile([128, CH], F32, tag="proj",
                                  name=f"ps_{wname}_{j}")
                for dk in range(4):
                    nc.tensor.matmul(ps[:], W[wname][:, dk, j * 128:(j + 1) * 128],
                                     xt[:, dk, :], start=(dk == 0), stop=(dk == 3))
                nc.scalar.activation(dst_fn(j), ps[:], AF.Identity,
                                     bias=bias_cols[bname][:, j:j + 1], scale=1.0)

            def proj_v_group(xt, c, tt):
                ps = ps_proj.tile([128, D], F32, tag="proj", name=f"ps_v_{tt}")
                for dk in range(4):
                    nc.tensor.matmul(ps[:], xt[:, dk, tt * 128:(tt + 1) * 128],
                                     W["wv"][:, dk, :], start=(dk == 0),
                                     stop=(dk == 3), skip_group_check=True)
                nc.scalar.copy(vtm[:, (c * 4 + tt) % 12, :, 0:64],
                               ps[:].rearrange("p (h d) -> p h d", h=H))

            def pool_chunk(c):
                base = c * CH
                ta = pool_tmp.tile([128, 4, CH + 2], BF16, tag="ta")
                nc.vector.tensor_add(ta[:], qraw[:, :, base:base + CH + 2],
                                     qraw[:, :, base + 1:base + CH + 3])
                tb = pool_tmp.tile([128, 4, CH], BF16, tag="tb")
                nc.vector.tensor_add(tb[:], ta[:, :, 0:CH], ta[:, :, 2:CH + 2])
                nc.vector.tensor_add(qpT[:, :, c % 3, :], tb[:],
                                     qraw[:, :, base + 4:base + CH + 4])

            def dsa_scores(c, lt):
                """MM1 + exp + mask for tile lt of chunk c -> masked sbuf tile."""
                st = ps_st.tile([128, 8, 128], F32, tag="st", name=f"st_{c}_{lt}")
                for h in range(H):
                    hp = PERM[h]
                    base = (h % 2) * 64
                    lhsT = kT[base:base + 64, h // 2, c % 3, lt * 128:(lt + 1) * 128]
                    rhs = qpT[base:base + 64, h // 2, c % 3, lt * 128:(lt + 1) * 128]
                    nc.tensor.matmul(st[:, hp, :], lhsT, rhs, start=True, stop=True,
                                     skip_group_check=True)
                expS = dsa_sb.tile([128, 8, 128], BF16, tag="expS",
                                   name=f"expS_{c}_{lt}")
                nc.scalar.activation(expS[:], st[:], AF.Exp, scale=SCALE / QNB)
                masked = dsa_sb.tile([128, 8, 128], BF16, tag="masked",
                                     name=f"masked_{c}_{lt}")
                nc.vector.tensor_mul(masked[:], expS[:],
                                     mask_sb[:].unsqueeze(1).to_broadcast((128, 8, 128)))
                return masked

            def dsa_out(c, lt, masked, ax_out):
                """attn@V with ones-col denominators, then normalize."""
                outp = ps_out.tile([128, 2, 512], F32, tag="outp",
                                   name=f"outp_{c}_{lt}")
                for h in range(H):
                    hp = PERM[h]
                    nc.tensor.matmul(outp[:, h // 4, (h % 4) * 65:(h % 4) * 65 + 65],
                                     masked[:, hp, :],
                                     vtm[:, (c * 4 + lt) % 12, h, :],
                                     start=True, stop=True, skip_group_check=True)
                recip = dsa_sb.tile([128, 2, 4], F32, tag="recip",
                                    name=f"recip_{c}_{lt}")
                den_view = bass.AP(outp.tensor, outp[:].offset + 64,
                                   [outp[:].ap[0], [512, 2], [65, 4]])
                nc.vector.reciprocal(recip[:], den_view)
                av_view = bass.AP(outp.tensor, outp[:].offset,
                                  [outp[:].ap[0], [512, 2], [65, 4], [1, 64]])
                nc.vector.tensor_mul(
                    ax_out.rearrange("p (a b d) -> p a b d", a=2, b=4),
                    av_view,
                    recip[:].unsqueeze(3).to_broadcast((128, 2, 4, 64)))

            def dsa_group_list(c, ax):
                masked = {}
                g = []
                g.append(lambda: masked.__setitem__(0, dsa_scores(c, 0)))
                g.append(lambda: masked.__setitem__(1, dsa_scores(c, 1)))
                g.append(lambda: dsa_out(c, 0, masked.pop(0), ax[:, 0, :]))
                g.append(lambda: masked.__setitem__(2, dsa_scores(c, 2)))
                g.append(lambda: dsa_out(c, 1, masked.pop(1), ax[:, 1, :]))
                g.append(lambda: masked.__setitem__(3, dsa_scores(c, 3)))
                g.append(lambda: dsa_out(c, 2, masked.pop(2), ax[:, 2, :]))
                g.append(lambda: dsa_out(c, 3, masked.pop(3), ax[:, 3, :]))
                return g

            def store_ax(c, ax):
                dst = axd.ap().rearrange("(cc lt p) d -> cc p lt d", lt=4, p=128)[c]
                wi = nc.gpsimd.dma_start(out=dst, in_=ax[:])
                axd_writers[c] = wi

            def issue_axt(c):
                ti = nc.sync.dma_start(
                    out=axt[:, :, c * CH:(c + 1) * CH],
                    in_=axd[c * CH:(c + 1) * CH, :],
                    transpose=True)
                add_dep_helper(ti.ins, axd_writers[c].ins,
                               reason="axt transpose reads axd chunk")

            for c in range(NCHUNK + 2):
                pgroups = []
                if c < NCHUNK:
                    qxt = load_xt("q", qb, c)
                    kxt = load_xt("k", kb, c)
                    vxt = load_xt("v", vb, c)
                    if c == 3:
                        load_p2_consts(xt_gate[("q", 2)])
                    for j in range(4):
                        pgroups.append(lambda j=j, x=qxt, c=c: proj_fm_group(
                            x, "wq", "bq_c",
                            lambda jj, c=c: qraw[:, jj, 2 + c * CH:2 + (c + 1) * CH], j))
                    for j in range(4):
                        pgroups.append(lambda j=j, x=kxt, c=c: proj_fm_group(
                            x, "wk", "bk_c", lambda jj, c=c: kT[:, jj, c % 3, :], j))
                    for tt in range(4):
                        pgroups.append(lambda tt=tt, x=vxt, c=c: proj_v_group(x, c, tt))
                dgroups = []
                ax = None
                if c >= 2:
                    ax = ax_pool.tile([128, 4, D], BF16, tag="ax", name=f"ax_{c - 2}")
                    dgroups = dsa_group_list(c - 2, ax)
                # weave: spread D groups evenly through the P stream;
                # pool(c-1) after the 4 Q-projection groups
                npg, ndg = len(pgroups), len(dgroups)
                dpos = {int(round((k + 1) * npg / (ndg + 1))): k for k in range(ndg)} \
                    if npg else {}
                for i in range(max(npg, 1)):
                    if i < npg:
                        pgroups[i]()
                    if i == 3 and 1 <= c <= NCHUNK:
                        pool_chunk(c - 1)
                    if i in dpos:
                        dgroups[dpos[i]]()
                if not pgroups:
                    if 1 <= c <= NCHUNK:
                        pool_chunk(c - 1)
                    for g in dgroups:
                        g()
                if ax is not None:
                    store_ax(c - 2, ax)


        # ================= phase 2 =================
        with ExitStack() as ctx:
            p2 = ctx.enter_context(tc.tile_pool(name="p2", bufs=1))

            pv = p2.tile([128, 4, WIN, D], BF16, tag="pv")
            wtn = p2.tile([128, 4, WN], BF16, tag="wtn")
            pqT = p2.tile([128, 4, WN], BF16, tag="pqT")
            pkT = p2.tile([128, 4, WN], BF16, tag="pkT")
            esA = p2.tile([128, H, 4, WN], BF16, tag="esA")
            zt = p2.tile([128, 4, QLEN], BF16, tag="zt")
            recip_sb = p2.tile([128, H, 4], F32, tag="recips")

            # issue axt transposes + pv gathers in dependency-arrival order
            srcv = axd.ap().rearrange("(cc p w) d -> cc p w d", p=128, w=PW)
            for c in range(NCHUNK):
                issue_axt(c)
                if c % 2 == 1:
                    cc = c // 2
                    gi = nc.sync.dma_start(out=pv[:, cc, :, :], in_=srcv[cc, :, 1:PW, :])
                    add_dep_helper(gi.ins, axd_writers[2 * cc].ins, reason="pv gather")
                    add_dep_helper(gi.ins, axd_writers[2 * cc + 1].ins, reason="pv gather")

            # ---- win_tok (+bv) LN + GELU + pq/pk, per 128-window group ----
            with ExitStack() as lctx:
                ps_ln = lctx.enter_context(
                    tc.tile_pool(name="ps_ln", bufs=3, space="PSUM"))
                lnp = lctx.enter_context(tc.tile_pool(name="lnp", bufs=2))

                lnA = {}

                def ln_phase_a(g):
                    """Moments + rstd; ACT funcs all within one table set
                    (Identity/Square/Copy/Sqrt)."""
                    wt_g = axt[:, :, g * GW * PW:(g + 1) * GW * PW:PW]
                    wtb = lnp.tile([128, 4, GW], BF16, tag="wtb", bufs=4,
                                   name=f"wtb_{g}")
                    for j in range(4):
                        nc.scalar.activation(wtb[:, j, :], wt_g[:, j, :],
                                             AF.Identity,
                                             bias=bias_cols["bv_c"][:, j:j + 1],
                                             scale=1.0)
                    wsq = lnp.tile([128, 4, GW], BF16, tag="wsq", name=f"wsq_{g}")
                    nc.scalar.activation(wsq[:], wtb[:], AF.Square)
                    ps_mu = ps_ln.tile([128, GW], F32, tag="psln", name=f"psmu_{g}")
                    ps_var = ps_ln.tile([128, GW], F32, tag="psln", name=f"psvar_{g}")
                    for j in range(4):
                        nc.tensor.matmul(ps_mu[:], ones_full[:], wtb[:, j, :],
                                         start=(j == 0), stop=(j == 3),
                                         skip_group_check=True)
                        nc.tensor.matmul(ps_var[:], ones_full[:], wsq[:, j, :],
                                         start=(j == 0), stop=(j == 3),
                                         skip_group_check=True)
                    mu = lnp.tile([128, GW], F32, tag="mu", bufs=4, name=f"mu_{g}")
                    nc.scalar.mul(mu[:], ps_mu[:], 1.0 / D)
                    ex2 = lnp.tile([128, GW], F32, tag="ex2", bufs=1,
                                   name=f"ex2_{g}")
                    nc.scalar.mul(ex2[:], ps_var[:], 1.0 / D)
                    var = lnp.tile([128, GW], F32, tag="var", bufs=1,
                                   name=f"var_{g}")
                    nc.vector.tensor_mul(var[:], mu[:], mu[:])
                    nc.vector.tensor_sub(var[:], ex2[:], var[:])
                    sd = lnp.tile([128, GW], F32, tag="sd", bufs=1, name=f"sd_{g}")
                    nc.scalar.activation(sd[:], var[:], AF.Sqrt, bias=eps_sb[:])
                    rstd = lnp.tile([128, GW], F32, tag="rstd", bufs=4,
                                    name=f"rstd_{g}")
                    nc.vector.reciprocal(rstd[:], sd[:])
                    lnA[g] = (wtb, mu, rstd)

                def ln_phase_b(g):
                    """GELU + pq/pk projections (Gelu/Identity table set)."""
                    gs = g * GW
                    wtb, mu, rstd = lnA.pop(g)
                    for j in range(4):
                        tmp = lnp.tile([128, GW], F32, tag="lnt", name=f"lnt_{g}_{j}")
                        nc.vector.tensor_sub(tmp[:], wtb[:, j, :], mu[:])
                        nc.vector.tensor_mul(tmp[:], tmp[:], rstd[:])
                        nc.scalar.activation(wtn[:, j, gs:gs + GW], tmp[:],
                                             AF.Gelu,
                                             bias=bias_cols["ln_b_c"][:, j:j + 1],
                                             scale=bias_cols["ln_g_c"][:, j:j + 1])
                    for dst, wname, bname in ((pqT, "wpq", "bpq_c"),
                                              (pkT, "wpk", "bpk_c")):
                        for j in range(4):
                            ps = ps_ln.tile([128, GW], F32, tag="psln",
                                            name=f"pp_{wname}_{g}_{j}")
                            for dk in range(4):
                                nc.tensor.matmul(
                                    ps[:], W[wname][:, dk, j * 128:(j + 1) * 128],
                                    wtn[:, dk, gs:gs + GW],
                                    start=(dk == 0), stop=(dk == 3))
                            nc.vector.tensor_scalar_add(
                                dst[:, j, gs:gs + GW], ps[:],
                                bias_cols[bname][:, j:j + 1])

                # A0..A2 then B0..B2 (one Sqrt->Gelu table switch), then the
                # last group's A3+B3 pair on the critical path (one more
                # switch pair).
                for g in range(NG - 1):
                    ln_phase_a(g)
                for g in range(NG - 1):
                    ln_phase_b(g)
                ln_phase_a(NG - 1)
                ln_phase_b(NG - 1)

            # ---- PSA: raw exp scores; den via N=1 matmuls; window-major pout
            with ExitStack() as pctx:
                # PSUM budget (8 banks): es/fin share slots (disjoint
                # lifetimes, same shape) 2 + po 2 + ztps 2 + den 1 = 7.
                ps_es = pctx.enter_context(
                    tc.tile_pool(name="ps_es", bufs=2, space="PSUM"))
                ps_po = pctx.enter_context(
                    tc.tile_pool(name="ps_po", bufs=2, space="PSUM"))
                ps_ztden = pctx.enter_context(
                    tc.tile_pool(name="ps_ztden", bufs=2, space="PSUM"))
                ps_fin = ps_es
                zwp = pctx.enter_context(tc.tile_pool(name="zwp", bufs=3))
                ztp = pctx.enter_context(tc.tile_pool(name="ztp", bufs=2))
                osb = pctx.enter_context(tc.tile_pool(name="osb", bufs=4))

                def psa_scores(h):
                    base = (h % 2) * 64
                    for cp in range(2):
                        ps = ps_es.tile([128, 2, WN], F32, tag="es",
                                        name=f"es_{h}_{cp}")
                        for k in range(2):
                            cc = cp * 2 + k
                            nc.tensor.matmul(
                                ps[:, k, :], pkT[base:base + 64, h // 2,
                                                 cc * 128:(cc + 1) * 128],
                                pqT[base:base + 64, h // 2, :],
                                start=True, stop=True, skip_group_check=True)
                        nc.scalar.activation(esA[:, h, 2 * cp:2 * cp + 2, :],
                                             ps[:], AF.Exp, scale=SCALE)

                def pout_one(h, qt, zwin):
                    # [pout | den] share one PSUM bank: cols 0:448 accumulate
                    # raw-exp attn @ pv, col 448 accumulates the softmax
                    # denominator against a ones column.
                    po = ps_po.tile([128, WIN * HD + 1], F32, tag="po",
                                    name=f"po_{h}_{qt}")
                    pov = po[:, 0:WIN * HD].rearrange("p (i d) -> p i d", i=WIN)
                    for cc in range(4):
                        nc.tensor.matmul(
                            pov, esA[:, h, cc, qt * 128:(qt + 1) * 128],
                            pv[:, cc, :, h * 64:(h + 1) * 64],
                            start=(cc == 0), stop=(cc == 3),
                            skip_group_check=True)
                    for cc in range(4):
                        nc.tensor.matmul(
                            po[:, WIN * HD:WIN * HD + 1],
                            esA[:, h, cc, qt * 128:(qt + 1) * 128],
                            ones_col[:], start=(cc == 0), stop=(cc == 3),
                            skip_group_check=True)
                    rc = recip_sb[:, h, qt:qt + 1]
                    nc.vector.reciprocal(rc, po[:, WIN * HD:WIN * HD + 1])
                    ztmp = ztp.tile([128, WIN, HD], BF16, tag="ztmp",
                                    name=f"ztmp_{h}_{qt}")
                    nc.vector.tensor_scalar_mul(ztmp[:], pov, rc)
                    nc.vector.tensor_add(zwin[:, :, h * 64:(h + 1) * 64], ztmp[:],
                                         pv[:, qt, :, h * 64:(h + 1) * 64])

                def ztrans_one(qt, ii, zwin):
                    """Transpose payload slots ii..ii+1 (or just ii at the
                    tail) of group qt into feature-major zt."""
                    ni = min(2, WIN - ii)
                    zt_ps = ps_ztden.tile([128, 4, 2, 128], BF16, tag="ztps",
                                          name=f"ztps_{qt}_{ii}")
                    for di in range(ni):
                        for fg in range(4):
                            nc.tensor.transpose(
                                zt_ps[:, fg, di, :],
                                zwin[:, ii + di, fg * 128:(fg + 1) * 128],
                                ident_sb[:])
                    base = qt * GW * WIN
                    dst = zt[:, :, base + ii:base + GW * WIN:WIN]
                    dst = bass.AP(dst.tensor, dst.offset,
                                  [dst.ap[0], dst.ap[1], [1, ni], [WIN, 128]])
                    src_ap = zt_ps[:, :, 0:ni, :]
                    nc.scalar.copy(dst, src_ap)

                osb_tiles = {}

                def fin_one(tt):
                    psf = ps_fin.tile([128, 2, WN], F32, tag="es",
                                      name=f"fin_{tt}")
                    ps = psf[:, 0, :]
                    for dk in range(4):
                        nc.tensor.matmul(ps, zt[:, dk, tt * 128:(tt + 1) * 128],
                                         W["wo"][:, dk, :], start=(dk == 0),
                                         stop=(dk == 3), skip_group_check=True)
                    ot = osb.tile([128, D], F32, tag="osb", name=f"osb_{tt}")
                    nc.vector.tensor_add(ot[:], ps, bo_sb[:])
                    outv = out.ap().rearrange("(tt p) d -> tt p d", p=128)
                    nc.sync.dma_start(out=outv[tt], in_=ot[:])

                # head-outer pipeline: as soon as head h's exp-scores are
                # done, its denominators and all four pout groups flow; the
                # transposes + final projections drain afterwards per group.
                for h in range(H):
                    psa_scores(h)

                zw = {}
                prev = None

                def tail_items(qt):
                    items = []
                    zwin_p = zw[qt]
                    for ii in range(0, WIN, 2):
                        items.append(lambda ii=ii: ztrans_one(qt, ii, zwin_p))
                    for j in range(WIN):
                        items.append(lambda j=j: fin_one(qt * WIN + j))
                    return items

                for qt in range(NG):
                    zw[qt] = zwp.tile([128, WIN, D], BF16, tag="zwin",
                                      name=f"zwin_{qt}")
                    titems = tail_items(prev) if prev is not None else []
                    ti = 0
                    for h in range(H):
                        pout_one(h, qt, zw[qt])
                        for _ in range(2):
                            if ti < len(titems) and h >= 3 and \
                                    ((h - 3) * 16) // H >= ti:
                                titems[ti]()
                                ti += 1
                    while ti < len(titems):
                        titems[ti]()
                        ti += 1
                    if prev is not None:
                        zw.pop(prev)
                    prev = qt
                for it in tail_items(prev):
                    it()


_NC_CACHE = None


def _get_program():
    global _NC_CACHE
    if _NC_CACHE is None:
        _NC_CACHE = build_program()
    return _NC_CACHE


def _host_consts(Wk, bk, Wv, bv, Wq, bq, ln_g, ln_b, Wpq, bpq, Wpk, bpk, Wo, bo):
    bf = ml_dtypes.bfloat16
    col = lambda b: np.asarray(b, np.float32).reshape(4, 128).T.copy()
    bo2 = np.asarray(bo, np.float32) + 2.0 * (
        np.asarray(bv, np.float32) @ np.asarray(Wo, np.float32))
    consts = {
        "wq": np.asarray(Wq, np.float32).astype(bf),
        "wk": np.asarray(Wk, np.float32).astype(bf),
        "wv": np.asarray(Wv, np.float32).astype(bf),
        "wpq": np.asarray(Wpq, np.float32).astype(bf),
        "wpk": np.asarray(Wpk, np.float32).astype(bf),
        "wo": np.asarray(Wo, np.float32).astype(bf),
        "bq_c": col(bq), "bk_c": col(bk),
        "bpq_c": col(bpq), "bpk_c": col(bpk),
        "ln_g_c": col(ln_g), "ln_b_c": col(ln_b),
        "bv_c": col(bv),
        "bo_r": bo2.reshape(1, D).astype(bf),
        "ident": np.eye(128, dtype=np.float32).astype(bf),
    }
    m = np.zeros((128, 128), np.float32)
    for g in range(16):
        m[g * PW:(g + 1) * PW, g * PW:(g + 1) * PW] = 1.0
    consts["bmask"] = m.astype(bf)
    return consts


def kernel(k, v, q, query_len, Wk, bk, Wv, bv, Wq, bq, ln_g, ln_b,
           Wpq, bpq, Wpk, bpk, Wo, bo):
    nc = _get_program()
    consts = _host_consts(Wk, bk, Wv, bv, Wq, bq, ln_g, ln_b,
                          Wpq, bpq, Wpk, bpk, Wo, bo)
    k = np.asarray(k, np.float32)
    v = np.asarray(v, np.float32)
    q = np.asarray(q, np.float32)
    in_maps = []
    for b in range(B):
        m = {"q": np.ascontiguousarray(q[b]), "k": np.ascontiguousarray(k[b]),
             "v": np.ascontiguousarray(v[b])}
        m.update(consts)
        in_maps.append(m)
    res = run_bass_kernel_spmd(nc, in_maps, core_ids=list(range(B)))
    return np.stack([res.results[b]["out"] for b in range(B)], axis=0)


if __name__ == "__main__":
    nc = build_program()
    print("program built ok")


# revision 72
# speedup vs baseline: 1.5343x; 1.0011x over previous
"""Trainium2 Bass kernel for DeformableMultiHeadedAttention.

Data-parallel over batch B=8 across 8 NeuronCores (one batch element per
core, identical programs, no collectives).

Per-core pipeline (matmuls bf16 with f32 accumulate):
  1. q,k,v [4096,512] f32 -> SWDGE cast-DMA -> DRAM bf16 -> batched HWDGE
     DMA-transpose (one [512,512] xbar transpose per chunk) -> feature-major
     XT [128,4,tok] chunks in SBUF.
  2. Projections on PE: K'T/Q'T feature-major (lhsT=W, rhs=XT); V' token-major
     (lhsT=XT tile, rhs=W), bv folded out on host (bo' = bo + 2*bv@Wo, LN
     input gets +bv on chip).
  3. Q pooling (AvgPool k=5, stride 1, zero pad) as 3 shifted adds; the 1/5
     is folded into the softmax exp scale.
  4. DSA (windows of 8 tokens): per 128-token tile, 8 heads: S_T[k,q] on PE,
     exp on ACT, block-diag mask mul on DVE, attn@V plus ones-col denominator
     sharing the lhsT, per-partition 1/den scale on DVE. Token-major DSA
     output -> DRAM (bf16).
  5. DRAM round-trips: batched DMA-transpose -> attn_xT feature-major;
     strided gather -> PV window-major [kw, (slot, head, hd)].
  6. Incremental (per 128-window group, overlapping phase 1 tail): win_tok
     +bv, LayerNorm moments via ones-matmuls, exact GELU, pq/pk projections.
  7. PSA restructured: raw exp-scores kept unnormalized; denominators via
     N=1 matmuls against a ones column (per-partition 1/den on DVE); pout
     window-major [wq, (slot,hd)] at M=128 (half the PE rows of the
     feature-major form); z = pout*recip + attn_x in window-major form;
     PE identity-transposes + strided ACT copies build feature-major zT.
  8. out = Z @ Wo + bo' with Z as the stationary operand -> token-major f32
     output, streamed per window-group.
"""

import sys
from contextlib import ExitStack

for _p in ("/opt/trn_rl_repo/concourse", "/opt/trn_rl_repo"):
    if _p not in sys.path:
        sys.path.insert(0, _p)

import numpy as np
import ml_dtypes

import concourse.bass as bass
import concourse.mybir as mybir
import concourse.tile as tile
from concourse import bacc
from concourse.tile import add_dep_helper
from concourse.bass_utils import run_bass_kernel_spmd

BF16 = mybir.dt.bfloat16
F32 = mybir.dt.float32
AF = mybir.ActivationFunctionType
ALU = mybir.AluOpType

B, M, D = 8, 4096, 512
H, HD = 8, 64
WIN = 7
PW = WIN + 1
QNB = 5
QLEN = 3584
WN = M // PW
SCALE = D ** -0.5
EPS = 1e-5
NCHUNK = 8
CH = 512
NG = 4                   # window groups of 128 for phase 2
GW = WN // NG            # 128 windows per group
PERM = [(h % 2) * 4 + h // 2 for h in range(H)]  # head -> DSA psum slot


def build_program():
    nc = bacc.Bacc("TRN2", target_bir_lowering=False, debug=False, num_devices=8)

    t = {}
    t["q_in"] = nc.dram_tensor("q", [M, D], F32, kind="ExternalInput")
    t["k_in"] = nc.dram_tensor("k", [M, D], F32, kind="ExternalInput")
    t["v_in"] = nc.dram_tensor("v", [M, D], F32, kind="ExternalInput")
    for nm in ("wq", "wk", "wv", "wpq", "wpk", "wo"):
        t[nm] = nc.dram_tensor(nm, [D, D], BF16, kind="ExternalInput")
    for nm in ("bq_c", "bk_c", "bpq_c", "bpk_c", "ln_g_c", "ln_b_c", "bv_c"):
        t[nm] = nc.dram_tensor(nm, [128, 4], F32, kind="ExternalInput")
    t["bo_r"] = nc.dram_tensor("bo_r", [1, D], BF16, kind="ExternalInput")
    t["bmask"] = nc.dram_tensor("bmask", [128, 128], BF16, kind="ExternalInput")
    t["ident"] = nc.dram_tensor("ident", [128, 128], BF16, kind="ExternalInput")
    t["out"] = nc.dram_tensor("out", [QLEN, D], F32, kind="ExternalOutput")
    t["axd"] = nc.dram_tensor("axd_s", [M, D], BF16, kind="Internal")
    t["zd"] = nc.dram_tensor("zd_s", [QLEN, D], BF16, kind="Internal")
    t["qb"] = nc.dram_tensor("qb_s", [M, D], BF16, kind="Internal")
    t["kb"] = nc.dram_tensor("kb_s", [M, D], BF16, kind="Internal")
    t["vb"] = nc.dram_tensor("vb_s", [M, D], BF16, kind="Internal")

    with tile.TileContext(nc) as tc:
        _build(nc, tc, t)
    nc.compile()
    return nc


def _build(nc, tc, t):
    qb, kb, vb = t["qb"], t["kb"], t["vb"]
    axd, out = t["axd"], t["out"]
    zd = t["zd"]

    with ExitStack() as octx:
        singles = octx.enter_context(tc.tile_pool(name="singles", bufs=1))

        # phase-1 weights first (needed by the first projections), then the
        # input casts, then everything else so the casts win the DMA engines.
        cast_insts = {"q": [], "k": [], "v": []}

        def issue_casts(lo, hi):
            for nm, srcd, dst in (("q", t["q_in"], qb), ("k", t["k_in"], kb),
                                  ("v", t["v_in"], vb)):
                ci = nc.gpsimd.dma_start(
                    out=dst[lo * CH:hi * CH, :],
                    in_=srcd[lo * CH:hi * CH, :])
                cast_insts[nm].append(((lo, hi), ci))

        issue_casts(0, 1)
        W = {}

        def load_w(nm):
            W[nm] = singles.tile([128, 4, D], BF16, tag=nm, name=f"w_{nm}")
            nc.scalar.dma_start(out=W[nm][:],
                                in_=t[nm].ap().rearrange("(c p) d -> p c d", p=128))

        load_w("wq")
        issue_casts(1, 2)
        load_w("wk")
        load_w("wv")

        bias_cols = {}
        for nm in ("bq_c", "bk_c"):
            bias_cols[nm] = singles.tile([128, 4], F32, tag=nm, name=f"bc_{nm}")
            nc.scalar.dma_start(out=bias_cols[nm][:], in_=t[nm][:, :])
        mask_sb = singles.tile([128, 128], BF16)
        nc.scalar.dma_start(out=mask_sb[:], in_=t["bmask"][:, :])
        ones_row = singles.tile([1, 128], BF16)
        nc.vector.memset(ones_row[:], 1.0)
        ones_col = singles.tile([128, 1], BF16)
        nc.vector.memset(ones_col[:], 1.0)
        ones_full = singles.tile([128, 128], BF16)
        nc.vector.memset(ones_full[:], 1.0)
        eps_sb = singles.tile([128, 1], F32)
        nc.vector.memset(eps_sb[:], EPS)

        issue_casts(2, 4)
        issue_casts(4, 6)
        issue_casts(6, 8)

        axd_writers = {}
        p2a = octx.enter_context(tc.tile_pool(name="p2a", bufs=1))
        axt = p2a.tile([128, 4, M], BF16, tag="axt")

        # ================= phase 2 weights ==============================
        # deferred behind the early input casts so they don't hog the DMA
        # engines during the pipeline ramp
        # Allocated here; DMAs issued mid-phase-1 (see chunk loop, c==3)
        # on the sync queue so they neither hog the DMA engines at startup
        # nor get scheduled into the phase boundary.
        for nm in ("wpq", "wpk", "wo"):
            W[nm] = singles.tile([128, 4, D], BF16, tag=nm, name=f"w_{nm}")
        for nm in ("bpq_c", "bpk_c", "ln_g_c", "ln_b_c", "bv_c"):
            bias_cols[nm] = singles.tile([128, 4], F32, tag=nm, name=f"bc_{nm}")
        bo_sb = singles.tile([128, D], BF16)
        ident_sb = singles.tile([128, 128], BF16)

        def load_p2_consts(gate):
            dis = []
            for nm in ("wpq", "wpk", "wo"):
                dis.append(nc.sync.dma_start(
                    out=W[nm][:],
                    in_=t[nm].ap().rearrange("(c p) d -> p c d", p=128)))
            for nm in ("bpq_c", "bpk_c", "ln_g_c", "ln_b_c", "bv_c"):
                dis.append(nc.sync.dma_start(out=bias_cols[nm][:], in_=t[nm][:, :]))
            dis.append(nc.sync.dma_start(
                out=bo_sb[:],
                in_=t["bo_r"].ap().to_broadcast((128, D))))
            dis.append(nc.sync.dma_start(out=ident_sb[:], in_=t["ident"][:, :]))
            for di in dis:
                add_dep_helper(di.ins, gate.ins,
                               reason="const loads after startup transposes")

        # ================= phase 1 =================
        with ExitStack() as ctx:
            p1 = ctx.enter_context(tc.tile_pool(name="p1", bufs=1))
            kT = p1.tile([128, 4, 3, CH], BF16, tag="kT")        # ring of 3 chunks
            qpT = p1.tile([128, 4, 3, CH], BF16, tag="qpT")      # ring of 3 chunks
            vtm = p1.tile([128, 12, 8, 65], BF16, tag="vtm")     # ring of 12 tiles, 65-col/head
            nc.vector.memset(vtm[:, :, :, 64:65], 1.0)           # ones col for denominators
            qraw = p1.tile([128, 4, M + 4], BF16, tag="qraw")    # full, padded +-2
            nc.vector.memset(qraw[:, :, 0:2], 0.0)
            nc.vector.memset(qraw[:, :, M + 2:M + 4], 0.0)

            xtp = ctx.enter_context(tc.tile_pool(name="xtp", bufs=2))
            ps_proj = ctx.enter_context(tc.tile_pool(name="ps_proj", bufs=2, space="PSUM"))
            ps_st = ctx.enter_context(tc.tile_pool(name="ps_st", bufs=2, space="PSUM"))
            ps_out = ctx.enter_context(tc.tile_pool(name="ps_out", bufs=1, space="PSUM"))
            dsa_sb = ctx.enter_context(tc.tile_pool(name="dsa_sb", bufs=2))
            pool_tmp = ctx.enter_context(tc.tile_pool(name="pool_tmp", bufs=2))
            ax_pool = ctx.enter_context(tc.tile_pool(name="ax_sb", bufs=2))

            xt_gate = {}

            def load_xt(nm, dram, c):
                xt = xtp.tile([128, 4, CH], BF16, tag=f"xt_{nm}", name=f"xt_{nm}_{c}")
                ti = nc.sync.dma_start(out=xt[:],
                                       in_=dram[c * CH:(c + 1) * CH, :],
                                       transpose=True)
                for (lo, hi), ci in cast_insts[nm]:
                    if lo <= c < hi:
                        add_dep_helper(ti.ins, ci.ins,
                                       reason="transpose reads cast output")
                xt_gate[(nm, c)] = ti
                return xt

            def proj_fm_group(xt, wname, bname, dst_fn, j):
                ps = ps_proj.tile([128, CH], F32, tag="proj",
                                  name=f"ps_{wname}_{j}")
                for dk in range(4):
                    nc.tensor.matmul(ps[:], W[wname][:, dk, j * 128:(j + 1) * 128],
                                     xt[:, dk, :], start=(dk == 0), stop=(dk == 3))
                nc.scalar.activation(dst_fn(j), ps[:], AF.Identity,
                                     bias=bias_cols[bname][:, j:j + 1], scale=1.0)

            def proj_v_group(xt, c, tt):
                ps = ps_proj.tile([128, D], F32, tag="proj", name=f"ps_v_{tt}")
                for dk in range(4):
                    nc.tensor.matmul(ps[:], xt[:, dk, tt * 128:(tt + 1) * 128],
                                     W["wv"][:, dk, :], start=(dk == 0),
                                     stop=(dk == 3), skip_group_check=True)
                nc.scalar.copy(vtm[:, (c * 4 + tt) % 12, :, 0:64],
                               ps[:].rearrange("p (h d) -> p h d", h=H))

            def pool_chunk(c):
                base = c * CH
                ta = pool_tmp.tile([128, 4, CH + 2], BF16, tag="ta")
                nc.vector.tensor_add(ta[:], qraw[:, :, base:base + CH + 2],
                                     qraw[:, :, base + 1:base + CH + 3])
                tb = pool_tmp.tile([128, 4, CH], BF16, tag="tb")
                nc.vector.tensor_add(tb[:], ta[:, :, 0:CH], ta[:, :, 2:CH + 2])
                nc.vector.tensor_add(qpT[:, :, c % 3, :], tb[:],
                                     qraw[:, :, base + 4:base + CH + 4])

            def dsa_scores(c, lt):
                """MM1 + exp + mask for tile lt of chunk c -> masked sbuf tile."""
                st = ps_st.tile([128, 8, 128], F32, tag="st", name=f"st_{c}_{lt}")
                for h in range(H):
                    hp = PERM[h]
                    base = (h % 2) * 64
                    lhsT = kT[base:base + 64, h // 2, c % 3, lt * 128:(lt + 1) * 128]
                    rhs = qpT[base:base + 64, h // 2, c % 3, lt * 128:(lt + 1) * 128]
                    nc.tensor.matmul(st[:, hp, :], lhsT, rhs, start=True, stop=True,
                                     skip_group_check=True)
                expS = dsa_sb.tile([128, 8, 128], BF16, tag="expS",
                                   name=f"expS_{c}_{lt}")
                nc.scalar.activation(expS[:], st[:], AF.Exp, scale=SCALE / QNB)
                masked = dsa_sb.tile([128, 8, 128], BF16, tag="masked",
                                     name=f"masked_{c}_{lt}")
                nc.vector.tensor_mul(masked[:], expS[:],
                                     mask_sb[:].unsqueeze(1).to_broadcast((128, 8, 128)))
                return masked

            def dsa_out(c, lt, masked, ax_out):
                """attn@V with ones-col denominators, then normalize."""
                outp = ps_out.tile([128, 2, 512], F32, tag="outp",
                                   name=f"outp_{c}_{lt}")
                for h in range(H):
                    hp = PERM[h]
                    nc.tensor.matmul(outp[:, h // 4, (h % 4) * 65:(h % 4) * 65 + 65],
                                     masked[:, hp, :],
                                     vtm[:, (c * 4 + lt) % 12, h, :],
                                     start=True, stop=True, skip_group_check=True)
                recip = dsa_sb.tile([128, 2, 4], F32, tag="recip",
                                    name=f"recip_{c}_{lt}")
                den_view = bass.AP(outp.tensor, outp[:].offset + 64,
                                   [outp[:].ap[0], [512, 2], [65, 4]])
                nc.vector.reciprocal(recip[:], den_view)
                av_view = bass.AP(outp.tensor, outp[:].offset,
                                  [outp[:].ap[0], [512, 2], [65, 4], [1, 64]])
                nc.vector.tensor_mul(
                    ax_out.rearrange("p (a b d) -> p a b d", a=2, b=4),
                    av_view,
                    recip[:].unsqueeze(3).to_broadcast((128, 2, 4, 64)))

            def dsa_group_list(c, ax):
                masked = {}
                g = []
                g.append(lambda: masked.__setitem__(0, dsa_scores(c, 0)))
                g.append(lambda: masked.__setitem__(1, dsa_scores(c, 1)))
                g.append(lambda: dsa_out(c, 0, masked.pop(0), ax[:, 0, :]))
                g.append(lambda: masked.__setitem__(2, dsa_scores(c, 2)))
                g.append(lambda: dsa_out(c, 1, masked.pop(1), ax[:, 1, :]))
                g.append(lambda: masked.__setitem__(3, dsa_scores(c, 3)))
                g.append(lambda: dsa_out(c, 2, masked.pop(2), ax[:, 2, :]))
                g.append(lambda: dsa_out(c, 3, masked.pop(3), ax[:, 3, :]))
                return g

            def store_ax(c, ax):
                dst = axd.ap().rearrange("(cc lt p) d -> cc p lt d", lt=4, p=128)[c]
                wi = nc.gpsimd.dma_start(out=dst, in_=ax[:])
                axd_writers[c] = wi

            def issue_axt(c):
                ti = nc.sync.dma_start(
                    out=axt[:, :, c * CH:(c + 1) * CH],
                    in_=axd[c * CH:(c + 1) * CH, :],
                    transpose=True)
                add_dep_helper(ti.ins, axd_writers[c].ins,
                               reason="axt transpose reads axd chunk")

            for c in range(NCHUNK + 2):
                pgroups = []
                if c < NCHUNK:
                    qxt = load_xt("q", qb, c)
                    kxt = load_xt("k", kb, c)
                    vxt = load_xt("v", vb, c)
                    if c == 3:
                        load_p2_consts(xt_gate[("q", 2)])
                    for j in range(4):
                        pgroups.append(lambda j=j, x=qxt, c=c: proj_fm_group(
                            x, "wq", "bq_c",
                            lambda jj, c=c: qraw[:, jj, 2 + c * CH:2 + (c + 1) * CH], j))
                    for j in range(4):
                        pgroups.append(lambda j=j, x=kxt, c=c: proj_fm_group(
                            x, "wk", "bk_c", lambda jj, c=c: kT[:, jj, c % 3, :], j))
                    for tt in range(4):
                        pgroups.append(lambda tt=tt, x=vxt, c=c: proj_v_group(x, c, tt))
                dgroups = []
                ax = None
                if c >= 2:
                    ax = ax_pool.tile([128, 4, D], BF16, tag="ax", name=f"ax_{c - 2}")
                    dgroups = dsa_group_list(c - 2, ax)
                # weave: spread D groups evenly through the P stream;
                # pool(c-1) after the 4 Q-projection groups
                npg, ndg = len(pgroups), len(dgroups)
                dpos = {int(round((k + 1) * npg / (ndg + 1))): k for k in range(ndg)} \
                    if npg else {}
                for i in range(max(npg, 1)):
                    if i < npg:
                        pgroups[i]()
                    if i == 3 and 1 <= c <= NCHUNK:
                        pool_chunk(c - 1)
                    if i in dpos:
                        dgroups[dpos[i]]()
                if not pgroups:
                    if 1 <= c <= NCHUNK:
                        pool_chunk(c - 1)
                    for g in dgroups:
                        g()
                if ax is not None:
                    store_ax(c - 2, ax)


        # ================= phase 2 =================
        with ExitStack() as ctx:
            p2 = ctx.enter_context(tc.tile_pool(name="p2", bufs=1))

            pv = p2.tile([128, 4, WIN, D], BF16, tag="pv")
            wtn = p2.tile([128, 4, WN], BF16, tag="wtn")
            pqT = p2.tile([128, 4, WN], BF16, tag="pqT")
            pkT = p2.tile([128, 4, WN], BF16, tag="pkT")
            esA = p2.tile([128, H, 4, WN], BF16, tag="esA")
            zt = p2.tile([128, 4, QLEN], BF16, tag="zt")
            recip_sb = p2.tile([128, H, 4], F32, tag="recips")

            # issue axt transposes + pv gathers in dependency-arrival order
            srcv = axd.ap().rearrange("(cc p w) d -> cc p w d", p=128, w=PW)
            for c in range(NCHUNK):
                issue_axt(c)
                if c % 2 == 1:
                    cc = c // 2
                    gi = nc.sync.dma_start(out=pv[:, cc, :, :], in_=srcv[cc, :, 1:PW, :])
                    add_dep_helper(gi.ins, axd_writers[2 * cc].ins, reason="pv gather")
                    add_dep_helper(gi.ins, axd_writers[2 * cc + 1].ins, reason="pv gather")

            # ---- win_tok (+bv) LN + GELU + pq/pk, per 128-window group ----
            with ExitStack() as lctx:
                ps_ln = lctx.enter_context(
                    tc.tile_pool(name="ps_ln", bufs=3, space="PSUM"))
                lnp = lctx.enter_context(tc.tile_pool(name="lnp", bufs=2))

                lnA = {}

                def ln_phase_a(g):
                    """Moments + rstd; ACT funcs all within one table set
                    (Identity/Square/Copy/Sqrt)."""
                    wt_g = axt[:, :, g * GW * PW:(g + 1) * GW * PW:PW]
                    wtb = lnp.tile([128, 4, GW], BF16, tag="wtb", bufs=4,
                                   name=f"wtb_{g}")
                    for j in range(4):
                        nc.scalar.activation(wtb[:, j, :], wt_g[:, j, :],
                                             AF.Identity,
                                             bias=bias_cols["bv_c"][:, j:j + 1],
                                             scale=1.0)
                    wsq = lnp.tile([128, 4, GW], BF16, tag="wsq", name=f"wsq_{g}")
                    nc.scalar.activation(wsq[:], wtb[:], AF.Square)
                    ps_mu = ps_ln.tile([128, GW], F32, tag="psln", name=f"psmu_{g}")
                    ps_var = ps_ln.tile([128, GW], F32, tag="psln", name=f"psvar_{g}")
                    for j in range(4):
                        nc.tensor.matmul(ps_mu[:], ones_full[:], wtb[:, j, :],
                                         start=(j == 0), stop=(j == 3),
                                         skip_group_check=True)
                        nc.tensor.matmul(ps_var[:], ones_full[:], wsq[:, j, :],
                                         start=(j == 0), stop=(j == 3),
                                         skip_group_check=True)
                    mu = lnp.tile([128, GW], F32, tag="mu", bufs=4, name=f"mu_{g}")
                    nc.scalar.mul(mu[:], ps_mu[:], 1.0 / D)
                    ex2 = lnp.tile([128, GW], F32, tag="ex2", bufs=1,
                                   name=f"ex2_{g}")
                    nc.scalar.mul(ex2[:], ps_var[:], 1.0 / D)
                    var = lnp.tile([128, GW], F32, tag="var", bufs=1,
                                   name=f"var_{g}")
                    nc.vector.tensor_mul(var[:], mu[:], mu[:])
                    nc.vector.tensor_sub(var[:], ex2[:], var[:])
                    sd = lnp.tile([128, GW], F32, tag="sd", bufs=1, name=f"sd_{g}")
                    nc.scalar.activation(sd[:], var[:], AF.Sqrt, bias=eps_sb[:])
                    rstd = lnp.tile([128, GW], F32, tag="rstd", bufs=4,
                                    name=f"rstd_{g}")
                    nc.vector.reciprocal(rstd[:], sd[:])
                    lnA[g] = (wtb, mu, rstd)

                def ln_phase_b(g):
                    """GELU + pq/pk projections (Gelu/Identity table set)."""
                    gs = g * GW
                    wtb, mu, rstd = lnA.pop(g)
                    for j in range(4):
                        tmp = lnp.tile([128, GW], F32, tag="lnt", name=f"lnt_{g}_{j}")
                        nc.vector.tensor_sub(tmp[:], wtb[:, j, :], mu[:])
                        nc.vector.tensor_mul(tmp[:], tmp[:], rstd[:])
                        nc.scalar.activation(wtn[:, j, gs:gs + GW], tmp[:],
                                             AF.Gelu,
                                             bias=bias_cols["ln_b_c"][:, j:j + 1],
                                             scale=bias_cols["ln_g_c"][:, j:j + 1])
                    for dst, wname, bname in ((pqT, "wpq", "bpq_c"),
                                              (pkT, "wpk", "bpk_c")):
                        for j in range(4):
                            ps = ps_ln.tile([128, GW], F32, tag="psln",
                                            name=f"pp_{wname}_{g}_{j}")
                            for dk in range(4):
                                nc.tensor.matmul(
                                    ps[:], W[wname][:, dk, j * 128:(j + 1) * 128],
                                    wtn[:, dk, gs:gs + GW],
                                    start=(dk == 0), stop=(dk == 3))
                            nc.vector.tensor_scalar_add(
                                dst[:, j, gs:gs + GW], ps[:],
                                bias_cols[bname][:, j:j + 1])

                # A0..A2 then B0..B2 (one Sqrt->Gelu table switch), then the
                # last group's A3+B3 pair on the critical path (one more
                # switch pair).
                for g in range(NG - 1):
                    ln_phase_a(g)
                for g in range(NG - 1):
                    ln_phase_b(g)
                ln_phase_a(NG - 1)
                ln_phase_b(NG - 1)

            # ---- PSA: raw exp scores; den via N=1 matmuls; window-major pout
            with ExitStack() as pctx:
                # PSUM budget (8 banks): es/fin share slots (disjoint
                # lifetimes, same shape) 2 + po 2 + ztps 2 + den 1 = 7.
                ps_es = pctx.enter_context(
                    tc.tile_pool(name="ps_es", bufs=2, space="PSUM"))
                ps_po = pctx.enter_context(
                    tc.tile_pool(name="ps_po", bufs=2, space="PSUM"))
                ps_ztden = pctx.enter_context(
                    tc.tile_pool(name="ps_ztden", bufs=2, space="PSUM"))
                ps_fin = ps_es
                zwp = pctx.enter_context(tc.tile_pool(name="zwp", bufs=3))
                ztp = pctx.enter_context(tc.tile_pool(name="ztp", bufs=2))
                osb = pctx.enter_context(tc.tile_pool(name="osb", bufs=6))

                def psa_scores(h):
                    base = (h % 2) * 64
                    for cp in range(2):
                        ps = ps_es.tile([128, 2, WN], F32, tag="es",
                                        name=f"es_{h}_{cp}")
                        for k in range(2):
                            cc = cp * 2 + k
                            nc.tensor.matmul(
                                ps[:, k, :], pkT[base:base + 64, h // 2,
                                                 cc * 128:(cc + 1) * 128],
                                pqT[base:base + 64, h // 2, :],
                                start=True, stop=True, skip_group_check=True)
                        nc.scalar.activation(esA[:, h, 2 * cp:2 * cp + 2, :],
                                             ps[:], AF.Exp, scale=SCALE)

                def pout_one(h, qt, zwin):
                    # [pout | den] share one PSUM bank: cols 0:448 accumulate
                    # raw-exp attn @ pv, col 448 accumulates the softmax
                    # denominator against a ones column.
                    po = ps_po.tile([128, WIN * HD + 1], F32, tag="po",
                                    name=f"po_{h}_{qt}")
                    pov = po[:, 0:WIN * HD].rearrange("p (i d) -> p i d", i=WIN)
                    for cc in range(4):
                        nc.tensor.matmul(
                            pov, esA[:, h, cc, qt * 128:(qt + 1) * 128],
                            pv[:, cc, :, h * 64:(h + 1) * 64],
                            start=(cc == 0), stop=(cc == 3),
                            skip_group_check=True)
                    for cc in range(4):
                        nc.tensor.matmul(
                            po[:, WIN * HD:WIN * HD + 1],
                            esA[:, h, cc, qt * 128:(qt + 1) * 128],
                            ones_col[:], start=(cc == 0), stop=(cc == 3),
                            skip_group_check=True)
                    rc = recip_sb[:, h, qt:qt + 1]
                    nc.vector.reciprocal(rc, po[:, WIN * HD:WIN * HD + 1])
                    ztmp = ztp.tile([128, WIN, HD], BF16, tag="ztmp",
                                    name=f"ztmp_{h}_{qt}")
                    nc.vector.tensor_scalar_mul(ztmp[:], pov, rc)
                    nc.vector.tensor_add(zwin[:, :, h * 64:(h + 1) * 64], ztmp[:],
                                         pv[:, qt, :, h * 64:(h + 1) * 64])

                def ztrans_one(qt, ii, zwin):
                    """Transpose payload slots ii..ii+1 (or just ii at the
                    tail) of group qt into feature-major zt."""
                    ni = min(2, WIN - ii)
                    zt_ps = ps_ztden.tile([128, 4, 2, 128], BF16, tag="ztps",
                                          name=f"ztps_{qt}_{ii}")
                    for di in range(ni):
                        for fg in range(4):
                            nc.tensor.transpose(
                                zt_ps[:, fg, di, :],
                                zwin[:, ii + di, fg * 128:(fg + 1) * 128],
                                ident_sb[:])
                    base = qt * GW * WIN
                    dst = zt[:, :, base + ii:base + GW * WIN:WIN]
                    dst = bass.AP(dst.tensor, dst.offset,
                                  [dst.ap[0], dst.ap[1], [1, ni], [WIN, 128]])
                    src_ap = zt_ps[:, :, 0:ni, :]
                    nc.scalar.copy(dst, src_ap)

                osb_tiles = {}

                def fin_one(tt):
                    psf = ps_fin.tile([128, 2, WN], F32, tag="es",
                                      name=f"fin_{tt}")
                    ps = psf[:, 0, :]
                    for dk in range(4):
                        nc.tensor.matmul(ps, zt[:, dk, tt * 128:(tt + 1) * 128],
                                         W["wo"][:, dk, :], start=(dk == 0),
                                         stop=(dk == 3), skip_group_check=True)
                    ot = osb.tile([128, D], F32, tag="osb", name=f"osb_{tt}")
                    nc.vector.tensor_add(ot[:], ps, bo_sb[:])
                    outv = out.ap().rearrange("(tt p) d -> tt p d", p=128)
                    nc.sync.dma_start(out=outv[tt], in_=ot[:])

                # head-outer pipeline: as soon as head h's exp-scores are
                # done, its denominators and all four pout groups flow; the
                # transposes + final projections drain afterwards per group.
                for h in range(H):
                    psa_scores(h)

                zw = {}
                prev = None

                def tail_items(qt):
                    items = []
                    zwin_p = zw[qt]
                    for ii in range(0, WIN, 2):
                        items.append(lambda ii=ii: ztrans_one(qt, ii, zwin_p))
                    for j in range(WIN):
                        items.append(lambda j=j: fin_one(qt * WIN + j))
                    return items

                for qt in range(NG):
                    zw[qt] = zwp.tile([128, WIN, D], BF16, tag="zwin",
                                      name=f"zwin_{qt}")
                    titems = tail_items(prev) if prev is not None else []
                    ti = 0
                    for h in range(H):
                        pout_one(h, qt, zw[qt])
                        for _ in range(2):
                            if ti < len(titems) and h >= 3 and \
                                    ((h - 3) * 16) // H >= ti:
                                titems[ti]()
                                ti += 1
                    while ti < len(titems):
                        titems[ti]()
                        ti += 1
                    if prev is not None:
                        zw.pop(prev)
                    prev = qt
                for it in tail_items(prev):
                    it()


_NC_CACHE = None


def _get_program():
    global _NC_CACHE
    if _NC_CACHE is None:
        _NC_CACHE = build_program()
    return _NC_CACHE


def _host_consts(Wk, bk, Wv, bv, Wq, bq, ln_g, ln_b, Wpq, bpq, Wpk, bpk, Wo, bo):
    bf = ml_dtypes.bfloat16
    col = lambda b: np.asarray(b, np.float32).reshape(4, 128).T.copy()
    bo2 = np.asarray(bo, np.float32) + 2.0 * (
        np.asarray(bv, np.float32) @ np.asarray(Wo, np.float32))
    consts = {
        "wq": np.asarray(Wq, np.float32).astype(bf),
        "wk": np.asarray(Wk, np.float32).astype(bf),
        "wv": np.asarray(Wv, np.float32).astype(bf),
        "wpq": np.asarray(Wpq, np.float32).astype(bf),
        "wpk": np.asarray(Wpk, np.float32).astype(bf),
        "wo": np.asarray(Wo, np.float32).astype(bf),
        "bq_c": col(bq), "bk_c": col(bk),
        "bpq_c": col(bpq), "bpk_c": col(bpk),
        "ln_g_c": col(ln_g), "ln_b_c": col(ln_b),
        "bv_c": col(bv),
        "bo_r": bo2.reshape(1, D).astype(bf),
        "ident": np.eye(128, dtype=np.float32).astype(bf),
    }
    m = np.zeros((128, 128), np.float32)
    for g in range(16):
        m[g * PW:(g + 1) * PW, g * PW:(g + 1) * PW] = 1.0
    consts["bmask"] = m.astype(bf)
    return consts


def kernel(k, v, q, query_len, Wk, bk, Wv, bv, Wq, bq, ln_g, ln_b,
           Wpq, bpq, Wpk, bpk, Wo, bo):
    nc = _get_program()
    consts = _host_consts(Wk, bk, Wv, bv, Wq, bq, ln_g, ln_b,
                          Wpq, bpq, Wpk, bpk, Wo, bo)
    k = np.asarray(k, np.float32)
    v = np.asarray(v, np.float32)
    q = np.asarray(q, np.float32)
    in_maps = []
    for b in range(B):
        m = {"q": np.ascontiguousarray(q[b]), "k": np.ascontiguousarray(k[b]),
             "v": np.ascontiguousarray(v[b])}
        m.update(consts)
        in_maps.append(m)
    res = run_bass_kernel_spmd(nc, in_maps, core_ids=list(range(B)))
    return np.stack([res.results[b]["out"] for b in range(B)], axis=0)


if __name__ == "__main__":
    nc = build_program()
    print("program built ok")


# revision 75
# speedup vs baseline: 1.5493x; 1.0098x over previous
"""Trainium2 Bass kernel for DeformableMultiHeadedAttention.

Data-parallel over batch B=8 across 8 NeuronCores (one batch element per
core, identical programs, no collectives).

Per-core pipeline (matmuls bf16 with f32 accumulate):
  1. q,k,v [4096,512] f32 -> SWDGE cast-DMA -> DRAM bf16 -> batched HWDGE
     DMA-transpose (one [512,512] xbar transpose per chunk) -> feature-major
     XT [128,4,tok] chunks in SBUF.
  2. Projections on PE: K'T/Q'T feature-major (lhsT=W, rhs=XT); V' token-major
     (lhsT=XT tile, rhs=W), bv folded out on host (bo' = bo + 2*bv@Wo, LN
     input gets +bv on chip).
  3. Q pooling (AvgPool k=5, stride 1, zero pad) as 3 shifted adds; the 1/5
     is folded into the softmax exp scale.
  4. DSA (windows of 8 tokens): per 128-token tile, 8 heads: S_T[k,q] on PE,
     exp on ACT, block-diag mask mul on DVE, attn@V plus ones-col denominator
     sharing the lhsT, per-partition 1/den scale on DVE. Token-major DSA
     output -> DRAM (bf16).
  5. DRAM round-trips: batched DMA-transpose -> attn_xT feature-major;
     strided gather -> PV window-major [kw, (slot, head, hd)].
  6. Incremental (per 128-window group, overlapping phase 1 tail): win_tok
     +bv, LayerNorm moments via ones-matmuls, exact GELU, pq/pk projections.
  7. PSA restructured: raw exp-scores kept unnormalized; denominators via
     N=1 matmuls against a ones column (per-partition 1/den on DVE); pout
     window-major [wq, (slot,hd)] at M=128 (half the PE rows of the
     feature-major form); z = pout*recip + attn_x in window-major form;
     PE identity-transposes + strided ACT copies build feature-major zT.
  8. out = Z @ Wo + bo' with Z as the stationary operand -> token-major f32
     output, streamed per window-group.
"""

import sys
from contextlib import ExitStack

for _p in ("/opt/trn_rl_repo/concourse", "/opt/trn_rl_repo"):
    if _p not in sys.path:
        sys.path.insert(0, _p)

import numpy as np
import ml_dtypes

import concourse.bass as bass
import concourse.mybir as mybir
import concourse.tile as tile
from concourse import bacc
from concourse.tile import add_dep_helper
from concourse.bass_utils import run_bass_kernel_spmd

BF16 = mybir.dt.bfloat16
F32 = mybir.dt.float32
AF = mybir.ActivationFunctionType
ALU = mybir.AluOpType

B, M, D = 8, 4096, 512
H, HD = 8, 64
WIN = 7
PW = WIN + 1
QNB = 5
QLEN = 3584
WN = M // PW
SCALE = D ** -0.5
EPS = 1e-5
NCHUNK = 8
CH = 512
NG = 4                   # window groups of 128 for phase 2
GW = WN // NG            # 128 windows per group
PERM = [(h % 2) * 4 + h // 2 for h in range(H)]  # head -> DSA psum slot


def build_program():
    nc = bacc.Bacc("TRN2", target_bir_lowering=False, debug=False, num_devices=8)

    t = {}
    t["q_in"] = nc.dram_tensor("q", [M, D], F32, kind="ExternalInput")
    t["k_in"] = nc.dram_tensor("k", [M, D], F32, kind="ExternalInput")
    t["v_in"] = nc.dram_tensor("v", [M, D], F32, kind="ExternalInput")
    for nm in ("wq", "wk", "wv", "wpq", "wpk", "wo"):
        t[nm] = nc.dram_tensor(nm, [D, D], BF16, kind="ExternalInput")
    for nm in ("bq_c", "bk_c", "bpq_c", "bpk_c", "ln_g_c", "ln_b_c", "bv_c"):
        t[nm] = nc.dram_tensor(nm, [128, 4], F32, kind="ExternalInput")
    t["bo_r"] = nc.dram_tensor("bo_r", [1, D], BF16, kind="ExternalInput")
    t["bmask"] = nc.dram_tensor("bmask", [128, 128], BF16, kind="ExternalInput")
    t["ident"] = nc.dram_tensor("ident", [128, 128], BF16, kind="ExternalInput")
    t["out"] = nc.dram_tensor("out", [QLEN, D], F32, kind="ExternalOutput")
    t["axd"] = nc.dram_tensor("axd_s", [M, D], BF16, kind="Internal")
    t["zd"] = nc.dram_tensor("zd_s", [QLEN, D], BF16, kind="Internal")
    t["qb"] = nc.dram_tensor("qb_s", [M, D], BF16, kind="Internal")
    t["kb"] = nc.dram_tensor("kb_s", [M, D], BF16, kind="Internal")
    t["vb"] = nc.dram_tensor("vb_s", [M, D], BF16, kind="Internal")

    with tile.TileContext(nc) as tc:
        _build(nc, tc, t)
    nc.compile()
    return nc


def _build(nc, tc, t):
    qb, kb, vb = t["qb"], t["kb"], t["vb"]
    axd, out = t["axd"], t["out"]
    zd = t["zd"]

    with ExitStack() as octx:
        singles = octx.enter_context(tc.tile_pool(name="singles", bufs=1))

        # phase-1 weights first (needed by the first projections), then the
        # input casts, then everything else so the casts win the DMA engines.
        cast_insts = {"q": [], "k": [], "v": []}

        def issue_casts(lo, hi):
            for nm, srcd, dst in (("q", t["q_in"], qb), ("k", t["k_in"], kb),
                                  ("v", t["v_in"], vb)):
                ci = nc.gpsimd.dma_start(
                    out=dst[lo * CH:hi * CH, :],
                    in_=srcd[lo * CH:hi * CH, :])
                cast_insts[nm].append(((lo, hi), ci))

        issue_casts(0, 1)
        W = {}

        def load_w(nm):
            W[nm] = singles.tile([128, 4, D], BF16, tag=nm, name=f"w_{nm}")
            nc.scalar.dma_start(out=W[nm][:],
                                in_=t[nm].ap().rearrange("(c p) d -> p c d", p=128))

        load_w("wq")
        issue_casts(1, 2)
        load_w("wk")
        load_w("wv")

        bias_cols = {}
        for nm in ("bq_c", "bk_c"):
            bias_cols[nm] = singles.tile([128, 4], F32, tag=nm, name=f"bc_{nm}")
            nc.scalar.dma_start(out=bias_cols[nm][:], in_=t[nm][:, :])
        mask_sb = singles.tile([128, 128], BF16)
        nc.scalar.dma_start(out=mask_sb[:], in_=t["bmask"][:, :])
        ones_row = singles.tile([1, 128], BF16)
        nc.vector.memset(ones_row[:], 1.0)
        ones_col = singles.tile([128, 1], BF16)
        nc.vector.memset(ones_col[:], 1.0)
        ones_full = singles.tile([128, 128], BF16)
        nc.vector.memset(ones_full[:], 1.0)
        eps_sb = singles.tile([128, 1], F32)
        nc.vector.memset(eps_sb[:], EPS)

        issue_casts(2, 4)
        issue_casts(4, 6)
        issue_casts(6, 8)

        axd_writers = {}
        p2a = octx.enter_context(tc.tile_pool(name="p2a", bufs=1))
        axt = p2a.tile([128, 4, M], BF16, tag="axt")

        # ================= phase 2 weights ==============================
        # deferred behind the early input casts so they don't hog the DMA
        # engines during the pipeline ramp
        # Allocated here; DMAs issued mid-phase-1 (see chunk loop, c==3)
        # on the sync queue so they neither hog the DMA engines at startup
        # nor get scheduled into the phase boundary.
        for nm in ("wpq", "wpk", "wo"):
            W[nm] = singles.tile([128, 4, D], BF16, tag=nm, name=f"w_{nm}")
        for nm in ("bpq_c", "bpk_c", "ln_g_c", "ln_b_c", "bv_c"):
            bias_cols[nm] = singles.tile([128, 4], F32, tag=nm, name=f"bc_{nm}")
        bo_sb = singles.tile([128, D], BF16)
        ident_sb = singles.tile([128, 128], BF16)

        def load_p2_consts(gate):
            dis = []
            for nm in ("wpq", "wpk", "wo"):
                dis.append(nc.sync.dma_start(
                    out=W[nm][:],
                    in_=t[nm].ap().rearrange("(c p) d -> p c d", p=128)))
            for nm in ("bpq_c", "bpk_c", "ln_g_c", "ln_b_c", "bv_c"):
                dis.append(nc.sync.dma_start(out=bias_cols[nm][:], in_=t[nm][:, :]))
            dis.append(nc.sync.dma_start(
                out=bo_sb[:],
                in_=t["bo_r"].ap().to_broadcast((128, D))))
            dis.append(nc.sync.dma_start(out=ident_sb[:], in_=t["ident"][:, :]))
            for di in dis:
                add_dep_helper(di.ins, gate.ins,
                               reason="const loads after startup transposes")

        # ================= phase 1 =================
        with ExitStack() as ctx:
            p1 = ctx.enter_context(tc.tile_pool(name="p1", bufs=1))
            kT = p1.tile([128, 4, 3, CH], BF16, tag="kT")        # ring of 3 chunks
            qpT = p1.tile([128, 4, 3, CH], BF16, tag="qpT")      # ring of 3 chunks
            vtm = p1.tile([128, 12, 8, 65], BF16, tag="vtm")     # ring of 12 tiles, 65-col/head
            nc.vector.memset(vtm[:, :, :, 64:65], 1.0)           # ones col for denominators
            qraw = p1.tile([128, 4, M + 4], BF16, tag="qraw")    # full, padded +-2
            nc.vector.memset(qraw[:, :, 0:2], 0.0)
            nc.vector.memset(qraw[:, :, M + 2:M + 4], 0.0)

            xtp = ctx.enter_context(tc.tile_pool(name="xtp", bufs=2))
            ps_proj = ctx.enter_context(tc.tile_pool(name="ps_proj", bufs=2, space="PSUM"))
            ps_st = ctx.enter_context(tc.tile_pool(name="ps_st", bufs=2, space="PSUM"))
            ps_out = ctx.enter_context(tc.tile_pool(name="ps_out", bufs=1, space="PSUM"))
            dsa_sb = ctx.enter_context(tc.tile_pool(name="dsa_sb", bufs=2))
            pool_tmp = ctx.enter_context(tc.tile_pool(name="pool_tmp", bufs=2))
            ax_pool = ctx.enter_context(tc.tile_pool(name="ax_sb", bufs=2))

            xt_gate = {}

            def load_xt(nm, dram, c):
                xt = xtp.tile([128, 4, CH], BF16, tag=f"xt_{nm}", name=f"xt_{nm}_{c}")
                ti = nc.sync.dma_start(out=xt[:],
                                       in_=dram[c * CH:(c + 1) * CH, :],
                                       transpose=True)
                for (lo, hi), ci in cast_insts[nm]:
                    if lo <= c < hi:
                        add_dep_helper(ti.ins, ci.ins,
                                       reason="transpose reads cast output")
                xt_gate[(nm, c)] = ti
                return xt

            def proj_fm_group(xt, wname, bname, dst_fn, j):
                ps = ps_proj.tile([128, CH], F32, tag="proj",
                                  name=f"ps_{wname}_{j}")
                for dk in range(4):
                    nc.tensor.matmul(ps[:], W[wname][:, dk, j * 128:(j + 1) * 128],
                                     xt[:, dk, :], start=(dk == 0), stop=(dk == 3))
                nc.scalar.activation(dst_fn(j), ps[:], AF.Identity,
                                     bias=bias_cols[bname][:, j:j + 1], scale=1.0)

            def proj_v_group(xt, c, tt):
                ps = ps_proj.tile([128, D], F32, tag="proj", name=f"ps_v_{tt}")
                for dk in range(4):
                    nc.tensor.matmul(ps[:], xt[:, dk, tt * 128:(tt + 1) * 128],
                                     W["wv"][:, dk, :], start=(dk == 0),
                                     stop=(dk == 3), skip_group_check=True)
                nc.scalar.copy(vtm[:, (c * 4 + tt) % 12, :, 0:64],
                               ps[:].rearrange("p (h d) -> p h d", h=H))

            def pool_chunk(c):
                base = c * CH
                ta = pool_tmp.tile([128, 4, CH + 2], BF16, tag="ta")
                nc.vector.tensor_add(ta[:], qraw[:, :, base:base + CH + 2],
                                     qraw[:, :, base + 1:base + CH + 3])
                tb = pool_tmp.tile([128, 4, CH], BF16, tag="tb")
                nc.vector.tensor_add(tb[:], ta[:, :, 0:CH], ta[:, :, 2:CH + 2])
                nc.vector.tensor_add(qpT[:, :, c % 3, :], tb[:],
                                     qraw[:, :, base + 4:base + CH + 4])

            def dsa_scores(c, lt):
                """MM1 + exp + mask for tile lt of chunk c -> masked sbuf tile."""
                st = ps_st.tile([128, 8, 128], F32, tag="st", name=f"st_{c}_{lt}")
                for h in range(H):
                    hp = PERM[h]
                    base = (h % 2) * 64
                    lhsT = kT[base:base + 64, h // 2, c % 3, lt * 128:(lt + 1) * 128]
                    rhs = qpT[base:base + 64, h // 2, c % 3, lt * 128:(lt + 1) * 128]
                    nc.tensor.matmul(st[:, hp, :], lhsT, rhs, start=True, stop=True,
                                     skip_group_check=True)
                expS = dsa_sb.tile([128, 8, 128], BF16, tag="expS",
                                   name=f"expS_{c}_{lt}")
                nc.scalar.activation(expS[:], st[:], AF.Exp, scale=SCALE / QNB)
                masked = dsa_sb.tile([128, 8, 128], BF16, tag="masked",
                                     name=f"masked_{c}_{lt}")
                nc.vector.tensor_mul(masked[:], expS[:],
                                     mask_sb[:].unsqueeze(1).to_broadcast((128, 8, 128)))
                return masked

            def dsa_out(c, lt, masked, ax_out):
                """attn@V with ones-col denominators, then normalize."""
                outp = ps_out.tile([128, 2, 512], F32, tag="outp",
                                   name=f"outp_{c}_{lt}")
                for h in range(H):
                    hp = PERM[h]
                    nc.tensor.matmul(outp[:, h // 4, (h % 4) * 65:(h % 4) * 65 + 65],
                                     masked[:, hp, :],
                                     vtm[:, (c * 4 + lt) % 12, h, :],
                                     start=True, stop=True, skip_group_check=True)
                recip = dsa_sb.tile([128, 2, 4], F32, tag="recip",
                                    name=f"recip_{c}_{lt}")
                den_view = bass.AP(outp.tensor, outp[:].offset + 64,
                                   [outp[:].ap[0], [512, 2], [65, 4]])
                nc.vector.reciprocal(recip[:], den_view)
                av_view = bass.AP(outp.tensor, outp[:].offset,
                                  [outp[:].ap[0], [512, 2], [65, 4], [1, 64]])
                nc.vector.tensor_mul(
                    ax_out.rearrange("p (a b d) -> p a b d", a=2, b=4),
                    av_view,
                    recip[:].unsqueeze(3).to_broadcast((128, 2, 4, 64)))

            def dsa_group_list(c, ax):
                masked = {}
                g = []
                g.append(lambda: masked.__setitem__(0, dsa_scores(c, 0)))
                g.append(lambda: masked.__setitem__(1, dsa_scores(c, 1)))
                g.append(lambda: dsa_out(c, 0, masked.pop(0), ax[:, 0, :]))
                g.append(lambda: masked.__setitem__(2, dsa_scores(c, 2)))
                g.append(lambda: dsa_out(c, 1, masked.pop(1), ax[:, 1, :]))
                g.append(lambda: masked.__setitem__(3, dsa_scores(c, 3)))
                g.append(lambda: dsa_out(c, 2, masked.pop(2), ax[:, 2, :]))
                g.append(lambda: dsa_out(c, 3, masked.pop(3), ax[:, 3, :]))
                return g

            def store_ax(c, ax):
                dst = axd.ap().rearrange("(cc lt p) d -> cc p lt d", lt=4, p=128)[c]
                wi = nc.gpsimd.dma_start(out=dst, in_=ax[:])
                axd_writers[c] = wi

            def issue_axt(c):
                ti = nc.sync.dma_start(
                    out=axt[:, :, c * CH:(c + 1) * CH],
                    in_=axd[c * CH:(c + 1) * CH, :],
                    transpose=True)
                add_dep_helper(ti.ins, axd_writers[c].ins,
                               reason="axt transpose reads axd chunk")

            for c in range(NCHUNK + 2):
                pgroups = []
                if c < NCHUNK:
                    qxt = load_xt("q", qb, c)
                    kxt = load_xt("k", kb, c)
                    vxt = load_xt("v", vb, c)
                    if c == 3:
                        load_p2_consts(xt_gate[("q", 2)])
                    for j in range(4):
                        pgroups.append(lambda j=j, x=qxt, c=c: proj_fm_group(
                            x, "wq", "bq_c",
                            lambda jj, c=c: qraw[:, jj, 2 + c * CH:2 + (c + 1) * CH], j))
                    for j in range(4):
                        pgroups.append(lambda j=j, x=kxt, c=c: proj_fm_group(
                            x, "wk", "bk_c", lambda jj, c=c: kT[:, jj, c % 3, :], j))
                    for tt in range(4):
                        pgroups.append(lambda tt=tt, x=vxt, c=c: proj_v_group(x, c, tt))
                dgroups = []
                ax = None
                if c >= 2:
                    ax = ax_pool.tile([128, 4, D], BF16, tag="ax", name=f"ax_{c - 2}")
                    dgroups = dsa_group_list(c - 2, ax)
                # weave: spread D groups evenly through the P stream;
                # pool(c-1) after the 4 Q-projection groups
                npg, ndg = len(pgroups), len(dgroups)
                dpos = {int(round(k * npg / ndg)): k for k in range(ndg)} \
                    if npg else {}
                for i in range(max(npg, 1)):
                    if i < npg:
                        pgroups[i]()
                    if i == 3 and 1 <= c <= NCHUNK:
                        pool_chunk(c - 1)
                    if i in dpos:
                        dgroups[dpos[i]]()
                if not pgroups:
                    if 1 <= c <= NCHUNK:
                        pool_chunk(c - 1)
                    for g in dgroups:
                        g()
                if ax is not None:
                    store_ax(c - 2, ax)


        # ================= phase 2 =================
        with ExitStack() as ctx:
            p2 = ctx.enter_context(tc.tile_pool(name="p2", bufs=1))

            pv = p2.tile([128, 4, WIN, D], BF16, tag="pv")
            wtn = p2.tile([128, 4, WN], BF16, tag="wtn")
            pqT = p2.tile([128, 4, WN], BF16, tag="pqT")
            pkT = p2.tile([128, 4, WN], BF16, tag="pkT")
            esA = p2.tile([128, H, 4, WN], BF16, tag="esA")
            zt = p2.tile([128, 4, QLEN], BF16, tag="zt")
            recip_sb = p2.tile([128, H, 4], F32, tag="recips")

            # issue axt transposes + pv gathers in dependency-arrival order
            srcv = axd.ap().rearrange("(cc p w) d -> cc p w d", p=128, w=PW)
            for c in range(NCHUNK):
                issue_axt(c)
                if c % 2 == 1:
                    cc = c // 2
                    gi = nc.sync.dma_start(out=pv[:, cc, :, :], in_=srcv[cc, :, 1:PW, :])
                    add_dep_helper(gi.ins, axd_writers[2 * cc].ins, reason="pv gather")
                    add_dep_helper(gi.ins, axd_writers[2 * cc + 1].ins, reason="pv gather")

            # ---- win_tok (+bv) LN + GELU + pq/pk, per 128-window group ----
            with ExitStack() as lctx:
                ps_ln = lctx.enter_context(
                    tc.tile_pool(name="ps_ln", bufs=3, space="PSUM"))
                lnp = lctx.enter_context(tc.tile_pool(name="lnp", bufs=2))

                lnA = {}

                def ln_phase_a(g):
                    """Moments + rstd; ACT funcs all within one table set
                    (Identity/Square/Copy/Sqrt)."""
                    wt_g = axt[:, :, g * GW * PW:(g + 1) * GW * PW:PW]
                    wtb = lnp.tile([128, 4, GW], BF16, tag="wtb", bufs=4,
                                   name=f"wtb_{g}")
                    for j in range(4):
                        nc.scalar.activation(wtb[:, j, :], wt_g[:, j, :],
                                             AF.Identity,
                                             bias=bias_cols["bv_c"][:, j:j + 1],
                                             scale=1.0)
                    wsq = lnp.tile([128, 4, GW], BF16, tag="wsq", name=f"wsq_{g}")
                    nc.scalar.activation(wsq[:], wtb[:], AF.Square)
                    ps_mu = ps_ln.tile([128, GW], F32, tag="psln", name=f"psmu_{g}")
                    ps_var = ps_ln.tile([128, GW], F32, tag="psln", name=f"psvar_{g}")
                    for j in range(4):
                        nc.tensor.matmul(ps_mu[:], ones_full[:], wtb[:, j, :],
                                         start=(j == 0), stop=(j == 3),
                                         skip_group_check=True)
                        nc.tensor.matmul(ps_var[:], ones_full[:], wsq[:, j, :],
                                         start=(j == 0), stop=(j == 3),
                                         skip_group_check=True)
                    mu = lnp.tile([128, GW], F32, tag="mu", bufs=4, name=f"mu_{g}")
                    nc.scalar.mul(mu[:], ps_mu[:], 1.0 / D)
                    ex2 = lnp.tile([128, GW], F32, tag="ex2", bufs=1,
                                   name=f"ex2_{g}")
                    nc.scalar.mul(ex2[:], ps_var[:], 1.0 / D)
                    var = lnp.tile([128, GW], F32, tag="var", bufs=1,
                                   name=f"var_{g}")
                    nc.vector.tensor_mul(var[:], mu[:], mu[:])
                    nc.vector.tensor_sub(var[:], ex2[:], var[:])
                    sd = lnp.tile([128, GW], F32, tag="sd", bufs=1, name=f"sd_{g}")
                    nc.scalar.activation(sd[:], var[:], AF.Sqrt, bias=eps_sb[:])
                    rstd = lnp.tile([128, GW], F32, tag="rstd", bufs=4,
                                    name=f"rstd_{g}")
                    nc.vector.reciprocal(rstd[:], sd[:])
                    lnA[g] = (wtb, mu, rstd)

                def ln_phase_b(g):
                    """GELU + pq/pk projections (Gelu/Identity table set)."""
                    gs = g * GW
                    wtb, mu, rstd = lnA.pop(g)
                    for j in range(4):
                        tmp = lnp.tile([128, GW], F32, tag="lnt", name=f"lnt_{g}_{j}")
                        nc.vector.tensor_sub(tmp[:], wtb[:, j, :], mu[:])
                        nc.vector.tensor_mul(tmp[:], tmp[:], rstd[:])
                        nc.scalar.activation(wtn[:, j, gs:gs + GW], tmp[:],
                                             AF.Gelu,
                                             bias=bias_cols["ln_b_c"][:, j:j + 1],
                                             scale=bias_cols["ln_g_c"][:, j:j + 1])
                    for dst, wname, bname in ((pqT, "wpq", "bpq_c"),
                                              (pkT, "wpk", "bpk_c")):
                        for j in range(4):
                            ps = ps_ln.tile([128, GW], F32, tag="psln",
                                            name=f"pp_{wname}_{g}_{j}")
                            for dk in range(4):
                                nc.tensor.matmul(
                                    ps[:], W[wname][:, dk, j * 128:(j + 1) * 128],
                                    wtn[:, dk, gs:gs + GW],
                                    start=(dk == 0), stop=(dk == 3))
                            nc.vector.tensor_scalar_add(
                                dst[:, j, gs:gs + GW], ps[:],
                                bias_cols[bname][:, j:j + 1])

                # A0..A2 then B0..B2 (one Sqrt->Gelu table switch), then the
                # last group's A3+B3 pair on the critical path (one more
                # switch pair).
                for g in range(NG - 1):
                    ln_phase_a(g)
                for g in range(NG - 1):
                    ln_phase_b(g)
                ln_phase_a(NG - 1)
                ln_phase_b(NG - 1)

            # ---- PSA: raw exp scores; den via N=1 matmuls; window-major pout
            with ExitStack() as pctx:
                # PSUM budget (8 banks): es/fin share slots (disjoint
                # lifetimes, same shape) 2 + po 2 + ztps 2 + den 1 = 7.
                ps_es = pctx.enter_context(
                    tc.tile_pool(name="ps_es", bufs=2, space="PSUM"))
                ps_po = pctx.enter_context(
                    tc.tile_pool(name="ps_po", bufs=2, space="PSUM"))
                ps_ztden = pctx.enter_context(
                    tc.tile_pool(name="ps_ztden", bufs=2, space="PSUM"))
                ps_fin = ps_es
                zwp = pctx.enter_context(tc.tile_pool(name="zwp", bufs=3))
                ztp = pctx.enter_context(tc.tile_pool(name="ztp", bufs=2))
                osb = pctx.enter_context(tc.tile_pool(name="osb", bufs=6))

                def psa_scores(h):
                    base = (h % 2) * 64
                    for cp in range(2):
                        ps = ps_es.tile([128, 2, WN], F32, tag="es",
                                        name=f"es_{h}_{cp}")
                        for k in range(2):
                            cc = cp * 2 + k
                            nc.tensor.matmul(
                                ps[:, k, :], pkT[base:base + 64, h // 2,
                                                 cc * 128:(cc + 1) * 128],
                                pqT[base:base + 64, h // 2, :],
                                start=True, stop=True, skip_group_check=True)
                        nc.scalar.activation(esA[:, h, 2 * cp:2 * cp + 2, :],
                                             ps[:], AF.Exp, scale=SCALE)

                def pout_one(h, qt, zwin):
                    # [pout | den] share one PSUM bank: cols 0:448 accumulate
                    # raw-exp attn @ pv, col 448 accumulates the softmax
                    # denominator against a ones column.
                    po = ps_po.tile([128, WIN * HD + 1], F32, tag="po",
                                    name=f"po_{h}_{qt}")
                    pov = po[:, 0:WIN * HD].rearrange("p (i d) -> p i d", i=WIN)
                    for cc in range(4):
                        nc.tensor.matmul(
                            pov, esA[:, h, cc, qt * 128:(qt + 1) * 128],
                            pv[:, cc, :, h * 64:(h + 1) * 64],
                            start=(cc == 0), stop=(cc == 3),
                            skip_group_check=True)
                    for cc in range(4):
                        nc.tensor.matmul(
                            po[:, WIN * HD:WIN * HD + 1],
                            esA[:, h, cc, qt * 128:(qt + 1) * 128],
                            ones_col[:], start=(cc == 0), stop=(cc == 3),
                            skip_group_check=True)
                    rc = recip_sb[:, h, qt:qt + 1]
                    nc.vector.reciprocal(rc, po[:, WIN * HD:WIN * HD + 1])
                    ztmp = ztp.tile([128, WIN, HD], BF16, tag="ztmp",
                                    name=f"ztmp_{h}_{qt}")
                    nc.vector.tensor_scalar_mul(ztmp[:], pov, rc)
                    nc.vector.tensor_add(zwin[:, :, h * 64:(h + 1) * 64], ztmp[:],
                                         pv[:, qt, :, h * 64:(h + 1) * 64])

                def ztrans_one(qt, ii, zwin):
                    """Transpose payload slots ii..ii+1 (or just ii at the
                    tail) of group qt into feature-major zt."""
                    ni = min(2, WIN - ii)
                    zt_ps = ps_ztden.tile([128, 4, 2, 128], BF16, tag="ztps",
                                          name=f"ztps_{qt}_{ii}")
                    for di in range(ni):
                        for fg in range(4):
                            nc.tensor.transpose(
                                zt_ps[:, fg, di, :],
                                zwin[:, ii + di, fg * 128:(fg + 1) * 128],
                                ident_sb[:])
                    base = qt * GW * WIN
                    dst = zt[:, :, base + ii:base + GW * WIN:WIN]
                    dst = bass.AP(dst.tensor, dst.offset,
                                  [dst.ap[0], dst.ap[1], [1, ni], [WIN, 128]])
                    src_ap = zt_ps[:, :, 0:ni, :]
                    nc.scalar.copy(dst, src_ap)

                osb_tiles = {}

                def fin_one(tt):
                    psf = ps_fin.tile([128, 2, WN], F32, tag="es",
                                      name=f"fin_{tt}")
                    ps = psf[:, 0, :]
                    for dk in range(4):
                        nc.tensor.matmul(ps, zt[:, dk, tt * 128:(tt + 1) * 128],
                                         W["wo"][:, dk, :], start=(dk == 0),
                                         stop=(dk == 3), skip_group_check=True)
                    ot = osb.tile([128, D], F32, tag="osb", name=f"osb_{tt}")
                    nc.vector.tensor_add(ot[:], ps, bo_sb[:])
                    outv = out.ap().rearrange("(tt p) d -> tt p d", p=128)
                    nc.sync.dma_start(out=outv[tt], in_=ot[:])

                # head-outer pipeline: as soon as head h's exp-scores are
                # done, its denominators and all four pout groups flow; the
                # transposes + final projections drain afterwards per group.
                for h in range(H):
                    psa_scores(h)

                zw = {}
                prev = None

                def tail_items(qt):
                    items = []
                    zwin_p = zw[qt]
                    for ii in range(0, WIN, 2):
                        items.append(lambda ii=ii: ztrans_one(qt, ii, zwin_p))
                    for j in range(WIN):
                        items.append(lambda j=j: fin_one(qt * WIN + j))
                    return items

                for qt in range(NG):
                    zw[qt] = zwp.tile([128, WIN, D], BF16, tag="zwin",
                                      name=f"zwin_{qt}")
                    titems = tail_items(prev) if prev is not None else []
                    ti = 0
                    for h in range(H):
                        pout_one(h, qt, zw[qt])
                        for _ in range(2):
                            if ti < len(titems) and h >= 3 and \
                                    ((h - 3) * 16) // H >= ti:
                                titems[ti]()
                                ti += 1
                    while ti < len(titems):
                        titems[ti]()
                        ti += 1
                    if prev is not None:
                        zw.pop(prev)
                    prev = qt
                for it in tail_items(prev):
                    it()


_NC_CACHE = None


def _get_program():
    global _NC_CACHE
    if _NC_CACHE is None:
        _NC_CACHE = build_program()
    return _NC_CACHE


def _host_consts(Wk, bk, Wv, bv, Wq, bq, ln_g, ln_b, Wpq, bpq, Wpk, bpk, Wo, bo):
    bf = ml_dtypes.bfloat16
    col = lambda b: np.asarray(b, np.float32).reshape(4, 128).T.copy()
    bo2 = np.asarray(bo, np.float32) + 2.0 * (
        np.asarray(bv, np.float32) @ np.asarray(Wo, np.float32))
    consts = {
        "wq": np.asarray(Wq, np.float32).astype(bf),
        "wk": np.asarray(Wk, np.float32).astype(bf),
        "wv": np.asarray(Wv, np.float32).astype(bf),
        "wpq": np.asarray(Wpq, np.float32).astype(bf),
        "wpk": np.asarray(Wpk, np.float32).astype(bf),
        "wo": np.asarray(Wo, np.float32).astype(bf),
        "bq_c": col(bq), "bk_c": col(bk),
        "bpq_c": col(bpq), "bpk_c": col(bpk),
        "ln_g_c": col(ln_g), "ln_b_c": col(ln_b),
        "bv_c": col(bv),
        "bo_r": bo2.reshape(1, D).astype(bf),
        "ident": np.eye(128, dtype=np.float32).astype(bf),
    }
    m = np.zeros((128, 128), np.float32)
    for g in range(16):
        m[g * PW:(g + 1) * PW, g * PW:(g + 1) * PW] = 1.0
    consts["bmask"] = m.astype(bf)
    return consts


def kernel(k, v, q, query_len, Wk, bk, Wv, bv, Wq, bq, ln_g, ln_b,
           Wpq, bpq, Wpk, bpk, Wo, bo):
    nc = _get_program()
    consts = _host_consts(Wk, bk, Wv, bv, Wq, bq, ln_g, ln_b,
                          Wpq, bpq, Wpk, bpk, Wo, bo)
    k = np.asarray(k, np.float32)
    v = np.asarray(v, np.float32)
    q = np.asarray(q, np.float32)
    in_maps = []
    for b in range(B):
        m = {"q": np.ascontiguousarray(q[b]), "k": np.ascontiguousarray(k[b]),
             "v": np.ascontiguousarray(v[b])}
        m.update(consts)
        in_maps.append(m)
    res = run_bass_kernel_spmd(nc, in_maps, core_ids=list(range(B)))
    return np.stack([res.results[b]["out"] for b in range(B)], axis=0)


if __name__ == "__main__":
    nc = build_program()
    print("program built ok")
